# revision 8
# baseline (speedup 1.0000x reference)
# GCN (2-layer GCNConv + BatchNorm + ReLU + global mean pool) on 8 TRN2 NeuronCores.
#
# Math (reference):
#   deg[v]  = in-degree incl. self-loop;  dinv = deg^-1/2
#   layer(x, W, b): h = D^-1/2 (A+I) D^-1/2 (x W) + b
#   h1 = relu(batchnorm(layer1));  h2 = layer2(h1);  out = segment_mean(h2, batch)
#
# Sharding (v2 — source-partitioned edges + ReduceScatter):
#   Core k owns nodes [k*SL, (k+1)*SL) and all edges whose SRC falls in that
#   range (plus its own self-loop edges).  Layer 1:
#     * xs = dinv * x (own slice) -> local gather table (DRAM), so the edge
#       gather needs NO collective at all.
#     * per-edge: dma_gather xs[src] rows from the local table, then
#       dma_scatter_add into a full-size [8*SLP, 64] DRAM accumulator at the
#       global dst row.  Self-loops ride along as ordinary (v, v) edges.
#     * one ReduceScatter (add) hands each core the reduced rows of its own
#       slice — far cheaper than all-gathering the full table since collective
#       cost tracks the OUTPUT size.
#   BatchNorm stats via an accumulated A^T[A|1] matmul + algebraic reduction
#   + one tiny [64,65] all-reduce (overlapped with the W1 matmul work).
#   Layer 2 + pooling collapse into dense matmuls: since mean-pool
#   P (and the outer D^-1/2) are linear, out = sum_k (R_k @ xs2_k) W2 + b2
#   with R_k[g, u] = sum_{edges u->w owned by k} P[g,w] dinv_w  (+ self term),
#   built on the host from pure index data.  No second edge phase, no second
#   table, no second big collective — just 50 accumulating [128,64]x[128,64]
#   matmuls and a tiny [64,64] all-reduce.
#
# dma_scatter_add races (loses updates) for duplicate dst rows within one
# instruction, so edges are packed into instruction "bins" with unique dst
# rows per bin via rotation binning: occurrence o of dst row d goes to bin
# (d + o) % nbins.  Scatter row indices are signed int16, so bins are split
# into lo (row < 32768) / hi buckets scattered at different out_ap bases.
# Pad slots gather row 0 and scatter into a dead pad row (junk, multiplied by
# dinv=0 downstream).
#
# Host-side preprocessing uses only index data (edge_index, batch): degree
# computation, edge partitioning/binning, the R_k pooling matrices.  Feature
# data is never touched on the host.

import os

import numpy as np

N_NODES = 50000
N_EDGES = 800000
D = 64
NCORES = 8
NUM_GRAPHS = 64
BN_EPS = 1e-5
SPLIT = 32768  # int16 scatter index limit


class Cfg:
    def __init__(self, n, sl):
        self.N = n                    # total nodes
        self.SL = sl                  # owned nodes per core
        self.SLP = ((sl + 127) // 128) * 128   # padded slice rows
        assert self.SL < self.SLP, "need a dead pad row per slice"
        self.NT = self.SLP // 128     # 128-row node tiles per slice
        self.NG = NCORES * self.SLP   # padded global accumulator rows
        self.CAP = 7680               # max slots per gather/scatter instruction
        # per-instruction (bucket, slot count); filled by prepare_inputs
        self.seg = []                 # list of (bucket 0/1, padded slot count)


LAST_EXEC_TIME_NS = None
_NC_CACHE = {}
_LAST_IN_MAPS = None


def build(cfg):
    import concourse.mybir as mybir
    import concourse.tile as tile
    from concourse import bacc
    from concourse.masks import make_identity

    f32 = mybir.dt.float32
    i16 = mybir.dt.int16
    SL, SLP, NT, NG = cfg.SL, cfg.SLP, cfg.NT, cfg.NG
    NN = float(cfg.N)
    RG = [list(range(NCORES))]
    segs = cfg.seg
    tot_s = sum(c for _, c in segs)

    nc = bacc.Bacc(
        "TRN2", target_bir_lowering=False, debug=False, num_devices=NCORES
    )

    # --- external inputs (per-core values supplied via in_maps) ---
    xsl = nc.declare_dram_parameter("xsl", [SLP, D], f32, isOutput=False)
    dinv_in = nc.declare_dram_parameter("dinv_in", [128, NT], f32, isOutput=False)
    gidx_d = nc.declare_dram_parameter("gidx", [128, tot_s // 16], i16, isOutput=False)
    sidx_d = nc.declare_dram_parameter("sidx", [128, tot_s // 16], i16, isOutput=False)
    rkt_d = nc.declare_dram_parameter("rkt", [SLP, D], f32, isOutput=False)
    p1_d = nc.declare_dram_parameter("p1", [1, NUM_GRAPHS], f32, isOutput=False)
    w1_d = nc.declare_dram_parameter("w1", [D, D], f32, isOutput=False)
    b1_d = nc.declare_dram_parameter("b1", [D, 1], f32, isOutput=False)
    ga_d = nc.declare_dram_parameter("ga", [D, 1], f32, isOutput=False)
    be_d = nc.declare_dram_parameter("be", [D, 1], f32, isOutput=False)
    w2_d = nc.declare_dram_parameter("w2", [D, D], f32, isOutput=False)
    b2_d = nc.declare_dram_parameter("b2", [1, D], f32, isOutput=False)
    out_d = nc.declare_dram_parameter("out", [NUM_GRAPHS, D], f32, isOutput=True)

    # --- internal DRAM ---
    table1 = nc.dram_tensor("table1", [SLP, D], f32)
    acc = nc.dram_tensor("acc", [NG, D], f32)
    rs_out = nc.dram_tensor("rs_out", [SLP, D], f32)
    ars_in = nc.dram_tensor("ars_in", [D, D + 1], f32)
    ars_out = nc.dram_tensor("ars_out", [D, D + 1], f32, addr_space="Shared")
    aro_in = nc.dram_tensor("aro_in", [NUM_GRAPHS, D], f32)
    aro_out = nc.dram_tensor("aro_out", [NUM_GRAPHS, D], f32, addr_space="Shared")

    with tile.TileContext(nc) as tc:
        with (
            tc.tile_pool(name="const", bufs=1) as const,
            tc.tile_pool(name="persist", bufs=1) as persist,
            tc.tile_pool(name="work", bufs=2) as work,
            tc.tile_pool(name="msgp", bufs=3) as msgp,
            tc.tile_pool(name="spsum", bufs=1, space="PSUM") as spsum,
            tc.tile_pool(name="wpsum", bufs=2, space="PSUM") as wpsum,
        ):
            ablate = os.environ.get("GNN_ABLATE", "")

            # --- zero the DRAM accumulator (4 chunks on idle engines) ---
            zt = persist.tile([128, 6400], f32, name="zt")
            nc.vector.memset(zt[:], 0.0)
            acc_flat = acc[:, :].rearrange("n d -> (n d)")
            CHUNK = NG * D // 4
            for i, eng in enumerate((nc.scalar, nc.scalar, nc.sync, nc.sync)):
                ap = acc_flat.rearrange("(c p x) -> c p x", c=4, p=128)[i]
                eng.dma_start(out=ap, in_=zt[:, : CHUNK // 128])

            # --- constants into SBUF ---
            w1s = const.tile([D, D], f32)
            nc.sync.dma_start(out=w1s[:], in_=w1_d[:, :])
            w2s = const.tile([D, D], f32)
            nc.sync.dma_start(out=w2s[:], in_=w2_d[:, :])
            b1c = const.tile([D, 1], f32)
            nc.sync.dma_start(out=b1c[:], in_=b1_d[:, :])
            gac = const.tile([D, 1], f32)
            nc.sync.dma_start(out=gac[:], in_=ga_d[:, :])
            bec = const.tile([D, 1], f32)
            nc.sync.dma_start(out=bec[:], in_=be_d[:, :])
            b2r = const.tile([1, D], f32)
            nc.sync.dma_start(out=b2r[:], in_=b2_d[:, :])
            p1s = const.tile([1, NUM_GRAPHS], f32)
            nc.sync.dma_start(out=p1s[:], in_=p1_d[:, :])
            dinvs = const.tile([128, NT], f32)
            nc.sync.dma_start(out=dinvs[:], in_=dinv_in[:, :])
            ident = const.tile([128, 128], f32)
            make_identity(nc, ident[:])
            ones64 = const.tile([D, 1], f32)
            nc.vector.memset(ones64[:], 1.0)
            epsc = const.tile([D, 1], f32)
            nc.vector.memset(epsc[:], BN_EPS)

            # --- edge index tiles ---
            gidx_t = persist.tile([128, tot_s // 16], i16, name="gidx_t")
            nc.scalar.dma_start(out=gidx_t[:], in_=gidx_d[:, :])
            sidx_t = persist.tile([128, tot_s // 16], i16, name="sidx_t")
            nc.scalar.dma_start(out=sidx_t[:], in_=sidx_d[:, :])

            # --- phase A: xs = dinv * x -> local gather table ---
            xs_t = work.tile([128, NT, D], f32, tag="big", name="xs_t")
            nc.sync.dma_start(
                out=xs_t[:], in_=xsl[:, :].rearrange("(g p) d -> p g d", p=128)
            )
            dinv_b = dinvs[:, :].rearrange("p (g o) -> p g o", o=1).to_broadcast(
                [128, NT, D]
            )
            nc.vector.tensor_tensor(
                out=xs_t[:], in0=xs_t[:], in1=dinv_b, op=mybir.AluOpType.mult
            )
            nc.sync.dma_start(
                out=table1[:, :].rearrange("(g p) d -> p g d", p=128), in_=xs_t[:]
            )

            # --- R_k^T for layer 2 (loaded during the edge phase) ---
            rkt_t = persist.tile([128, NT, D], f32, name="rkt_t")
            nc.sync.dma_start(
                out=rkt_t[:], in_=rkt_d[:, :].rearrange("(g p) d -> p g d", p=128)
            )

            # --- phase B: layer-1 edges (gather from local table, scatter-add
            #     into the global accumulator) ---
            if "noedge" not in ablate:
                off = 0
                for bkt, cnt in segs:
                    msg = msgp.tile([128, cfg.CAP // 128, D], f32, tag="msg",
                                    name="msg")
                    nc.gpsimd.dma_gather(
                        out_ap=msg[:, : cnt // 128, :],
                        in_ap=table1[0:SLP, :],
                        idxs_ap=gidx_t[:, off : off + cnt // 16],
                        num_idxs=cnt, num_idxs_reg=cnt, elem_size=D,
                        single_packet=False, queue_num=0,
                    )
                    base = 0 if bkt == 0 else SPLIT
                    span = SPLIT if bkt == 0 else NG - SPLIT
                    nc.gpsimd.dma_scatter_add(
                        acc[base : base + span, :],
                        msg[:, : cnt // 128, :],
                        sidx_t[:, off : off + cnt // 16],
                        cnt, cnt, D,
                        single_packet=False, queue_num=0,
                    )
                    off += cnt // 16

            # --- phase C: ReduceScatter -> own reduced slice ---
            def do_cc(kind, op, ins_ap, outs_ap):
                if "nocc" in ablate:
                    nc.sync.dma_start(
                        out=outs_ap, in_=ins_ap[0 : outs_ap.shape[0], :]
                    )
                    return
                nc.gpsimd.collective_compute(
                    kind, op, replica_groups=RG, ins=[ins_ap], outs=[outs_ap],
                )

            do_cc("ReduceScatter", mybir.AluOpType.add, acc[:, :], rs_out[:, :])

            # --- phase D: dense layer-1 + BN stats ---
            import concourse.mybir as mb

            z_t = work.tile([128, NT, D], f32, tag="big", name="z_t")
            nc.sync.dma_start(
                out=z_t[:], in_=rs_out[:, :].rearrange("(g p) d -> p g d", p=128)
            )
            aggs = persist.tile([128, NT, D + 1], f32, name="aggs")
            nc.vector.memset(aggs[:, :, D : D + 1], 1.0)
            nc.vector.tensor_tensor(
                out=aggs[:, :, :D], in0=z_t[:], in1=dinv_b, op=mybir.AluOpType.mult
            )

            stats_ps = spsum.tile([D, D + 1], f32, name="stats_ps")
            hT_big = persist.tile([D, NT * 128], f32, name="hT_big")
            ND = NT if "noD" not in ablate else 1
            for b in range(ND):
                nc.tensor.matmul(
                    out=stats_ps[:], lhsT=aggs[:, b, :D], rhs=aggs[:, b, :],
                    start=(b == 0), stop=(b == ND - 1),
                )
            # stats all-reduce launched before the transposes/W1 matmuls so the
            # collective overlaps with PE work
            stats_sb = persist.tile([D, D + 1], f32, name="stats_sb")
            nc.vector.tensor_copy(out=stats_sb[:], in_=stats_ps[:])
            nc.sync.dma_start(out=ars_in[:, :], in_=stats_sb[:])
            do_cc("AllReduce", mybir.AluOpType.add, ars_in[:, :], ars_out[:, :])

            for b0 in range(0, ND, 4):
                bn = min(4, ND - b0)
                tp_ps = wpsum.tile([D, 512], f32, tag="ps_a", name="tp_ps")
                for j in range(bn):
                    b = b0 + j
                    nc.tensor.transpose(
                        out=tp_ps[:, j * 128 : (j + 1) * 128],
                        in_=aggs[:, b, :D], identity=ident[:],
                    )
                aggsT = work.tile([D, 512], f32, tag="aggsT", name="aggsT", bufs=2)
                nc.vector.tensor_copy(out=aggsT[:, : bn * 128], in_=tp_ps[:, : bn * 128])
                hT_ps = wpsum.tile([D, 512], f32, tag="ps_b", name="hT_ps")
                nc.tensor.matmul(
                    out=hT_ps[:, : bn * 128], lhsT=w1s[:], rhs=aggsT[:, : bn * 128],
                    start=True, stop=True,
                )
                nc.vector.tensor_copy(
                    out=hT_big[:, b0 * 128 : (b0 + bn) * 128],
                    in_=hT_ps[:, : bn * 128],
                )

            st = persist.tile([D, D + 1], f32, name="st")
            nc.sync.dma_start(out=st[:], in_=ars_out[:, :])

            # --- phase E: BN scalar algebra ---
            q_ps = wpsum.tile([D, 1], f32, tag="ps_a", name="q_ps")
            nc.tensor.matmul(out=q_ps[:], lhsT=w1s[:], rhs=st[:, D : D + 1], start=True, stop=True)
            mu = persist.tile([D, 1], f32, name="mu")
            nc.vector.tensor_scalar(
                out=mu[:], in0=q_ps[:], scalar1=1.0 / NN, scalar2=b1c[:],
                op0=mybir.AluOpType.mult, op1=mybir.AluOpType.add,
            )
            t1_ps = wpsum.tile([D, D], f32, tag="ps_b", name="t1_ps")
            nc.tensor.matmul(out=t1_ps[:], lhsT=st[:, :D], rhs=w1s[:], start=True, stop=True)
            m_sb = work.tile([D, D], f32, tag="m_sb", name="m_sb")
            nc.vector.tensor_tensor(out=m_sb[:], in0=w1s[:], in1=t1_ps[:], op=mybir.AluOpType.mult)
            d_ps = wpsum.tile([D, 1], f32, tag="ps_b", name="d_ps")
            nc.tensor.matmul(out=d_ps[:], lhsT=m_sb[:], rhs=ones64[:], start=True, stop=True)

            var = persist.tile([D, 1], f32, name="var")
            nc.vector.tensor_scalar_mul(out=var[:], in0=d_ps[:], scalar1=1.0 / NN)
            t2 = work.tile([D, 1], f32, tag="t2", name="t2")
            nc.vector.tensor_scalar_mul(out=t2[:], in0=q_ps[:], scalar1=2.0 / NN)
            nc.vector.tensor_tensor(out=t2[:], in0=t2[:], in1=b1c[:], op=mybir.AluOpType.mult)
            nc.vector.tensor_tensor(out=var[:], in0=var[:], in1=t2[:], op=mybir.AluOpType.add)
            t3 = work.tile([D, 1], f32, tag="t3", name="t3")
            nc.vector.tensor_tensor(out=t3[:], in0=b1c[:], in1=b1c[:], op=mybir.AluOpType.mult)
            nc.vector.tensor_tensor(out=var[:], in0=var[:], in1=t3[:], op=mybir.AluOpType.add)
            t4 = work.tile([D, 1], f32, tag="t4", name="t4")
            nc.vector.tensor_tensor(out=t4[:], in0=mu[:], in1=mu[:], op=mybir.AluOpType.mult)
            nc.vector.tensor_tensor(out=var[:], in0=var[:], in1=t4[:], op=mybir.AluOpType.subtract)

            sd = work.tile([D, 1], f32, tag="sd", name="sd")
            nc.scalar.activation(sd[:], var[:], mb.ActivationFunctionType.Sqrt, bias=epsc[:])
            rstd = work.tile([D, 1], f32, tag="rstd", name="rstd")
            nc.vector.reciprocal(out=rstd[:], in_=sd[:])
            a_sb = persist.tile([D, 1], f32, name="a_sb")
            nc.vector.tensor_tensor(out=a_sb[:], in0=gac[:], in1=rstd[:], op=mybir.AluOpType.mult)
            c_sb = persist.tile([D, 1], f32, name="c_sb")
            t5 = work.tile([D, 1], f32, tag="t5", name="t5")
            nc.vector.tensor_tensor(out=t5[:], in0=mu[:], in1=a_sb[:], op=mybir.AluOpType.mult)
            nc.vector.tensor_tensor(out=c_sb[:], in0=bec[:], in1=t5[:], op=mybir.AluOpType.subtract)
            # hT tiles exclude the b1 bias; fold it into the BN offset:
            # relu(a*(h+b1) + c) = relu(a*h + (c + a*b1))
            t6 = work.tile([D, 1], f32, tag="t6", name="t6")
            nc.vector.tensor_tensor(out=t6[:], in0=a_sb[:], in1=b1c[:], op=mybir.AluOpType.mult)
            nc.vector.tensor_tensor(out=c_sb[:], in0=c_sb[:], in1=t6[:], op=mybir.AluOpType.add)

            # --- phase F: BN+ReLU, transpose back, dinv fold -> xs2 ---
            xs2 = persist.tile([128, NT, D], f32, name="xs2")
            NF = NT if "noF" not in ablate else 0
            for b0 in range(0, NF, 4):
                bn = min(4, NF - b0)
                h1T = work.tile([D, 512], f32, tag="h1T", name="h1T", bufs=2)
                nc.scalar.activation(
                    h1T[:, : bn * 128],
                    hT_big[:, b0 * 128 : (b0 + bn) * 128],
                    mb.ActivationFunctionType.Relu,
                    bias=c_sb[:], scale=a_sb[:],
                )
                for j in range(bn):
                    b = b0 + j
                    nm_ps = wpsum.tile([128, D], f32, tag="ps_a", name="nm_ps")
                    nc.tensor.transpose(
                        out=nm_ps[:], in_=h1T[:, j * 128 : (j + 1) * 128],
                        identity=ident[:D, :D],
                    )
                    nc.vector.tensor_scalar_mul(
                        out=xs2[:, b, :], in0=nm_ps[:], scalar1=dinvs[:, b : b + 1]
                    )

            # --- phase G: layer 2 + pooling: poolT = xs2^T @ R_k, out = pool W2 ---
            poolT_ps = spsum.tile([D, NUM_GRAPHS], f32, name="poolT_ps")
            NG2 = NT if "noG" not in ablate else 1
            for b in range(NG2):
                nc.tensor.matmul(
                    out=poolT_ps[:], lhsT=xs2[:, b, :], rhs=rkt_t[:, b, :],
                    start=(b == 0), stop=(b == NG2 - 1),
                )
            poolT_sb = persist.tile([D, NUM_GRAPHS], f32, name="poolT_sb")
            nc.vector.tensor_copy(out=poolT_sb[:], in_=poolT_ps[:])
            out_ps = wpsum.tile([NUM_GRAPHS, D], f32, tag="ps_a", name="out_ps")
            nc.tensor.matmul(out=out_ps[:], lhsT=poolT_sb[:], rhs=w2s[:], start=True, stop=False)
            nc.tensor.matmul(out=out_ps[:], lhsT=p1s[:], rhs=b2r[:], start=False, stop=True)
            out_sb = persist.tile([NUM_GRAPHS, D], f32, name="out_sb")
            nc.vector.tensor_copy(out=out_sb[:], in_=out_ps[:])
            nc.sync.dma_start(out=aro_in[:, :], in_=out_sb[:])
            do_cc("AllReduce", mybir.AluOpType.add, aro_in[:, :], aro_out[:, :])
            nc.sync.dma_start(out=out_d[:, :], in_=aro_out[:, :])

    nc.compile()
    return nc


def _wrap16(v, n):
    """idx j at [j%16, j//16], replicated to 128 partitions (8 Q7 cores)."""
    assert v.shape[0] == n and n % 16 == 0
    t = v.astype(np.int16).reshape(n // 16, 16).T
    return np.tile(t, (8, 1))


def _bin_edges(gsrc, gdst, nbins):
    """Rotation binning: occurrence o of dst row d -> bin (d + o) % nbins.
    Rows are unique within each bin as long as multiplicity <= nbins.
    Returns per-bin (src, dst) arrays."""
    order = np.argsort(gdst, kind="stable")
    sd, ss = gdst[order], gsrc[order]
    if sd.shape[0] == 0:
        return [(np.zeros(0, np.int64), np.zeros(0, np.int64))] * nbins
    change = np.r_[True, sd[1:] != sd[:-1]]
    starts = np.flatnonzero(change)
    gid = np.cumsum(change) - 1
    occ = np.arange(sd.shape[0]) - starts[gid]
    assert int(occ.max()) < nbins, (int(occ.max()), nbins)
    b = (sd + occ) % nbins
    out = []
    for i in range(nbins):
        m = b == i
        out.append((ss[m], sd[m]))
    return out


def prepare_inputs(cfg, x, edge_index, batch, W1, b1, gamma, beta, W2, b2):
    """Host-side index preprocessing + per-core input maps.  Fills cfg.seg."""
    SL, SLP, NG = cfg.SL, cfg.SLP, cfg.NG
    n = cfg.N

    x = np.ascontiguousarray(np.asarray(x, dtype=np.float32))
    src = np.asarray(edge_index[0], dtype=np.int64)
    dst = np.asarray(edge_index[1], dtype=np.int64)
    batch = np.asarray(batch, dtype=np.int64)
    W1 = np.asarray(W1, dtype=np.float32)
    b1 = np.asarray(b1, dtype=np.float32)
    gamma = np.asarray(gamma, dtype=np.float32)
    beta = np.asarray(beta, dtype=np.float32)
    W2 = np.asarray(W2, dtype=np.float32)
    b2 = np.asarray(b2, dtype=np.float32)

    deg = np.bincount(dst, minlength=n).astype(np.float32) + 1.0  # + self-loop
    dinv = (1.0 / np.sqrt(deg)).astype(np.float32)

    cnt = np.bincount(batch, minlength=NUM_GRAPHS).astype(np.float32)
    w_graph = 1.0 / np.maximum(cnt, 1.0)
    pd = w_graph[batch] * dinv          # P[batch[v], v] * dinv_v  per node

    owner = src // SL
    src_local = src - owner * SL
    gdst = (dst // SL) * SLP + (dst - (dst // SL) * SL)
    loops = np.arange(n, dtype=np.int64)
    l_owner = loops // SL
    l_local = loops - l_owner * SL
    l_gdst = l_owner * SLP + l_local

    # per-core edge lists (edges by src owner + own self-loops), lo/hi buckets
    per_core = []
    for k in range(NCORES):
        sel = owner == k
        ls = l_owner == k
        es = np.concatenate([src_local[sel], l_local[ls]])
        ed = np.concatenate([gdst[sel], l_gdst[ls]])
        lo = ed < SPLIT
        per_core.append(((es[lo], ed[lo]), (es[~lo], ed[~lo])))

    # choose bin counts (shared across cores) per bucket
    def max_mult(arrs):
        m = 1
        for a in arrs:
            if a.shape[0]:
                m = max(m, int(np.bincount(a).max()))
        return m

    seg, core_bins = [], [[] for _ in range(NCORES)]
    for bkt in (0, 1):
        counts = [per_core[k][bkt][0].shape[0] for k in range(NCORES)]
        mm = max_mult([per_core[k][bkt][1] for k in range(NCORES)])
        nbins = max((max(counts) + cfg.CAP - 200) // (cfg.CAP - 200), mm, 1)
        while True:
            allb = [
                _bin_edges(per_core[k][bkt][0], per_core[k][bkt][1], nbins)
                for k in range(NCORES)
            ]
            sizes = [
                ((max(allb[k][i][0].shape[0] for k in range(NCORES)) + 127)
                 // 128) * 128
                for i in range(nbins)
            ]
            if all(s <= cfg.CAP for s in sizes):
                break
            nbins += 1
        for i in range(nbins):
            if sizes[i] == 0:
                continue
            seg.append((bkt, sizes[i]))
            for k in range(NCORES):
                core_bins[k].append((bkt, sizes[i], allb[k][i]))

    cfg.seg = seg

    # dead pad rows for scatter padding (always zero * dinv=0 downstream)
    trash_lo = SL                      # core 0's first pad row, < SPLIT
    trash_hi = NG - (SLP - SL)         # core 7's first pad row, >= SPLIT
    assert trash_hi >= SPLIT

    in_maps = []
    for k in range(NCORES):
        gl_parts, sc_parts = [], []
        for bkt, size, (es, ed) in core_bins[k]:
            m = es.shape[0]
            g = np.zeros(size, dtype=np.int64)
            s = np.full(size, (trash_lo if bkt == 0 else trash_hi - SPLIT),
                        dtype=np.int64)
            order = np.argsort(es, kind="stable")  # src-sorted for locality
            g[:m] = es[order]
            s[:m] = ed[order] - (0 if bkt == 0 else SPLIT)
            gl_parts.append(_wrap16(g, size))
            sc_parts.append(_wrap16(s, size))
        gidx = np.concatenate(gl_parts, axis=1)
        sidx = np.concatenate(sc_parts, axis=1)

        lo, hi = k * SL, min((k + 1) * SL, n)
        nsl = hi - lo
        xsl = np.zeros((SLP, D), dtype=np.float32)
        xsl[:nsl] = x[lo:hi]
        dsl = np.zeros(SLP, dtype=np.float32)
        dsl[:nsl] = dinv[lo:hi]
        dinv_in = dsl.reshape(cfg.NT, 128).T.copy()

        # R_k^T [SLP, 64]: R_kT[u, g] = sum_{edges (k*SL+u) -> w} P[g,w]*dinv_w
        #                             + P[g, k*SL+u]*dinv_{k*SL+u}
        sel = owner == k
        rkt = np.zeros((SLP, NUM_GRAPHS), dtype=np.float32)
        np.add.at(rkt, (src_local[sel], batch[dst[sel]]), pd[dst[sel]])
        rkt[np.arange(nsl), batch[lo:hi]] += pd[lo:hi]

        p1 = np.zeros((1, NUM_GRAPHS), dtype=np.float32)
        np.add.at(p1[0], batch[lo:hi], w_graph[batch[lo:hi]])

        in_maps.append({
            "xsl": xsl,
            "dinv_in": dinv_in,
            "gidx": np.ascontiguousarray(gidx),
            "sidx": np.ascontiguousarray(sidx),
            "rkt": rkt,
            "p1": p1,
            "w1": W1,
            "b1": b1.reshape(D, 1),
            "ga": gamma.reshape(D, 1),
            "be": beta.reshape(D, 1),
            "w2": W2,
            "b2": b2.reshape(1, D),
        })
    return in_maps


def kernel(x, edge_index, batch, W1, b1, gamma, beta, W2, b2):
    global LAST_EXEC_TIME_NS
    from concourse.bass_utils import run_bass_kernel_spmd

    cfg = Cfg(N_NODES, N_NODES // NCORES)
    in_maps = prepare_inputs(cfg, x, edge_index, batch, W1, b1, gamma, beta, W2, b2)

    key = (cfg.N, cfg.SL, tuple(cfg.seg))
    if key not in _NC_CACHE:
        _NC_CACHE[key] = build(cfg)
    nc = _NC_CACHE[key]
    global _LAST_IN_MAPS
    _LAST_IN_MAPS = in_maps

    trace = bool(int(os.environ.get("BASS_GNN_TRACE", "0")))
    if trace:
        try:
            res = run_bass_kernel_spmd(nc, in_maps, list(range(NCORES)), trace=True)
        except Exception:
            res = run_bass_kernel_spmd(nc, in_maps, list(range(NCORES)), trace=False)
    else:
        res = run_bass_kernel_spmd(nc, in_maps, list(range(NCORES)), trace=False)
    LAST_EXEC_TIME_NS = res.exec_time_ns
    return np.asarray(res.results[0]["out"], dtype=np.float32)


def modeled_time_ns(x=None, edge_index=None, **kw):
    """Cost-model execution time (MultiCoreSim, mocked collectives) for the
    current cached program; used when NTFF tracing is unavailable."""
    if not _NC_CACHE:
        return None
    nc = next(iter(_NC_CACHE.values()))
    ins = _LAST_IN_MAPS
    if ins is None:
        return None
    from concourse.bass_interp import MultiCoreSim

    sim = MultiCoreSim(nc, 2, debug_mock_collectives_without_correctness=True)
    for i, core in sim.cores.items():
        for name, val in ins[i].items():
            core.tensor(name)[:] = val
    sim.simulate()
    return int(sim.global_time)


# revision 25
# speedup vs baseline: 1.3512x; 1.3512x over previous
# GCN (2-layer GCNConv + BatchNorm + ReLU + global mean pool) on 8 TRN2 NeuronCores.
#
# Math (reference):
#   deg[v]  = in-degree incl. self-loop;  dinv = deg^-1/2
#   layer(x, W, b): h = D^-1/2 (A+I) D^-1/2 (x W) + b
#   h1 = relu(batchnorm(layer1));  h2 = layer2(h1);  out = segment_mean(h2, batch)
#
# Sharding (v4 — source-partitioned edges + fp16 ReduceScatter):
#   Core k owns nodes [k*SL, (k+1)*SL) and all edges whose SRC falls in that
#   range.  Layer 1:
#     * xs = dinv * x (own slice) -> local f32 gather table (DRAM); the edge
#       gather needs NO collective at all.
#     * per-edge: dma_gather xs[src] rows from the local table (f32, 256B
#       elems), convert the message tile to fp16 on DVE (hidden behind the
#       Pool-engine gather/scatter stream), then dma_scatter_add into a
#       global fp16 accumulator at the dst row.  The accumulator packs two
#       nodes per 256B row (scatter rows need 256B stride); node (p, g) of
#       core k lives at row k*SZJ + p*(NT/2) + g//2, column half g%2, so
#       scatter instructions are split by tile parity.
#     * one fp16 ReduceScatter hands each core the reduced rows of its own
#       slice (half the bytes of f32 — collective cost tracks output size).
#     * self-loops are folded in AFTER the ReduceScatter as one vector add
#       (z + xs) instead of 12.5k extra scatter slots.
#   BatchNorm stats via an accumulated A^T[A|1] matmul + algebraic reduction.
#   The tiny [64,65] stats reduction and the final [64,64] output reduction
#   use AllGather + local vector adds (cheaper than AllReduce).
#   Layer 2 + pooling collapse into dense matmuls: mean-pool P and the outer
#   D^-1/2 are linear, so out = sum_k (R_k @ xs2_k) W2 + b2 with
#   R_k[g, u] = sum_{edges u->w owned by k} P[g,w] dinv_w (+ self term),
#   built on the host from pure index data.  No second edge phase, no second
#   table, no second big collective.
#
# dma_scatter_add races (loses updates) for duplicate dst rows within one
# instruction, so edges are packed into instruction "bins" with unique dst
# rows per bin via rotation binning: occurrence o of dst row r goes to bin
# (r + o) % nbins.  The accumulator has 26112 rows, so scatter indices fit
# int16 with no bucketing.  Pad slots gather row 0 and scatter into a dead
# junk row (a reserved 64-row tile per core block).
#
# Host-side preprocessing uses only index data (edge_index, batch): degree
# computation, edge partitioning/binning, the R_k pooling matrices.  Feature
# data is never touched on the host.

import os

import numpy as np

N_NODES = 50000
N_EDGES = 800000
D = 64
NCORES = 8
NUM_GRAPHS = 64
BN_EPS = 1e-5


class Cfg:
    def __init__(self, n, sl):
        self.N = n                    # total nodes
        self.SL = sl                  # owned nodes per core
        slp = ((sl + 127) // 128) * 128
        if (slp // 128) % 2:
            slp += 128                # even tile count (node-pair packing)
        self.SLP = slp
        assert self.SL < self.SLP
        self.NT = self.SLP // 128     # 128-row node tiles per slice (even)
        self.HT = self.NT // 2
        self.SZ = 128 * self.HT       # real acc rows per core
        self.SZJ = self.SZ + 64       # + junk pair-tile
        self.CAP = 7680               # max slots per gather/scatter instruction
        # per-instruction (parity, padded slot count); filled by prepare_inputs
        self.seg = []


LAST_EXEC_TIME_NS = None
_NC_CACHE = {}
_LAST_IN_MAPS = None


def build(cfg):
    import concourse.mybir as mybir
    import concourse.tile as tile
    from concourse import bacc
    from concourse.bass import BassGpSimd
    from concourse.masks import make_identity

    f32 = mybir.dt.float32
    f16 = mybir.dt.float16
    i16 = mybir.dt.int16
    SLP, NT = cfg.SLP, cfg.NT
    NN = float(cfg.N)
    RG = [list(range(NCORES))]
    segs = cfg.seg
    tot_s = sum(c for _, c in segs)
    ACC_R = NCORES * cfg.SZJ

    nc = bacc.Bacc(
        "TRN2", target_bir_lowering=False, debug=False, num_devices=NCORES
    )

    # --- external inputs (per-core values supplied via in_maps) ---
    xsl = nc.declare_dram_parameter("xsl", [128, NT * D], f32, isOutput=False)
    dinv_in = nc.declare_dram_parameter("dinv_in", [128, NT], f32, isOutput=False)
    gidx_d = nc.declare_dram_parameter("gidx", [128, tot_s // 16], i16, isOutput=False)
    sidx_d = nc.declare_dram_parameter("sidx", [128, tot_s // 16], i16, isOutput=False)
    rkt_d = nc.declare_dram_parameter("rkt", [128, NT * D], f32, isOutput=False)
    p1_d = nc.declare_dram_parameter("p1", [1, NUM_GRAPHS], f32, isOutput=False)
    w1_d = nc.declare_dram_parameter("w1", [D, D], f32, isOutput=False)
    b1_d = nc.declare_dram_parameter("b1", [D, 1], f32, isOutput=False)
    ga_d = nc.declare_dram_parameter("ga", [D, 1], f32, isOutput=False)
    be_d = nc.declare_dram_parameter("be", [D, 1], f32, isOutput=False)
    w2_d = nc.declare_dram_parameter("w2", [D, D], f32, isOutput=False)
    b2_d = nc.declare_dram_parameter("b2", [1, D], f32, isOutput=False)
    out_d = nc.declare_dram_parameter("out", [NUM_GRAPHS, D], f32, isOutput=True)

    # --- internal DRAM ---
    table1 = nc.dram_tensor("table1", [SLP, D], f32)
    acc = nc.dram_tensor("acc", [ACC_R, 2 * D], f16)
    rs_out = nc.dram_tensor("rs_out", [cfg.SZJ, 2 * D], f16)
    ags_in = nc.dram_tensor("ags_in", [D, D + 1], f32)
    ags_out = nc.dram_tensor("ags_out", [NCORES * D, D + 1], f32, addr_space="Shared")
    ago_in = nc.dram_tensor("ago_in", [NUM_GRAPHS, D], f32)
    ago_out = nc.dram_tensor("ago_out", [NCORES * NUM_GRAPHS, D], f32, addr_space="Shared")

    cc_eng = os.environ.get("GNN_CC_ENG", "pool")

    def cc(kind, op, ins_ap, outs_ap):
        BassGpSimd.collective_compute(
            nc.gpsimd if cc_eng == "pool" else getattr(nc, cc_eng),
            kind, op, replica_groups=RG, ins=[ins_ap], outs=[outs_ap],
        )

    with tile.TileContext(nc) as tc:
        with (
            tc.tile_pool(name="const", bufs=1) as const,
            tc.tile_pool(name="persist", bufs=1) as persist,
            tc.tile_pool(name="work", bufs=2) as work,
            tc.tile_pool(name="msgp", bufs=3) as msgp,
            tc.tile_pool(name="msghp", bufs=2) as msghp,
            tc.tile_pool(name="spsum", bufs=1, space="PSUM") as spsum,
            tc.tile_pool(name="wpsum", bufs=2, space="PSUM") as wpsum,
            tc.tile_pool(name="fpsum", bufs=2, space="PSUM") as fpsum,
        ):
            ablate = os.environ.get("GNN_ABLATE", "")

            # --- zero tile for accumulator init (fp16) ---
            ZW = ACC_R * 2 * D // 8 // 128
            zt = persist.tile([128, ZW], f16, name="zt")
            nc.vector.memset(zt[:], 0.0)

            # --- phase A inputs: x slice (SP), edge indexes (Act) ---
            xs_t = persist.tile([128, NT, D], f32, name="xs_t")
            nc.sync.dma_start(
                out=xs_t[:], in_=xsl[:, :].rearrange("p (g d) -> p g d", d=D)
            )
            gidx_t = persist.tile([128, tot_s // 16], i16, name="gidx_t")
            nc.scalar.dma_start(out=gidx_t[:], in_=gidx_d[:, :])
            sidx_t = persist.tile([128, tot_s // 16], i16, name="sidx_t")
            nc.scalar.dma_start(out=sidx_t[:], in_=sidx_d[:, :])

            # --- constants into SBUF (Pool is idle until the first gather) ---
            w1s = const.tile([D, D], f32)
            nc.gpsimd.dma_start(out=w1s[:], in_=w1_d[:, :])
            w2s = const.tile([D, D], f32)
            nc.gpsimd.dma_start(out=w2s[:], in_=w2_d[:, :])
            b1c = const.tile([D, 1], f32)
            nc.gpsimd.dma_start(out=b1c[:], in_=b1_d[:, :])
            gac = const.tile([D, 1], f32)
            nc.gpsimd.dma_start(out=gac[:], in_=ga_d[:, :])
            bec = const.tile([D, 1], f32)
            nc.gpsimd.dma_start(out=bec[:], in_=be_d[:, :])
            b2r = const.tile([1, D], f32)
            nc.gpsimd.dma_start(out=b2r[:], in_=b2_d[:, :])
            p1s = const.tile([1, NUM_GRAPHS], f32)
            nc.gpsimd.dma_start(out=p1s[:], in_=p1_d[:, :])
            dinvs = const.tile([128, NT], f32)
            nc.gpsimd.dma_start(out=dinvs[:], in_=dinv_in[:, :])
            ident = const.tile([128, 128], f32)
            make_identity(nc, ident[:])
            ones64 = const.tile([D, 1], f32)
            nc.vector.memset(ones64[:], 1.0)
            epsc = const.tile([D, 1], f32)
            nc.vector.memset(epsc[:], BN_EPS)
            # preload the Sqrt/Relu activation tables off the critical path
            warm = const.tile([D, 1], f32)
            nc.scalar.activation(warm[:], epsc[:], mybir.ActivationFunctionType.Sqrt)
            nc.scalar.activation(warm[:], epsc[:], mybir.ActivationFunctionType.Relu)

            # --- phase A: xs = dinv * x -> local gather table (split SP/Act) ---
            dinv_b = dinvs[:, :].rearrange("p (g o) -> p g o", o=1).to_broadcast(
                [128, NT, D]
            )
            nc.vector.tensor_tensor(
                out=xs_t[:], in0=xs_t[:], in1=dinv_b, op=mybir.AluOpType.mult
            )
            tview = table1[:, :].rearrange("(g p) d -> p g d", p=128)
            nc.sync.dma_start(out=tview[:, : NT // 2, :], in_=xs_t[:, : NT // 2, :])
            nc.scalar.dma_start(out=tview[:, NT // 2 :, :], in_=xs_t[:, NT // 2 :, :])

            # --- accumulator zeroing: 8 chunks split across SP and Act ---
            acc_flat = acc[:, :].rearrange("n d -> (n d)")
            for j in range(8):
                ap = acc_flat.rearrange("(j p x) -> j p x", j=8, p=128)[j]
                (nc.sync if j % 2 else nc.scalar).dma_start(out=ap, in_=zt[:, :])

            # --- R_k^T for layer 2 (loaded during the edge phase) ---
            rkt_t = persist.tile([128, NT, D], f32, name="rkt_t")
            nc.sync.dma_start(
                out=rkt_t[:], in_=rkt_d[:, :].rearrange("p (g d) -> p g d", d=D)
            )

            # --- edge phase: gather f32 / convert fp16 / scatter-add fp16 ---
            import concourse.mybir as mb

            if "noedge" not in ablate:
                pend = None
                off = 0
                for c, cnt in segs:
                    msg = msgp.tile([128, cfg.CAP // 128, D], f32, tag="msg",
                                    name="msg")
                    nc.gpsimd.dma_gather(
                        out_ap=msg[:, : cnt // 128, :],
                        in_ap=table1[0:SLP, :],
                        idxs_ap=gidx_t[:, off : off + cnt // 16],
                        num_idxs=cnt, num_idxs_reg=cnt, elem_size=D,
                        single_packet=False, queue_num=0,
                    )
                    msgh = msghp.tile([128, cfg.CAP // 128, D], f16, tag="msgh",
                                      name="msgh")
                    nc.vector.tensor_copy(
                        out=msgh[:, : cnt // 128, :], in_=msg[:, : cnt // 128, :]
                    )
                    if pend is not None:
                        nc.gpsimd.dma_scatter_add(*pend, elem_step=2 * D, single_packet=False, queue_num=0)
                    pend = (
                        acc[:, c * D : (c + 1) * D],
                        msgh[:, : cnt // 128, :],
                        sidx_t[:, off : off + cnt // 16],
                        cnt, cnt, D,
                    )
                    off += cnt // 16
                if pend is not None:
                    nc.gpsimd.dma_scatter_add(*pend, elem_step=2 * D, single_packet=False, queue_num=0)

            # --- ReduceScatter -> own reduced slice (fp16) ---
            if "nocc" not in ablate:
                cc("ReduceScatter", mybir.AluOpType.add, acc[:, :], rs_out[:, :])
            else:
                nc.sync.dma_start(out=rs_out[:, :], in_=acc[0 : cfg.SZJ, :])

            # --- dense layer-1: z + self-loop, dinv scale, stats, W1 ---
            z_t = persist.tile([128, NT, D], f16, name="z_t")
            nc.sync.dma_start(
                out=z_t[:].rearrange("p g d -> p (g d)"),
                in_=rs_out[0 : cfg.SZ, :].rearrange("(p r) c -> p (r c)", p=128),
            )
            aggs = persist.tile([128, NT, D + 1], f32, name="aggs")
            nc.vector.memset(aggs[:, :, D : D + 1], 1.0)

            stats_ps = spsum.tile([D, D + 1], f32, name="stats_ps")
            hT_big = persist.tile([D, NT * 128], f32, name="hT_big")
            ND = NT if "noD" not in ablate else 1
            # pipeline the (z+xs)*dinv prep with the stats matmuls per chunk
            CH = 10
            for b0 in range(0, ND, CH):
                bn = min(CH, ND - b0)
                sl = slice(b0, b0 + bn)
                nc.vector.tensor_tensor(
                    out=aggs[:, sl, :D], in0=z_t[:, sl, :], in1=xs_t[:, sl, :],
                    op=mybir.AluOpType.add,
                )
                nc.vector.tensor_tensor(
                    out=aggs[:, sl, :D], in0=aggs[:, sl, :D],
                    in1=dinvs[:, sl].rearrange("p (g o) -> p g o", o=1)
                    .to_broadcast([128, bn, D]),
                    op=mybir.AluOpType.mult,
                )
                for b in range(b0, b0 + bn):
                    nc.tensor.matmul(
                        out=stats_ps[:], lhsT=aggs[:, b, :D], rhs=aggs[:, b, :],
                        start=(b == 0), stop=(b == ND - 1),
                    )
            # stats AllGather launched before the transposes/W1 matmuls so the
            # collective overlaps with PE work
            stats_sb = persist.tile([D, D + 1], f32, name="stats_sb")
            nc.vector.tensor_copy(out=stats_sb[:], in_=stats_ps[:])
            nc.sync.dma_start(out=ags_in[:, :], in_=stats_sb[:])
            if "nocc" not in ablate:
                cc("AllGather", mybir.AluOpType.bypass, ags_in[:, :], ags_out[:, :])
            else:
                nc.sync.dma_start(out=ags_out[0:D, :], in_=ags_in[:, :])

            for b0 in range(0, ND, 4):
                bn = min(4, ND - b0)
                tp_ps = wpsum.tile([D, 512], f32, tag="ps_a", name="tp_ps")
                for j in range(bn):
                    nc.tensor.transpose(
                        out=tp_ps[:, j * 128 : (j + 1) * 128],
                        in_=aggs[:, b0 + j, :D], identity=ident[:],
                    )
                aggsT = work.tile([D, 512], f32, tag="aggsT", name="aggsT", bufs=2)
                nc.vector.tensor_copy(out=aggsT[:, : bn * 128], in_=tp_ps[:, : bn * 128])
                hT_ps = wpsum.tile([D, 512], f32, tag="ps_b", name="hT_ps")
                nc.tensor.matmul(
                    out=hT_ps[:, : bn * 128], lhsT=w1s[:], rhs=aggsT[:, : bn * 128],
                    start=True, stop=True,
                )
                nc.vector.tensor_copy(
                    out=hT_big[:, b0 * 128 : (b0 + bn) * 128],
                    in_=hT_ps[:, : bn * 128],
                )

            stg = persist.tile([D, NCORES, D + 1], f32, name="stg")
            nc.scalar.dma_start(
                out=stg[:], in_=ags_out[:, :].rearrange("(j p) c -> p j c", p=D)
            )
            for h in (4, 2, 1):
                nc.vector.tensor_tensor(
                    out=stg[:, 0:h, :], in0=stg[:, 0:h, :],
                    in1=stg[:, h : 2 * h, :], op=mybir.AluOpType.add,
                )
            st = stg[:, 0, :]

            # --- BN scalar algebra ---
            q_ps = wpsum.tile([D, 1], f32, tag="ps_a", name="q_ps")
            nc.tensor.matmul(out=q_ps[:], lhsT=w1s[:], rhs=st[:, D : D + 1], start=True, stop=True)
            mu = persist.tile([D, 1], f32, name="mu")
            nc.vector.tensor_scalar(
                out=mu[:], in0=q_ps[:], scalar1=1.0 / NN, scalar2=b1c[:],
                op0=mybir.AluOpType.mult, op1=mybir.AluOpType.add,
            )
            t1_ps = wpsum.tile([D, D], f32, tag="ps_b", name="t1_ps")
            nc.tensor.matmul(out=t1_ps[:], lhsT=st[:, :D], rhs=w1s[:], start=True, stop=True)
            m_sb = work.tile([D, D], f32, tag="m_sb", name="m_sb")
            nc.vector.tensor_tensor(out=m_sb[:], in0=w1s[:], in1=t1_ps[:], op=mybir.AluOpType.mult)
            d_ps = wpsum.tile([D, 1], f32, tag="ps_b", name="d_ps")
            nc.tensor.matmul(out=d_ps[:], lhsT=m_sb[:], rhs=ones64[:], start=True, stop=True)

            var = persist.tile([D, 1], f32, name="var")
            nc.vector.tensor_scalar_mul(out=var[:], in0=d_ps[:], scalar1=1.0 / NN)
            t2 = work.tile([D, 1], f32, tag="t2", name="t2")
            nc.vector.tensor_scalar_mul(out=t2[:], in0=q_ps[:], scalar1=2.0 / NN)
            nc.vector.tensor_tensor(out=t2[:], in0=t2[:], in1=b1c[:], op=mybir.AluOpType.mult)
            nc.vector.tensor_tensor(out=var[:], in0=var[:], in1=t2[:], op=mybir.AluOpType.add)
            t3 = work.tile([D, 1], f32, tag="t3", name="t3")
            nc.vector.tensor_tensor(out=t3[:], in0=b1c[:], in1=b1c[:], op=mybir.AluOpType.mult)
            nc.vector.tensor_tensor(out=var[:], in0=var[:], in1=t3[:], op=mybir.AluOpType.add)
            t4 = work.tile([D, 1], f32, tag="t4", name="t4")
            nc.vector.tensor_tensor(out=t4[:], in0=mu[:], in1=mu[:], op=mybir.AluOpType.mult)
            nc.vector.tensor_tensor(out=var[:], in0=var[:], in1=t4[:], op=mybir.AluOpType.subtract)

            sd = work.tile([D, 1], f32, tag="sd", name="sd")
            nc.scalar.activation(sd[:], var[:], mb.ActivationFunctionType.Sqrt, bias=epsc[:])
            rstd = work.tile([D, 1], f32, tag="rstd", name="rstd")
            nc.vector.reciprocal(out=rstd[:], in_=sd[:])
            a_sb = persist.tile([D, 1], f32, name="a_sb")
            nc.vector.tensor_tensor(out=a_sb[:], in0=gac[:], in1=rstd[:], op=mybir.AluOpType.mult)
            c_sb = persist.tile([D, 1], f32, name="c_sb")
            t5 = work.tile([D, 1], f32, tag="t5", name="t5")
            nc.vector.tensor_tensor(out=t5[:], in0=mu[:], in1=a_sb[:], op=mybir.AluOpType.mult)
            nc.vector.tensor_tensor(out=c_sb[:], in0=bec[:], in1=t5[:], op=mybir.AluOpType.subtract)
            # hT tiles exclude the b1 bias; fold it into the BN offset:
            # relu(a*(h+b1) + c) = relu(a*h + (c + a*b1))
            t6 = work.tile([D, 1], f32, tag="t6", name="t6")
            nc.vector.tensor_tensor(out=t6[:], in0=a_sb[:], in1=b1c[:], op=mybir.AluOpType.mult)
            nc.vector.tensor_tensor(out=c_sb[:], in0=c_sb[:], in1=t6[:], op=mybir.AluOpType.add)

            # --- phase F: BN+ReLU, transpose back, dinv fold -> xs2;
            #     phase G interleaved: poolT += xs2_b^T @ R_b ---
            poolT_ps = spsum.tile([D, NUM_GRAPHS], f32, name="poolT_ps")
            xs2 = persist.tile([128, NT, D], f32, name="xs2")
            NF = NT if "noF" not in ablate else 0
            for b0 in range(0, NF, 4):
                bn = min(4, NF - b0)
                h1T = work.tile([D, 512], f32, tag="h1T", name="h1T", bufs=3)
                nc.scalar.activation(
                    h1T[:, : bn * 128],
                    hT_big[:, b0 * 128 : (b0 + bn) * 128],
                    mb.ActivationFunctionType.Relu,
                    bias=c_sb[:], scale=a_sb[:],
                )
                for j in range(bn):
                    b = b0 + j
                    nm_ps = fpsum.tile([128, D], f32, tag="ps_c", name="nm_ps")
                    nc.tensor.transpose(
                        out=nm_ps[:], in_=h1T[:, j * 128 : (j + 1) * 128],
                        identity=ident[:D, :D],
                    )
                    nc.vector.tensor_scalar_mul(
                        out=xs2[:, b, :], in0=nm_ps[:], scalar1=dinvs[:, b : b + 1]
                    )
                    nc.tensor.matmul(
                        out=poolT_ps[:], lhsT=xs2[:, b, :], rhs=rkt_t[:, b, :],
                        start=(b == 0), stop=(b == NF - 1),
                    )

            # --- output: pool @ W2 + p1^T b2; AllGather + local reduce ---
            poolT_sb = persist.tile([D, NUM_GRAPHS], f32, name="poolT_sb")
            nc.vector.tensor_copy(out=poolT_sb[:], in_=poolT_ps[:])
            out_ps = wpsum.tile([NUM_GRAPHS, D], f32, tag="ps_a", name="out_ps")
            nc.tensor.matmul(out=out_ps[:], lhsT=poolT_sb[:], rhs=w2s[:], start=True, stop=False)
            nc.tensor.matmul(out=out_ps[:], lhsT=p1s[:], rhs=b2r[:], start=False, stop=True)
            out_sb = persist.tile([NUM_GRAPHS, D], f32, name="out_sb")
            nc.vector.tensor_copy(out=out_sb[:], in_=out_ps[:])
            nc.sync.dma_start(out=ago_in[:, :], in_=out_sb[:])
            if "nocc" not in ablate:
                cc("AllGather", mybir.AluOpType.bypass, ago_in[:, :], ago_out[:, :])
            else:
                nc.sync.dma_start(out=ago_out[0:NUM_GRAPHS, :], in_=ago_in[:, :])
            og = persist.tile([NUM_GRAPHS, NCORES, D], f32, name="og")
            nc.scalar.dma_start(
                out=og[:],
                in_=ago_out[:, :].rearrange("(j p) d -> p j d", p=NUM_GRAPHS),
            )
            for h in (4, 2, 1):
                nc.vector.tensor_tensor(
                    out=og[:, 0:h, :], in0=og[:, 0:h, :],
                    in1=og[:, h : 2 * h, :], op=mybir.AluOpType.add,
                )
            nc.sync.dma_start(out=out_d[:, :], in_=og[:, 0, :])

    nc.compile()
    return nc


def _wrap16(v, n):
    """idx j at [j%16, j//16], replicated to 128 partitions (8 Q7 cores)."""
    assert v.shape[0] == n and n % 16 == 0
    t = v.astype(np.int16).reshape(n // 16, 16).T
    return np.tile(t, (8, 1))


def _bin_edges(gsrc, grow, nbins, nspill):
    """Rotation binning: occurrence o of dst row r -> bin (r + o) % nbins for
    o < nbins; higher occurrences spill into one extra bin per occurrence
    level (occurrence levels have unique rows by construction)."""
    order = np.argsort(grow, kind="stable")
    sd, ss = grow[order], gsrc[order]
    out = [(np.zeros(0, np.int64), np.zeros(0, np.int64))] * (nbins + nspill)
    if sd.shape[0] == 0:
        return out
    change = np.r_[True, sd[1:] != sd[:-1]]
    starts = np.flatnonzero(change)
    gid = np.cumsum(change) - 1
    occ = np.arange(sd.shape[0]) - starts[gid]
    assert int(occ.max()) < nbins + nspill, (int(occ.max()), nbins, nspill)
    b = np.where(occ < nbins, (sd + occ) % nbins, occ)
    return [(ss[b == i], sd[b == i]) for i in range(nbins + nspill)]


def prepare_inputs(cfg, x, edge_index, batch, W1, b1, gamma, beta, W2, b2):
    """Host-side index preprocessing + per-core input maps.  Fills cfg.seg."""
    SL, SLP, NT = cfg.SL, cfg.SLP, cfg.NT
    n = cfg.N

    x = np.ascontiguousarray(np.asarray(x, dtype=np.float32))
    src = np.asarray(edge_index[0], dtype=np.int64)
    dst = np.asarray(edge_index[1], dtype=np.int64)
    batch = np.asarray(batch, dtype=np.int64)
    W1 = np.asarray(W1, dtype=np.float32)
    b1 = np.asarray(b1, dtype=np.float32)
    gamma = np.asarray(gamma, dtype=np.float32)
    beta = np.asarray(beta, dtype=np.float32)
    W2 = np.asarray(W2, dtype=np.float32)
    b2 = np.asarray(b2, dtype=np.float32)

    deg = np.bincount(dst, minlength=n).astype(np.float32) + 1.0  # + self-loop
    dinv = (1.0 / np.sqrt(deg)).astype(np.float32)

    cnt = np.bincount(batch, minlength=NUM_GRAPHS).astype(np.float32)
    w_graph = 1.0 / np.maximum(cnt, 1.0)
    pd = w_graph[batch] * dinv          # P[batch[v], v] * dinv_v  per node

    owner = src // SL
    src_local = src - owner * SL

    # dst -> (parity, acc row): node (p, g) of core k ->
    # row k*SZJ + p*HT + g//2, column half g%2
    d_owner = dst // SL
    d_local = dst - d_owner * SL
    d_g = d_local // 128
    d_p = d_local - d_g * 128
    d_par = d_g % 2
    d_row = d_owner * cfg.SZJ + d_p * cfg.HT + d_g // 2

    per_core = [[None, None] for _ in range(NCORES)]
    for k in range(NCORES):
        sel = owner == k
        es, ed, ec = src_local[sel], d_row[sel], d_par[sel]
        for c in (0, 1):
            m = ec == c
            per_core[k][c] = (es[m], ed[m])

    # shared bin layout per parity
    seg, core_bins = [], [[] for _ in range(NCORES)]
    for c in (0, 1):
        counts = [per_core[k][c][0].shape[0] for k in range(NCORES)]
        mm = 1
        for k in range(NCORES):
            rows = per_core[k][c][1]
            if rows.shape[0]:
                mm = max(mm, int(np.bincount(rows).max()))
        nbins = max(-(-max(counts) // (cfg.CAP - 256)), 1)
        while True:
            nspill = max(mm - nbins, 0)
            allb = [
                _bin_edges(per_core[k][c][0], per_core[k][c][1], nbins, nspill)
                for k in range(NCORES)
            ]
            sizes = [
                ((max(allb[k][i][0].shape[0] for k in range(NCORES)) + 127)
                 // 128) * 128
                for i in range(nbins + nspill)
            ]
            if all(s <= cfg.CAP for s in sizes):
                break
            nbins += 1
        for i in range(nbins + nspill):
            if sizes[i] == 0:
                continue
            seg.append((c, sizes[i]))
            for k in range(NCORES):
                core_bins[k].append((c, sizes[i], allb[k][i]))

    cfg.seg = seg

    in_maps = []
    for k in range(NCORES):
        gl_parts, sc_parts = [], []
        for c, size, (es, ed) in core_bins[k]:
            m = es.shape[0]
            g = np.zeros(size, dtype=np.int64)
            s = np.full(size, cfg.SZ, dtype=np.int64)  # core 0 junk-tile row
            order = np.argsort(es, kind="stable")  # src-sorted for locality
            g[:m] = es[order]
            s[:m] = ed[order]
            gl_parts.append(_wrap16(g, size))
            sc_parts.append(_wrap16(s, size))
        gidx = np.concatenate(gl_parts, axis=1)
        sidx = np.concatenate(sc_parts, axis=1)

        lo, hi = k * SL, min((k + 1) * SL, n)
        nsl = hi - lo
        xsl = np.zeros((SLP, D), dtype=np.float32)
        xsl[:nsl] = x[lo:hi]
        xsl_pm = np.ascontiguousarray(
            xsl.reshape(NT, 128, D).transpose(1, 0, 2).reshape(128, NT * D)
        )
        dsl = np.zeros(SLP, dtype=np.float32)
        dsl[:nsl] = dinv[lo:hi]
        dinv_in = dsl.reshape(NT, 128).T.copy()

        # R_k^T [SLP, 64]: R_kT[u, g] = sum_{edges (k*SL+u) -> w} P[g,w]*dinv_w
        #                             + P[g, k*SL+u]*dinv_{k*SL+u}
        sel = owner == k
        rkt = np.zeros((SLP, NUM_GRAPHS), dtype=np.float32)
        np.add.at(rkt, (src_local[sel], batch[dst[sel]]), pd[dst[sel]])
        rkt[np.arange(nsl), batch[lo:hi]] += pd[lo:hi]
        rkt_pm = np.ascontiguousarray(
            rkt.reshape(NT, 128, NUM_GRAPHS).transpose(1, 0, 2).reshape(128, -1)
        )

        p1 = np.zeros((1, NUM_GRAPHS), dtype=np.float32)
        np.add.at(p1[0], batch[lo:hi], w_graph[batch[lo:hi]])

        in_maps.append({
            "xsl": xsl_pm,
            "dinv_in": dinv_in,
            "gidx": np.ascontiguousarray(gidx),
            "sidx": np.ascontiguousarray(sidx),
            "rkt": rkt_pm,
            "p1": p1,
            "w1": W1,
            "b1": b1.reshape(D, 1),
            "ga": gamma.reshape(D, 1),
            "be": beta.reshape(D, 1),
            "w2": W2,
            "b2": b2.reshape(1, D),
        })
    return in_maps


def kernel(x, edge_index, batch, W1, b1, gamma, beta, W2, b2):
    global LAST_EXEC_TIME_NS
    from concourse.bass_utils import run_bass_kernel_spmd

    cfg = Cfg(N_NODES, N_NODES // NCORES)
    in_maps = prepare_inputs(cfg, x, edge_index, batch, W1, b1, gamma, beta, W2, b2)

    key = (cfg.N, cfg.SL, tuple(cfg.seg))
    if key not in _NC_CACHE:
        _NC_CACHE[key] = build(cfg)
    nc = _NC_CACHE[key]
    global _LAST_IN_MAPS
    _LAST_IN_MAPS = in_maps

    trace = bool(int(os.environ.get("BASS_GNN_TRACE", "0")))
    if trace:
        try:
            res = run_bass_kernel_spmd(nc, in_maps, list(range(NCORES)), trace=True)
        except Exception:
            res = run_bass_kernel_spmd(nc, in_maps, list(range(NCORES)), trace=False)
    else:
        res = run_bass_kernel_spmd(nc, in_maps, list(range(NCORES)), trace=False)
    LAST_EXEC_TIME_NS = res.exec_time_ns
    return np.asarray(res.results[0]["out"], dtype=np.float32)


def modeled_time_ns(x=None, edge_index=None, **kw):
    """Cost-model execution time (MultiCoreSim, mocked collectives) for the
    current cached program; used when NTFF tracing is unavailable."""
    if not _NC_CACHE:
        return None
    nc = next(iter(_NC_CACHE.values()))
    ins = _LAST_IN_MAPS
    if ins is None:
        return None
    from concourse.bass_interp import MultiCoreSim

    sim = MultiCoreSim(nc, 2, debug_mock_collectives_without_correctness=True)
    for i, core in sim.cores.items():
        for name, val in ins[i].items():
            core.tensor(name)[:] = val
    sim.simulate()
    return int(sim.global_time)


# revision 39
# speedup vs baseline: 1.4338x; 1.0611x over previous
# GCN (2-layer GCNConv + BatchNorm + ReLU + global mean pool) on 8 TRN2 NeuronCores.
#
# Math (reference):
#   deg[v]  = in-degree incl. self-loop;  dinv = deg^-1/2
#   layer(x, W, b): h = D^-1/2 (A+I) D^-1/2 (x W) + b
#   h1 = relu(batchnorm(layer1));  h2 = layer2(h1);  out = segment_mean(h2, batch)
#
# Sharding (v4 — source-partitioned edges + fp16 ReduceScatter):
#   Core k owns nodes [k*SL, (k+1)*SL) and all edges whose SRC falls in that
#   range.  Layer 1:
#     * xs = dinv * x (own slice) -> local f32 gather table (DRAM); the edge
#       gather needs NO collective at all.
#     * per-edge: dma_gather xs[src] rows from the local table (f32, 256B
#       elems), convert the message tile to fp16 on DVE (hidden behind the
#       Pool-engine gather/scatter stream), then dma_scatter_add into a
#       global fp16 accumulator at the dst row.  The accumulator packs two
#       nodes per 256B row (scatter rows need 256B stride); node (p, g) of
#       core k lives at row k*SZJ + p*(NT/2) + g//2, column half g%2, so
#       scatter instructions are split by tile parity.
#     * one fp16 ReduceScatter hands each core the reduced rows of its own
#       slice (half the bytes of f32 — collective cost tracks output size).
#     * self-loops are folded in AFTER the ReduceScatter as one vector add
#       (z + xs) instead of 12.5k extra scatter slots.
#   BatchNorm stats via an accumulated A^T[A|1] matmul + algebraic reduction.
#   The tiny [64,65] stats reduction and the final [64,64] output reduction
#   use AllGather + local vector adds (cheaper than AllReduce).
#   Layer 2 + pooling collapse into dense matmuls: mean-pool P and the outer
#   D^-1/2 are linear, so out = sum_k (R_k @ xs2_k) W2 + b2 with
#   R_k[g, u] = sum_{edges u->w owned by k} P[g,w] dinv_w (+ self term),
#   built on the host from pure index data.  No second edge phase, no second
#   table, no second big collective.
#
# dma_scatter_add races (loses updates) for duplicate dst rows within one
# instruction, so edges are packed into instruction "bins" with unique dst
# rows per bin via rotation binning: occurrence o of dst row r goes to bin
# (r + o) % nbins.  The accumulator has 26112 rows, so scatter indices fit
# int16 with no bucketing.  Pad slots gather row 0 and scatter into a dead
# junk row (a reserved 64-row tile per core block).
#
# Host-side preprocessing uses only index data (edge_index, batch): degree
# computation, edge partitioning/binning, the R_k pooling matrices.  Feature
# data is never touched on the host.

import os

import numpy as np

N_NODES = 50000
N_EDGES = 800000
D = 64
NCORES = 8
NUM_GRAPHS = 64
BN_EPS = 1e-5


class Cfg:
    def __init__(self, n, sl):
        self.N = n                    # total nodes
        self.SL = sl                  # owned nodes per core
        slp = ((sl + 127) // 128) * 128
        if (slp // 128) % 2:
            slp += 128                # even tile count (node-pair packing)
        self.SLP = slp
        assert self.SL < self.SLP
        self.NT = self.SLP // 128     # 128-row node tiles per slice (even)
        self.HT = self.NT // 2
        self.SZ = 128 * self.HT       # real acc rows per core
        self.SZJ = self.SZ + 64       # + junk pair-tile
        self.CAP = 7680               # max slots per gather/scatter instruction
        # per-instruction (parity, padded slot count); filled by prepare_inputs
        self.seg = []


LAST_EXEC_TIME_NS = None
_NC_CACHE = {}
_LAST_IN_MAPS = None


def build(cfg):
    import concourse.mybir as mybir
    import concourse.tile as tile
    from concourse import bacc
    from concourse.bass import BassGpSimd
    from concourse.masks import make_identity

    f32 = mybir.dt.float32
    f16 = mybir.dt.float16
    i16 = mybir.dt.int16
    SLP, NT = cfg.SLP, cfg.NT
    NN = float(cfg.N)
    RG = [list(range(NCORES))]
    segs = cfg.seg
    tot_s = sum(c for _, c in segs)
    ACC_R = NCORES * cfg.SZJ

    nc = bacc.Bacc(
        "TRN2", target_bir_lowering=False, debug=False, num_devices=NCORES
    )

    # --- external inputs (per-core values supplied via in_maps) ---
    xsl = nc.declare_dram_parameter("xsl", [128, NT * D], f32, isOutput=False)
    dinv_in = nc.declare_dram_parameter("dinv_in", [128, NT], f32, isOutput=False)
    gidx_d = nc.declare_dram_parameter("gidx", [128, tot_s // 16], i16, isOutput=False)
    sidx_d = nc.declare_dram_parameter("sidx", [128, tot_s // 16], i16, isOutput=False)
    rkt_d = nc.declare_dram_parameter("rkt", [128, NT * D], f32, isOutput=False)
    p1_d = nc.declare_dram_parameter("p1", [1, NUM_GRAPHS], f32, isOutput=False)
    w1_d = nc.declare_dram_parameter("w1", [D, D], f32, isOutput=False)
    b1_d = nc.declare_dram_parameter("b1", [D, 1], f32, isOutput=False)
    ga_d = nc.declare_dram_parameter("ga", [D, 1], f32, isOutput=False)
    be_d = nc.declare_dram_parameter("be", [D, 1], f32, isOutput=False)
    w2_d = nc.declare_dram_parameter("w2", [D, D], f32, isOutput=False)
    b2_d = nc.declare_dram_parameter("b2", [1, D], f32, isOutput=False)
    out_d = nc.declare_dram_parameter("out", [NUM_GRAPHS, D], f32, isOutput=True)

    # --- internal DRAM ---
    table1 = nc.dram_tensor("table1", [SLP, D], f32)
    acc = nc.dram_tensor("acc", [ACC_R, 2 * D], f16)
    rs_out = nc.dram_tensor("rs_out", [cfg.SZJ, 2 * D], f16)
    ags_in = nc.dram_tensor("ags_in", [D, D + 1], f16)
    ags_out = nc.dram_tensor("ags_out", [NCORES * D, D + 1], f16, addr_space="Shared")
    ago_in = nc.dram_tensor("ago_in", [NUM_GRAPHS, D], f16)
    ago_out = nc.dram_tensor("ago_out", [NCORES * NUM_GRAPHS, D], f16, addr_space="Shared")

    cc_eng = os.environ.get("GNN_CC_ENG", "pool")

    def cc(kind, op, ins_ap, outs_ap):
        BassGpSimd.collective_compute(
            nc.gpsimd if cc_eng == "pool" else getattr(nc, cc_eng),
            kind, op, replica_groups=RG, ins=[ins_ap], outs=[outs_ap],
        )

    with tile.TileContext(nc) as tc:
        with (
            tc.tile_pool(name="const", bufs=1) as const,
            tc.tile_pool(name="persist", bufs=1) as persist,
            tc.tile_pool(name="work", bufs=2) as work,
            tc.tile_pool(name="msgp", bufs=3) as msgp,
            tc.tile_pool(name="msghp", bufs=2) as msghp,
            tc.tile_pool(name="spsum", bufs=1, space="PSUM") as spsum,
            tc.tile_pool(name="wpsum", bufs=2, space="PSUM") as wpsum,
            tc.tile_pool(name="fpsum", bufs=2, space="PSUM") as fpsum,
        ):
            ablate = os.environ.get("GNN_ABLATE", "")

            # --- zero tile for accumulator init (fp16) ---
            ZW = ACC_R * 2 * D // 8 // 128
            zt = persist.tile([128, ZW], f16, name="zt")
            nc.vector.memset(zt[:], 0.0)

            # --- phase A inputs: x slice (SP), edge indexes (Act) ---
            xs_t = persist.tile([128, NT, D], f32, name="xs_t")
            nc.sync.dma_start(
                out=xs_t[:], in_=xsl[:, :].rearrange("p (g d) -> p g d", d=D)
            )
            gidx_t = persist.tile([128, tot_s // 16], i16, name="gidx_t")
            nc.scalar.dma_start(out=gidx_t[:], in_=gidx_d[:, :])
            sidx_t = persist.tile([128, tot_s // 16], i16, name="sidx_t")
            nc.scalar.dma_start(out=sidx_t[:], in_=sidx_d[:, :])

            # --- constants into SBUF (Pool is idle until the first gather) ---
            w1s = const.tile([D, D], f32)
            nc.gpsimd.dma_start(out=w1s[:], in_=w1_d[:, :])
            w2s = const.tile([D, D], f32)
            nc.gpsimd.dma_start(out=w2s[:], in_=w2_d[:, :])
            b1c = const.tile([D, 1], f32)
            nc.gpsimd.dma_start(out=b1c[:], in_=b1_d[:, :])
            gac = const.tile([D, 1], f32)
            nc.gpsimd.dma_start(out=gac[:], in_=ga_d[:, :])
            bec = const.tile([D, 1], f32)
            nc.gpsimd.dma_start(out=bec[:], in_=be_d[:, :])
            b2r = const.tile([1, D], f32)
            nc.gpsimd.dma_start(out=b2r[:], in_=b2_d[:, :])
            p1s = const.tile([1, NUM_GRAPHS], f32)
            nc.gpsimd.dma_start(out=p1s[:], in_=p1_d[:, :])
            dinvs = const.tile([128, NT], f32)
            nc.gpsimd.dma_start(out=dinvs[:], in_=dinv_in[:, :])
            ident = const.tile([128, 128], f32)
            make_identity(nc, ident[:])
            ones64 = const.tile([D, 1], f32)
            nc.vector.memset(ones64[:], 1.0)
            epsc = const.tile([D, 1], f32)
            nc.vector.memset(epsc[:], BN_EPS)
            # preload the Sqrt/Relu activation tables off the critical path
            warm = const.tile([D, 1], f32)
            nc.scalar.activation(warm[:], epsc[:], mybir.ActivationFunctionType.Sqrt)
            nc.scalar.activation(warm[:], epsc[:], mybir.ActivationFunctionType.Relu)

            # --- phase A: xs = dinv * x -> local gather table (split SP/Act),
            #     interleaved with the 8 accumulator zero chunks ---
            dinv_b = dinvs[:, :].rearrange("p (g o) -> p g o", o=1).to_broadcast(
                [128, NT, D]
            )
            nc.vector.tensor_tensor(
                out=xs_t[:], in0=xs_t[:], in1=dinv_b, op=mybir.AluOpType.mult
            )
            acc_flat = acc[:, :].rearrange("n d -> (n d)")

            def zchunk(eng, j):
                ap = acc_flat.rearrange("(j p x) -> j p x", j=8, p=128)[j]
                eng.dma_start(out=ap, in_=zt[:, :])

            tview = table1[:, :].rearrange("(g p) d -> p g d", p=128)
            zchunk(nc.sync, 0)
            zchunk(nc.sync, 1)
            nc.sync.dma_start(out=tview[:, : NT // 2, :], in_=xs_t[:, : NT // 2, :])
            nc.scalar.dma_start(out=tview[:, NT // 2 :, :], in_=xs_t[:, NT // 2 :, :])
            zchunk(nc.sync, 2)
            zchunk(nc.sync, 3)
            zchunk(nc.sync, 4)
            zchunk(nc.scalar, 5)
            zchunk(nc.scalar, 6)
            zchunk(nc.scalar, 7)

            # --- R_k^T for layer 2 (loaded during the edge phase) ---
            rkt_t = persist.tile([128, NT, D], f32, name="rkt_t")
            nc.sync.dma_start(
                out=rkt_t[:], in_=rkt_d[:, :].rearrange("p (g d) -> p g d", d=D)
            )

            # --- edge phase: gather f32 / convert fp16 / scatter-add fp16 ---
            import concourse.mybir as mb

            if "noedge" not in ablate:
                pend = None
                off = 0
                for c, cnt in segs:
                    msg = msgp.tile([128, cfg.CAP // 128, D], f32, tag="msg",
                                    name="msg")
                    nc.gpsimd.dma_gather(
                        out_ap=msg[:, : cnt // 128, :],
                        in_ap=table1[0:SLP, :],
                        idxs_ap=gidx_t[:, off : off + cnt // 16],
                        num_idxs=cnt, num_idxs_reg=cnt, elem_size=D,
                        single_packet=False, queue_num=0,
                    )
                    msgh = msghp.tile([128, cfg.CAP // 128, D], f16, tag="msgh",
                                      name="msgh")
                    nc.vector.tensor_copy(
                        out=msgh[:, : cnt // 128, :], in_=msg[:, : cnt // 128, :]
                    )
                    if pend is not None:
                        nc.gpsimd.dma_scatter_add(*pend, elem_step=2 * D, single_packet=False, queue_num=0)
                    pend = (
                        acc[:, c * D : (c + 1) * D],
                        msgh[:, : cnt // 128, :],
                        sidx_t[:, off : off + cnt // 16],
                        cnt, cnt, D,
                    )
                    off += cnt // 16
                if pend is not None:
                    nc.gpsimd.dma_scatter_add(*pend, elem_step=2 * D, single_packet=False, queue_num=0)

            # --- ReduceScatter -> own reduced slice (fp16) ---
            if "nocc" not in ablate:
                cc("ReduceScatter", mybir.AluOpType.add, acc[:, :], rs_out[:, :])
            else:
                nc.sync.dma_start(out=rs_out[:, :], in_=acc[0 : cfg.SZJ, :])

            # warm the PE p-state during the collective (it idles otherwise,
            # and the first post-RS matmuls would run at the slow p-state)
            warm_ps = fpsum.tile([D, D], f32, tag="ps_c", name="warm_ps")
            for _ in range(24):
                nc.tensor.matmul(
                    out=warm_ps[:], lhsT=ident[:D, :D], rhs=ident[:D, :D],
                    start=True, stop=True,
                )

            # --- dense layer-1: z + self-loop, dinv scale, stats, W1 ---
            z_t = persist.tile([128, NT, D], f16, name="z_t")
            zsrc = rs_out[0 : cfg.SZ, :].rearrange("(p r) c -> p (r c)", p=128)
            zdst = z_t[:].rearrange("p g d -> p (g d)")
            HW_ = NT * D // 2
            nc.sync.dma_start(out=zdst[:, :HW_], in_=zsrc[:, :HW_])
            nc.sync.dma_start(out=zdst[:, HW_:], in_=zsrc[:, HW_:])
            aggs = persist.tile([128, NT, D + 1], f32, name="aggs")
            nc.vector.memset(aggs[:, :, D : D + 1], 1.0)

            stats_ps = spsum.tile([D, D + 1], f32, name="stats_ps")
            hT_big = persist.tile([D, NT * 128], f32, name="hT_big")
            ND = NT if "noD" not in ablate else 1
            # pipeline the (z+xs)*dinv prep with the stats matmuls per chunk
            CH = 10
            for b0 in range(0, ND, CH):
                bn = min(CH, ND - b0)
                sl = slice(b0, b0 + bn)
                nc.vector.tensor_tensor(
                    out=aggs[:, sl, :D], in0=z_t[:, sl, :], in1=xs_t[:, sl, :],
                    op=mybir.AluOpType.add,
                )
                nc.vector.tensor_tensor(
                    out=aggs[:, sl, :D], in0=aggs[:, sl, :D],
                    in1=dinvs[:, sl].rearrange("p (g o) -> p g o", o=1)
                    .to_broadcast([128, bn, D]),
                    op=mybir.AluOpType.mult,
                )
                for b in range(b0, b0 + bn):
                    nc.tensor.matmul(
                        out=stats_ps[:], lhsT=aggs[:, b, :D], rhs=aggs[:, b, :],
                        start=(b == 0), stop=(b == ND - 1),
                    )
            # stats AllGather launched before the transposes/W1 matmuls so the
            # collective overlaps with PE work
            stats_sb = persist.tile([D, D + 1], f16, name="stats_sb")
            nc.vector.tensor_copy(out=stats_sb[:], in_=stats_ps[:])
            nc.sync.dma_start(out=ags_in[:, :], in_=stats_sb[:])
            if "nocc" not in ablate:
                cc("AllGather", mybir.AluOpType.bypass, ags_in[:, :], ags_out[:, :])
            else:
                nc.sync.dma_start(out=ags_out[0:D, :], in_=ags_in[:, :])

            for b0 in range(0, ND, 4):
                bn = min(4, ND - b0)
                tp_ps = wpsum.tile([D, 512], f32, tag="ps_a", name="tp_ps")
                for j in range(bn):
                    nc.tensor.transpose(
                        out=tp_ps[:, j * 128 : (j + 1) * 128],
                        in_=aggs[:, b0 + j, :D], identity=ident[:],
                    )
                aggsT = work.tile([D, 512], f32, tag="aggsT", name="aggsT", bufs=2)
                nc.vector.tensor_copy(out=aggsT[:, : bn * 128], in_=tp_ps[:, : bn * 128])
                hT_ps = wpsum.tile([D, 512], f32, tag="ps_b", name="hT_ps")
                nc.tensor.matmul(
                    out=hT_ps[:, : bn * 128], lhsT=w1s[:], rhs=aggsT[:, : bn * 128],
                    start=True, stop=True,
                )
                nc.vector.tensor_copy(
                    out=hT_big[:, b0 * 128 : (b0 + bn) * 128],
                    in_=hT_ps[:, : bn * 128],
                )

            stg = persist.tile([D, NCORES, D + 1], f16, name="stg")
            nc.scalar.dma_start(
                out=stg[:], in_=ags_out[:, :].rearrange("(j p) c -> p j c", p=D)
            )
            for h in (4, 2, 1):
                nc.vector.tensor_tensor(
                    out=stg[:, 0:h, :], in0=stg[:, 0:h, :],
                    in1=stg[:, h : 2 * h, :], op=mybir.AluOpType.add,
                )
            st32 = persist.tile([D, D + 1], f32, name="st32")
            nc.vector.tensor_copy(out=st32[:], in_=stg[:, 0, :])
            st = st32[:]

            # --- BN scalar algebra ---
            q_ps = wpsum.tile([D, 1], f32, tag="ps_a", name="q_ps")
            nc.tensor.matmul(out=q_ps[:], lhsT=w1s[:], rhs=st[:, D : D + 1], start=True, stop=True)
            mu = persist.tile([D, 1], f32, name="mu")
            nc.vector.tensor_scalar(
                out=mu[:], in0=q_ps[:], scalar1=1.0 / NN, scalar2=b1c[:],
                op0=mybir.AluOpType.mult, op1=mybir.AluOpType.add,
            )
            t1_ps = wpsum.tile([D, D], f32, tag="ps_b", name="t1_ps")
            nc.tensor.matmul(out=t1_ps[:], lhsT=st[:, :D], rhs=w1s[:], start=True, stop=True)
            m_sb = work.tile([D, D], f32, tag="m_sb", name="m_sb")
            nc.vector.tensor_tensor(out=m_sb[:], in0=w1s[:], in1=t1_ps[:], op=mybir.AluOpType.mult)
            d_ps = wpsum.tile([D, 1], f32, tag="ps_b", name="d_ps")
            nc.tensor.matmul(out=d_ps[:], lhsT=m_sb[:], rhs=ones64[:], start=True, stop=True)

            var = persist.tile([D, 1], f32, name="var")
            nc.vector.tensor_scalar_mul(out=var[:], in0=d_ps[:], scalar1=1.0 / NN)
            t2 = work.tile([D, 1], f32, tag="t2", name="t2")
            nc.vector.tensor_scalar_mul(out=t2[:], in0=q_ps[:], scalar1=2.0 / NN)
            nc.vector.tensor_tensor(out=t2[:], in0=t2[:], in1=b1c[:], op=mybir.AluOpType.mult)
            nc.vector.tensor_tensor(out=var[:], in0=var[:], in1=t2[:], op=mybir.AluOpType.add)
            t3 = work.tile([D, 1], f32, tag="t3", name="t3")
            nc.vector.tensor_tensor(out=t3[:], in0=b1c[:], in1=b1c[:], op=mybir.AluOpType.mult)
            nc.vector.tensor_tensor(out=var[:], in0=var[:], in1=t3[:], op=mybir.AluOpType.add)
            t4 = work.tile([D, 1], f32, tag="t4", name="t4")
            nc.vector.tensor_tensor(out=t4[:], in0=mu[:], in1=mu[:], op=mybir.AluOpType.mult)
            nc.vector.tensor_tensor(out=var[:], in0=var[:], in1=t4[:], op=mybir.AluOpType.subtract)

            sd = work.tile([D, 1], f32, tag="sd", name="sd")
            nc.scalar.activation(sd[:], var[:], mb.ActivationFunctionType.Sqrt, bias=epsc[:])
            rstd = work.tile([D, 1], f32, tag="rstd", name="rstd")
            nc.vector.reciprocal(out=rstd[:], in_=sd[:])
            a_sb = persist.tile([D, 1], f32, name="a_sb")
            nc.vector.tensor_tensor(out=a_sb[:], in0=gac[:], in1=rstd[:], op=mybir.AluOpType.mult)
            c_sb = persist.tile([D, 1], f32, name="c_sb")
            t5 = work.tile([D, 1], f32, tag="t5", name="t5")
            nc.vector.tensor_tensor(out=t5[:], in0=mu[:], in1=a_sb[:], op=mybir.AluOpType.mult)
            nc.vector.tensor_tensor(out=c_sb[:], in0=bec[:], in1=t5[:], op=mybir.AluOpType.subtract)
            # hT tiles exclude the b1 bias; fold it into the BN offset:
            # relu(a*(h+b1) + c) = relu(a*h + (c + a*b1))
            t6 = work.tile([D, 1], f32, tag="t6", name="t6")
            nc.vector.tensor_tensor(out=t6[:], in0=a_sb[:], in1=b1c[:], op=mybir.AluOpType.mult)
            nc.vector.tensor_tensor(out=c_sb[:], in0=c_sb[:], in1=t6[:], op=mybir.AluOpType.add)

            # --- phase F: BN+ReLU, transpose back, dinv fold -> xs2;
            #     phase G interleaved: poolT += xs2_b^T @ R_b ---
            poolT_ps = spsum.tile([D, NUM_GRAPHS], f32, name="poolT_ps")
            xs2 = persist.tile([128, NT, D], f32, name="xs2")
            NF = NT if "noF" not in ablate else 0
            for b0 in range(0, NF, 4):
                bn = min(4, NF - b0)
                h1T = work.tile([D, 512], f32, tag="h1T", name="h1T", bufs=3)
                nc.scalar.activation(
                    h1T[:, : bn * 128],
                    hT_big[:, b0 * 128 : (b0 + bn) * 128],
                    mb.ActivationFunctionType.Relu,
                    bias=c_sb[:], scale=a_sb[:],
                )
                nm_ps = fpsum.tile([128, 4 * D], f32, tag="ps_c", name="nm_ps")
                for j in range(bn):
                    nc.tensor.transpose(
                        out=nm_ps[:, j * D : (j + 1) * D],
                        in_=h1T[:, j * 128 : (j + 1) * 128],
                        identity=ident[:D, :D],
                    )
                nc.vector.tensor_tensor(
                    out=xs2[:, b0 : b0 + bn, :], in0=nm_ps[:, : bn * D].rearrange(
                        "p (g d) -> p g d", d=D),
                    in1=dinvs[:, b0 : b0 + bn].rearrange(
                        "p (g o) -> p g o", o=1).to_broadcast([128, bn, D]),
                    op=mybir.AluOpType.mult,
                )
                for j in range(bn):
                    b = b0 + j
                    nc.tensor.matmul(
                        out=poolT_ps[:], lhsT=xs2[:, b, :], rhs=rkt_t[:, b, :],
                        start=(b == 0), stop=(b == NF - 1),
                    )

            # --- output: pool @ W2 + p1^T b2; AllGather + local reduce ---
            poolT_sb = persist.tile([D, NUM_GRAPHS], f32, name="poolT_sb")
            nc.vector.tensor_copy(out=poolT_sb[:], in_=poolT_ps[:])
            out_ps = wpsum.tile([NUM_GRAPHS, D], f32, tag="ps_a", name="out_ps")
            nc.tensor.matmul(out=out_ps[:], lhsT=poolT_sb[:], rhs=w2s[:], start=True, stop=False)
            nc.tensor.matmul(out=out_ps[:], lhsT=p1s[:], rhs=b2r[:], start=False, stop=True)
            out_sb = persist.tile([NUM_GRAPHS, D], f16, name="out_sb")
            nc.vector.tensor_copy(out=out_sb[:], in_=out_ps[:])
            nc.sync.dma_start(out=ago_in[:, :], in_=out_sb[:])
            if "nocc" not in ablate:
                cc("AllGather", mybir.AluOpType.bypass, ago_in[:, :], ago_out[:, :])
            else:
                nc.sync.dma_start(out=ago_out[0:NUM_GRAPHS, :], in_=ago_in[:, :])
            og = persist.tile([NUM_GRAPHS, NCORES, D], f16, name="og")
            nc.scalar.dma_start(
                out=og[:],
                in_=ago_out[:, :].rearrange("(j p) d -> p j d", p=NUM_GRAPHS),
            )
            for h in (4, 2, 1):
                nc.vector.tensor_tensor(
                    out=og[:, 0:h, :], in0=og[:, 0:h, :],
                    in1=og[:, h : 2 * h, :], op=mybir.AluOpType.add,
                )
            og32 = persist.tile([NUM_GRAPHS, D], f32, name="og32")
            nc.vector.tensor_copy(out=og32[:], in_=og[:, 0, :])
            nc.sync.dma_start(out=out_d[:, :], in_=og32[:])

    nc.compile()
    return nc


def _wrap16(v, n):
    """idx j at [j%16, j//16], replicated to 128 partitions (8 Q7 cores)."""
    assert v.shape[0] == n and n % 16 == 0
    t = v.astype(np.int16).reshape(n // 16, 16).T
    return np.tile(t, (8, 1))


def _bin_edges(gsrc, grow, nbins, nspill):
    """Rotation binning: occurrence o of dst row r -> bin (r + o) % nbins for
    o < nbins; higher occurrences spill into one extra bin per occurrence
    level (occurrence levels have unique rows by construction)."""
    order = np.argsort(grow, kind="stable")
    sd, ss = grow[order], gsrc[order]
    out = [(np.zeros(0, np.int64), np.zeros(0, np.int64))] * (nbins + nspill)
    if sd.shape[0] == 0:
        return out
    change = np.r_[True, sd[1:] != sd[:-1]]
    starts = np.flatnonzero(change)
    gid = np.cumsum(change) - 1
    occ = np.arange(sd.shape[0]) - starts[gid]
    assert int(occ.max()) < nbins + nspill, (int(occ.max()), nbins, nspill)
    b = np.where(occ < nbins, (sd + occ) % nbins, occ)
    return [(ss[b == i], sd[b == i]) for i in range(nbins + nspill)]


def prepare_inputs(cfg, x, edge_index, batch, W1, b1, gamma, beta, W2, b2):
    """Host-side index preprocessing + per-core input maps.  Fills cfg.seg."""
    SL, SLP, NT = cfg.SL, cfg.SLP, cfg.NT
    n = cfg.N

    x = np.ascontiguousarray(np.asarray(x, dtype=np.float32))
    src = np.asarray(edge_index[0], dtype=np.int64)
    dst = np.asarray(edge_index[1], dtype=np.int64)
    batch = np.asarray(batch, dtype=np.int64)
    W1 = np.asarray(W1, dtype=np.float32)
    b1 = np.asarray(b1, dtype=np.float32)
    gamma = np.asarray(gamma, dtype=np.float32)
    beta = np.asarray(beta, dtype=np.float32)
    W2 = np.asarray(W2, dtype=np.float32)
    b2 = np.asarray(b2, dtype=np.float32)

    deg = np.bincount(dst, minlength=n).astype(np.float32) + 1.0  # + self-loop
    dinv = (1.0 / np.sqrt(deg)).astype(np.float32)

    cnt = np.bincount(batch, minlength=NUM_GRAPHS).astype(np.float32)
    w_graph = 1.0 / np.maximum(cnt, 1.0)
    pd = w_graph[batch] * dinv          # P[batch[v], v] * dinv_v  per node

    owner = src // SL
    src_local = src - owner * SL

    # dst -> (parity, acc row): node (p, g) of core k ->
    # row k*SZJ + p*HT + g//2, column half g%2
    d_owner = dst // SL
    d_local = dst - d_owner * SL
    d_g = d_local // 128
    d_p = d_local - d_g * 128
    d_par = d_g % 2
    d_row = d_owner * cfg.SZJ + d_p * cfg.HT + d_g // 2

    per_core = [[None, None] for _ in range(NCORES)]
    for k in range(NCORES):
        sel = owner == k
        es, ed, ec = src_local[sel], d_row[sel], d_par[sel]
        for c in (0, 1):
            m = ec == c
            per_core[k][c] = (es[m], ed[m])

    # shared bin layout per parity
    seg, core_bins = [], [[] for _ in range(NCORES)]
    for c in (0, 1):
        counts = [per_core[k][c][0].shape[0] for k in range(NCORES)]
        mm = 1
        for k in range(NCORES):
            rows = per_core[k][c][1]
            if rows.shape[0]:
                mm = max(mm, int(np.bincount(rows).max()))
        nbins = max(-(-max(counts) // (cfg.CAP - 256)), 1)
        while True:
            nspill = max(mm - nbins, 0)
            allb = [
                _bin_edges(per_core[k][c][0], per_core[k][c][1], nbins, nspill)
                for k in range(NCORES)
            ]
            sizes = [
                ((max(allb[k][i][0].shape[0] for k in range(NCORES)) + 127)
                 // 128) * 128
                for i in range(nbins + nspill)
            ]
            if all(s <= cfg.CAP for s in sizes):
                break
            nbins += 1
        for i in range(nbins + nspill):
            if sizes[i] == 0:
                continue
            seg.append((c, sizes[i]))
            for k in range(NCORES):
                core_bins[k].append((c, sizes[i], allb[k][i]))

    cfg.seg = seg

    in_maps = []
    for k in range(NCORES):
        gl_parts, sc_parts = [], []
        for c, size, (es, ed) in core_bins[k]:
            m = es.shape[0]
            g = np.zeros(size, dtype=np.int64)
            s = np.full(size, cfg.SZ, dtype=np.int64)  # core 0 junk-tile row
            order = np.argsort(es, kind="stable")  # src-sorted for locality
            g[:m] = es[order]
            s[:m] = ed[order]
            gl_parts.append(_wrap16(g, size))
            sc_parts.append(_wrap16(s, size))
        gidx = np.concatenate(gl_parts, axis=1)
        sidx = np.concatenate(sc_parts, axis=1)

        lo, hi = k * SL, min((k + 1) * SL, n)
        nsl = hi - lo
        xsl = np.zeros((SLP, D), dtype=np.float32)
        xsl[:nsl] = x[lo:hi]
        xsl_pm = np.ascontiguousarray(
            xsl.reshape(NT, 128, D).transpose(1, 0, 2).reshape(128, NT * D)
        )
        dsl = np.zeros(SLP, dtype=np.float32)
        dsl[:nsl] = dinv[lo:hi]
        dinv_in = dsl.reshape(NT, 128).T.copy()

        # R_k^T [SLP, 64]: R_kT[u, g] = sum_{edges (k*SL+u) -> w} P[g,w]*dinv_w
        #                             + P[g, k*SL+u]*dinv_{k*SL+u}
        sel = owner == k
        rkt = np.zeros((SLP, NUM_GRAPHS), dtype=np.float32)
        np.add.at(rkt, (src_local[sel], batch[dst[sel]]), pd[dst[sel]])
        rkt[np.arange(nsl), batch[lo:hi]] += pd[lo:hi]
        rkt_pm = np.ascontiguousarray(
            rkt.reshape(NT, 128, NUM_GRAPHS).transpose(1, 0, 2).reshape(128, -1)
        )

        p1 = np.zeros((1, NUM_GRAPHS), dtype=np.float32)
        np.add.at(p1[0], batch[lo:hi], w_graph[batch[lo:hi]])

        in_maps.append({
            "xsl": xsl_pm,
            "dinv_in": dinv_in,
            "gidx": np.ascontiguousarray(gidx),
            "sidx": np.ascontiguousarray(sidx),
            "rkt": rkt_pm,
            "p1": p1,
            "w1": W1,
            "b1": b1.reshape(D, 1),
            "ga": gamma.reshape(D, 1),
            "be": beta.reshape(D, 1),
            "w2": W2,
            "b2": b2.reshape(1, D),
        })
    return in_maps


def kernel(x, edge_index, batch, W1, b1, gamma, beta, W2, b2):
    global LAST_EXEC_TIME_NS
    from concourse.bass_utils import run_bass_kernel_spmd

    cfg = Cfg(N_NODES, N_NODES // NCORES)
    in_maps = prepare_inputs(cfg, x, edge_index, batch, W1, b1, gamma, beta, W2, b2)

    key = (cfg.N, cfg.SL, tuple(cfg.seg))
    if key not in _NC_CACHE:
        _NC_CACHE[key] = build(cfg)
    nc = _NC_CACHE[key]
    global _LAST_IN_MAPS
    _LAST_IN_MAPS = in_maps

    trace = bool(int(os.environ.get("BASS_GNN_TRACE", "0")))
    if trace:
        try:
            res = run_bass_kernel_spmd(nc, in_maps, list(range(NCORES)), trace=True)
        except Exception:
            res = run_bass_kernel_spmd(nc, in_maps, list(range(NCORES)), trace=False)
    else:
        res = run_bass_kernel_spmd(nc, in_maps, list(range(NCORES)), trace=False)
    LAST_EXEC_TIME_NS = res.exec_time_ns
    return np.asarray(res.results[0]["out"], dtype=np.float32)


def modeled_time_ns(x=None, edge_index=None, **kw):
    """Cost-model execution time (MultiCoreSim, mocked collectives) for the
    current cached program; used when NTFF tracing is unavailable."""
    if not _NC_CACHE:
        return None
    nc = next(iter(_NC_CACHE.values()))
    ins = _LAST_IN_MAPS
    if ins is None:
        return None
    from concourse.bass_interp import MultiCoreSim

    sim = MultiCoreSim(nc, 2, debug_mock_collectives_without_correctness=True)
    for i, core in sim.cores.items():
        for name, val in ins[i].items():
            core.tensor(name)[:] = val
    sim.simulate()
    return int(sim.global_time)


# revision 58
# speedup vs baseline: 1.4424x; 1.0060x over previous
# GCN (2-layer GCNConv + BatchNorm + ReLU + global mean pool) on 8 TRN2 NeuronCores.
#
# Math (reference):
#   deg[v]  = in-degree incl. self-loop;  dinv = deg^-1/2
#   layer(x, W, b): h = D^-1/2 (A+I) D^-1/2 (x W) + b
#   h1 = relu(batchnorm(layer1));  h2 = layer2(h1);  out = segment_mean(h2, batch)
#
# Sharding (v4 — source-partitioned edges + fp16 ReduceScatter):
#   Core k owns nodes [k*SL, (k+1)*SL) and all edges whose SRC falls in that
#   range.  Layer 1:
#     * xs = dinv * x (own slice) -> local f32 gather table (DRAM); the edge
#       gather needs NO collective at all.
#     * per-edge: dma_gather xs[src] rows from the local table (f32, 256B
#       elems), convert the message tile to fp16 on DVE (hidden behind the
#       Pool-engine gather/scatter stream), then dma_scatter_add into a
#       global fp16 accumulator at the dst row.  The accumulator packs two
#       nodes per 256B row (scatter rows need 256B stride); node (p, g) of
#       core k lives at row k*SZJ + p*(NT/2) + g//2, column half g%2, so
#       scatter instructions are split by tile parity.
#     * one fp16 ReduceScatter hands each core the reduced rows of its own
#       slice (half the bytes of f32 — collective cost tracks output size).
#     * self-loops are folded in AFTER the ReduceScatter as one vector add
#       (z + xs) instead of 12.5k extra scatter slots.
#   BatchNorm stats via an accumulated A^T[A|1] matmul + algebraic reduction.
#   The tiny [64,65] stats reduction and the final [64,64] output reduction
#   use AllGather + local vector adds (cheaper than AllReduce).
#   Layer 2 + pooling collapse into dense matmuls: mean-pool P and the outer
#   D^-1/2 are linear, so out = sum_k (R_k @ xs2_k) W2 + b2 with
#   R_k[g, u] = sum_{edges u->w owned by k} P[g,w] dinv_w (+ self term),
#   built on the host from pure index data.  No second edge phase, no second
#   table, no second big collective.
#
# dma_scatter_add races (loses updates) for duplicate dst rows within one
# instruction, so edges are packed into instruction "bins" with unique dst
# rows per bin via rotation binning: occurrence o of dst row r goes to bin
# (r + o) % nbins.  The accumulator has 26112 rows, so scatter indices fit
# int16 with no bucketing.  Pad slots gather row 0 and scatter into a dead
# junk row (a reserved 64-row tile per core block).
#
# Host-side preprocessing uses only index data (edge_index, batch): degree
# computation, edge partitioning/binning, the R_k pooling matrices.  Feature
# data is never touched on the host.

import os

import numpy as np

N_NODES = 50000
N_EDGES = 800000
D = 64
NCORES = 8
NUM_GRAPHS = 64
BN_EPS = 1e-5


class Cfg:
    def __init__(self, n, sl):
        self.N = n                    # total nodes
        self.SL = sl                  # owned nodes per core
        slp = ((sl + 127) // 128) * 128
        if (slp // 128) % 2:
            slp += 128                # even tile count (node-pair packing)
        self.SLP = slp
        assert self.SL < self.SLP
        self.NT = self.SLP // 128     # 128-row node tiles per slice (even)
        self.HT = self.NT // 2
        self.SZ = 128 * self.HT       # real acc rows per core
        self.SZJ = self.SZ + 64       # + junk pair-tile
        self.CAP = 7680               # max slots per gather/scatter instruction
        # per-instruction (parity, padded slot count); filled by prepare_inputs
        self.seg = []


LAST_EXEC_TIME_NS = None
_NC_CACHE = {}
_LAST_IN_MAPS = None


def build(cfg):
    import concourse.mybir as mybir
    import concourse.tile as tile
    from concourse import bacc
    from concourse.bass import BassGpSimd
    from concourse.masks import make_identity

    f32 = mybir.dt.float32
    f16 = mybir.dt.float16
    i16 = mybir.dt.int16
    SLP, NT = cfg.SLP, cfg.NT
    NN = float(cfg.N)
    RG = [list(range(NCORES))]
    segs = cfg.seg
    tot_s = sum(c for _, c in segs)
    ACC_R = NCORES * cfg.SZJ

    nc = bacc.Bacc(
        "TRN2", target_bir_lowering=False, debug=False, num_devices=NCORES
    )

    # --- external inputs (per-core values supplied via in_maps) ---
    xsl = nc.declare_dram_parameter("xsl", [128, NT * D], f32, isOutput=False)
    dinv_in = nc.declare_dram_parameter("dinv_in", [128, NT], f32, isOutput=False)
    gidx_d = nc.declare_dram_parameter("gidx", [128, tot_s // 16], i16, isOutput=False)
    sidx_d = nc.declare_dram_parameter("sidx", [128, tot_s // 16], i16, isOutput=False)
    rkt_d = nc.declare_dram_parameter("rkt", [128, NT * D], f32, isOutput=False)
    p1_d = nc.declare_dram_parameter("p1", [1, NUM_GRAPHS], f32, isOutput=False)
    w1_d = nc.declare_dram_parameter("w1", [D, D], f32, isOutput=False)
    b1_d = nc.declare_dram_parameter("b1", [D, 1], f32, isOutput=False)
    ga_d = nc.declare_dram_parameter("ga", [D, 1], f32, isOutput=False)
    be_d = nc.declare_dram_parameter("be", [D, 1], f32, isOutput=False)
    w2_d = nc.declare_dram_parameter("w2", [D, D], f32, isOutput=False)
    b2_d = nc.declare_dram_parameter("b2", [1, D], f32, isOutput=False)
    out_d = nc.declare_dram_parameter("out", [NUM_GRAPHS, D], f32, isOutput=True)

    # --- internal DRAM ---
    table1 = nc.dram_tensor("table1", [SLP, D], f32)
    acc = nc.dram_tensor("acc", [ACC_R, 2 * D], f16)
    rs_out = nc.dram_tensor("rs_out", [cfg.SZJ, 2 * D], f16)
    ags_in = nc.dram_tensor("ags_in", [D, D + 1], f16)
    ags_out = nc.dram_tensor("ags_out", [NCORES * D, D + 1], f16, addr_space="Shared")
    ago_in = nc.dram_tensor("ago_in", [NUM_GRAPHS, D], f16)
    ago_out = nc.dram_tensor("ago_out", [NCORES * NUM_GRAPHS, D], f16, addr_space="Shared")

    cc_eng = os.environ.get("GNN_CC_ENG", "pool")

    def cc(kind, op, ins_ap, outs_ap):
        BassGpSimd.collective_compute(
            nc.gpsimd if cc_eng == "pool" else getattr(nc, cc_eng),
            kind, op, replica_groups=RG, ins=[ins_ap], outs=[outs_ap],
        )

    with tile.TileContext(nc) as tc:
        with (
            tc.tile_pool(name="const", bufs=1) as const,
            tc.tile_pool(name="persist", bufs=1) as persist,
            tc.tile_pool(name="work", bufs=2) as work,
            tc.tile_pool(name="msgp", bufs=3) as msgp,
            tc.tile_pool(name="msghp", bufs=2) as msghp,
            tc.tile_pool(name="spsum", bufs=1, space="PSUM") as spsum,
            tc.tile_pool(name="wpsum", bufs=2, space="PSUM") as wpsum,
            tc.tile_pool(name="fpsum", bufs=2, space="PSUM") as fpsum,
        ):
            ablate = os.environ.get("GNN_ABLATE", "")

            # --- zero tile for accumulator init (fp16) ---
            ZW = ACC_R * 2 * D // 8 // 128
            zt = persist.tile([128, ZW], f16, name="zt")
            nc.vector.memset(zt[:], 0.0)

            # --- phase A inputs: x slice halves in separate tiles so the
            #     scale/table pipeline isn't serialized by whole-tensor deps ---
            HN = NT // 2
            xs_a = persist.tile([128, HN, D], f32, name="xs_a")
            xs_b = persist.tile([128, NT - HN, D], f32, name="xs_b")
            xsl_v = xsl[:, :].rearrange("p (g d) -> p g d", d=D)
            nc.sync.dma_start(out=xs_a[:], in_=xsl_v[:, :HN, :])
            nc.sync.dma_start(out=xs_b[:], in_=xsl_v[:, HN:, :])
            gidx_t = persist.tile([128, tot_s // 16], i16, name="gidx_t")
            nc.scalar.dma_start(out=gidx_t[:], in_=gidx_d[:, :])

            # --- constants into SBUF (Pool is idle until the first gather);
            #     dinvs first: the phase-A scale waits on it ---
            dinvs = const.tile([128, NT], f32)
            nc.gpsimd.dma_start(out=dinvs[:], in_=dinv_in[:, :])
            w1s = const.tile([D, D], f32)
            nc.gpsimd.dma_start(out=w1s[:], in_=w1_d[:, :])
            w2s = const.tile([D, D], f32)
            nc.gpsimd.dma_start(out=w2s[:], in_=w2_d[:, :])
            b1c = const.tile([D, 1], f32)
            nc.gpsimd.dma_start(out=b1c[:], in_=b1_d[:, :])
            gac = const.tile([D, 1], f32)
            nc.gpsimd.dma_start(out=gac[:], in_=ga_d[:, :])
            bec = const.tile([D, 1], f32)
            nc.gpsimd.dma_start(out=bec[:], in_=be_d[:, :])
            b2r = const.tile([1, D], f32)
            nc.gpsimd.dma_start(out=b2r[:], in_=b2_d[:, :])
            p1s = const.tile([1, NUM_GRAPHS], f32)
            nc.gpsimd.dma_start(out=p1s[:], in_=p1_d[:, :])

            # --- phase A: xs = dinv * x -> local gather table (split SP/Act),
            #     interleaved with the 8 accumulator zero chunks ---
            dinv_b = dinvs[:, :].rearrange("p (g o) -> p g o", o=1).to_broadcast(
                [128, NT, D]
            )
            nc.vector.tensor_tensor(
                out=xs_a[:], in0=xs_a[:],
                in1=dinvs[:, :HN].rearrange("p (g o) -> p g o", o=1)
                .to_broadcast([128, HN, D]),
                op=mybir.AluOpType.mult,
            )
            nc.vector.tensor_tensor(
                out=xs_b[:], in0=xs_b[:],
                in1=dinvs[:, HN:].rearrange("p (g o) -> p g o", o=1)
                .to_broadcast([128, NT - HN, D]),
                op=mybir.AluOpType.mult,
            )
            # fp16 copies of xs / dinv (and identity) for the packed-DVE
            # post-RS path; built early so they hide under the edge phase
            xs16 = persist.tile([128, NT, D], f16, name="xs16")
            nc.vector.tensor_copy(out=xs16[:, :HN, :], in_=xs_a[:])
            nc.vector.tensor_copy(out=xs16[:, HN:, :], in_=xs_b[:])
            dinv16 = persist.tile([128, NT, D], f16, name="dinv16")
            nc.vector.tensor_copy(out=dinv16[:], in_=dinv_b)
            acc_flat = acc[:, :].rearrange("n d -> (n d)")

            def zchunk(eng, j):
                ap = acc_flat.rearrange("(j p x) -> j p x", j=8, p=128)[j]
                eng.dma_start(out=ap, in_=zt[:, :])

            tview = table1[:, :].rearrange("(g p) d -> p g d", p=128)
            zchunk(nc.gpsimd, 0)
            zchunk(nc.gpsimd, 1)
            zchunk(nc.gpsimd, 2)
            zchunk(nc.gpsimd, 3)
            nc.sync.dma_start(out=tview[:, :HN, :], in_=xs_a[:])
            nc.scalar.dma_start(out=tview[:, HN:, :], in_=xs_b[:])
            sidx_t = persist.tile([128, tot_s // 16], i16, name="sidx_t")
            nc.scalar.dma_start(out=sidx_t[:], in_=sidx_d[:, :])
            zchunk(nc.sync, 4)
            zchunk(nc.sync, 5)
            zchunk(nc.scalar, 6)
            zchunk(nc.scalar, 7)

            # identity / BN constants / activation-table warmup — needed only
            # from phase D on, emitted after the edge-phase-critical work
            ident = const.tile([128, 128], f32)
            make_identity(nc, ident[:])
            ones64 = const.tile([D, 1], f32)
            nc.vector.memset(ones64[:], 1.0)
            epsc = const.tile([D, 1], f32)
            nc.vector.memset(epsc[:], BN_EPS)
            warm = const.tile([D, 1], f32)
            nc.scalar.activation(warm[:], epsc[:], mybir.ActivationFunctionType.Sqrt)
            nc.scalar.activation(warm[:], epsc[:], mybir.ActivationFunctionType.Relu)

            # --- R_k^T for layer 2 (loaded during the edge phase) ---
            rkt_t = persist.tile([128, NT, D], f32, name="rkt_t")
            nc.sync.dma_start(
                out=rkt_t[:], in_=rkt_d[:, :].rearrange("p (g d) -> p g d", d=D)
            )

            # --- edge phase: gather f32 / convert fp16 / scatter-add fp16 ---
            import concourse.mybir as mb

            if "noedge" not in ablate:
                pend = None
                off = 0
                for c, cnt in segs:
                    msg = msgp.tile([128, cfg.CAP // 128, D], f32, tag="msg",
                                    name="msg")
                    nc.gpsimd.dma_gather(
                        out_ap=msg[:, : cnt // 128, :],
                        in_ap=table1[0:SLP, :],
                        idxs_ap=gidx_t[:, off : off + cnt // 16],
                        num_idxs=cnt, num_idxs_reg=cnt, elem_size=D,
                        single_packet=False, queue_num=0,
                    )
                    msgh = msghp.tile([128, cfg.CAP // 128, D], f16, tag="msgh",
                                      name="msgh")
                    nc.vector.tensor_copy(
                        out=msgh[:, : cnt // 128, :], in_=msg[:, : cnt // 128, :]
                    )
                    if pend is not None:
                        nc.gpsimd.dma_scatter_add(*pend, elem_step=2 * D, single_packet=False, queue_num=0)
                    pend = (
                        acc[:, c * D : (c + 1) * D],
                        msgh[:, : cnt // 128, :],
                        sidx_t[:, off : off + cnt // 16],
                        cnt, cnt, D,
                    )
                    off += cnt // 16
                if pend is not None:
                    nc.gpsimd.dma_scatter_add(*pend, elem_step=2 * D, single_packet=False, queue_num=0)

            # --- ReduceScatter -> own reduced slice (fp16) ---
            if "nocc" not in ablate:
                cc("ReduceScatter", mybir.AluOpType.add, acc[:, :], rs_out[:, :])
            else:
                nc.sync.dma_start(out=rs_out[:, :], in_=acc[0 : cfg.SZJ, :])

            # warm the PE p-state during the collective (it idles otherwise,
            # and the first post-RS matmuls would run at the slow p-state)
            warm_ps = fpsum.tile([D, D], f32, tag="ps_c", name="warm_ps")
            for _ in range(24):
                nc.tensor.matmul(
                    out=warm_ps[:], lhsT=ident[:D, :D], rhs=ident[:D, :D],
                    start=True, stop=True,
                )

            # --- dense layer-1: z + self-loop, dinv scale, stats, W1 ---
            z_t = persist.tile([128, NT, D], f16, name="z_t")
            zsrc = rs_out[0 : cfg.SZ, :].rearrange("(p r) c -> p (r c)", p=128)
            zdst = z_t[:].rearrange("p g d -> p (g d)")
            HW_ = NT * D // 2
            nc.sync.dma_start(out=zdst[:, :HW_], in_=zsrc[:, :HW_])
            nc.sync.dma_start(out=zdst[:, HW_:], in_=zsrc[:, HW_:])
            # keep the PE p-state warm through the z load so the stats
            # matmuls run at full clock
            for _ in range(10):
                nc.tensor.matmul(
                    out=warm_ps[:], lhsT=z_t[:, 0, :], rhs=z_t[:, 0, :],
                    start=True, stop=True,
                )
            aggs = persist.tile([128, NT, D + 1], f16, name="aggs")
            nc.vector.memset(aggs[:, :, D : D + 1], 1.0)
            ident16 = persist.tile([128, 128], f16, name="ident16")
            nc.vector.tensor_copy(out=ident16[:], in_=ident[:])
            w1s16 = persist.tile([D, D], f16, name="w1s16")
            nc.vector.tensor_copy(out=w1s16[:], in_=w1s[:])

            stats_ps = spsum.tile([D, D + 1], f32, name="stats_ps")
            hT_big = persist.tile([D, NT * 128], f32, name="hT_big")
            ND = NT if "noD" not in ablate else 1
            # pipeline the (z+xs)*dinv prep with the stats matmuls per chunk;
            # everything fp16 so the DVE runs in its packed 2x mode
            CH = 10
            for b0 in range(0, ND, CH):
                bn = min(CH, ND - b0)
                sl = slice(b0, b0 + bn)
                nc.vector.tensor_tensor(
                    out=aggs[:, sl, :D], in0=z_t[:, sl, :], in1=xs16[:, sl, :],
                    op=mybir.AluOpType.add,
                )
                nc.vector.tensor_tensor(
                    out=aggs[:, sl, :D], in0=aggs[:, sl, :D],
                    in1=dinv16[:, sl, :], op=mybir.AluOpType.mult,
                )
                for b in range(b0, b0 + bn):
                    nc.tensor.matmul(
                        out=stats_ps[:], lhsT=aggs[:, b, :D], rhs=aggs[:, b, :],
                        start=(b == 0), stop=(b == ND - 1),
                    )
            # stats AllGather launched before the transposes/W1 matmuls so the
            # collective overlaps with PE work
            stats_sb = persist.tile([D, D + 1], f16, name="stats_sb")
            nc.vector.tensor_copy(out=stats_sb[:], in_=stats_ps[:])
            nc.sync.dma_start(out=ags_in[:, :], in_=stats_sb[:])
            if "nocc" not in ablate:
                cc("AllGather", mybir.AluOpType.bypass, ags_in[:, :], ags_out[:, :])
            else:
                nc.sync.dma_start(out=ags_out[0:D, :], in_=ags_in[:, :])

            for b0 in range(0, ND, 4):
                bn = min(4, ND - b0)
                tp_ps = wpsum.tile([D, 512], f16, tag="ps_a", name="tp_ps")
                for j in range(bn):
                    nc.tensor.transpose(
                        out=tp_ps[:, j * 128 : (j + 1) * 128],
                        in_=aggs[:, b0 + j, :D], identity=ident16[:],
                    )
                aggsT = work.tile([D, 512], f16, tag="aggsT", name="aggsT", bufs=2)
                nc.vector.tensor_copy(out=aggsT[:, : bn * 128], in_=tp_ps[:, : bn * 128])
                hT_ps = wpsum.tile([D, 512], f32, tag="ps_b", name="hT_ps")
                nc.tensor.matmul(
                    out=hT_ps[:, : bn * 128], lhsT=w1s16[:], rhs=aggsT[:, : bn * 128],
                    start=True, stop=True,
                )
                nc.vector.tensor_copy(
                    out=hT_big[:, b0 * 128 : (b0 + bn) * 128],
                    in_=hT_ps[:, : bn * 128],
                )

            stg = persist.tile([D, NCORES, D + 1], f16, name="stg")
            nc.scalar.dma_start(
                out=stg[:], in_=ags_out[:, :].rearrange("(j p) c -> p j c", p=D)
            )
            for h in (4, 2, 1):
                nc.vector.tensor_tensor(
                    out=stg[:, 0:h, :], in0=stg[:, 0:h, :],
                    in1=stg[:, h : 2 * h, :], op=mybir.AluOpType.add,
                )
            st32 = persist.tile([D, D + 1], f32, name="st32")
            nc.vector.tensor_copy(out=st32[:], in_=stg[:, 0, :])
            st = st32[:]

            # --- BN scalar algebra ---
            q_ps = wpsum.tile([D, 1], f32, tag="ps_a", name="q_ps")
            nc.tensor.matmul(out=q_ps[:], lhsT=w1s[:], rhs=st[:, D : D + 1], start=True, stop=True)
            mu = persist.tile([D, 1], f32, name="mu")
            nc.vector.tensor_scalar(
                out=mu[:], in0=q_ps[:], scalar1=1.0 / NN, scalar2=b1c[:],
                op0=mybir.AluOpType.mult, op1=mybir.AluOpType.add,
            )
            t1_ps = wpsum.tile([D, D], f32, tag="ps_b", name="t1_ps")
            nc.tensor.matmul(out=t1_ps[:], lhsT=st[:, :D], rhs=w1s[:], start=True, stop=True)
            m_sb = work.tile([D, D], f32, tag="m_sb", name="m_sb")
            nc.vector.tensor_tensor(out=m_sb[:], in0=w1s[:], in1=t1_ps[:], op=mybir.AluOpType.mult)
            d_ps = wpsum.tile([D, 1], f32, tag="ps_b", name="d_ps")
            nc.tensor.matmul(out=d_ps[:], lhsT=m_sb[:], rhs=ones64[:], start=True, stop=True)

            var = persist.tile([D, 1], f32, name="var")
            nc.vector.tensor_scalar_mul(out=var[:], in0=d_ps[:], scalar1=1.0 / NN)
            t2 = work.tile([D, 1], f32, tag="t2", name="t2")
            nc.vector.tensor_scalar_mul(out=t2[:], in0=q_ps[:], scalar1=2.0 / NN)
            nc.vector.tensor_tensor(out=t2[:], in0=t2[:], in1=b1c[:], op=mybir.AluOpType.mult)
            nc.vector.tensor_tensor(out=var[:], in0=var[:], in1=t2[:], op=mybir.AluOpType.add)
            t3 = work.tile([D, 1], f32, tag="t3", name="t3")
            nc.vector.tensor_tensor(out=t3[:], in0=b1c[:], in1=b1c[:], op=mybir.AluOpType.mult)
            nc.vector.tensor_tensor(out=var[:], in0=var[:], in1=t3[:], op=mybir.AluOpType.add)
            t4 = work.tile([D, 1], f32, tag="t4", name="t4")
            nc.vector.tensor_tensor(out=t4[:], in0=mu[:], in1=mu[:], op=mybir.AluOpType.mult)
            nc.vector.tensor_tensor(out=var[:], in0=var[:], in1=t4[:], op=mybir.AluOpType.subtract)

            sd = work.tile([D, 1], f32, tag="sd", name="sd")
            nc.scalar.activation(sd[:], var[:], mb.ActivationFunctionType.Sqrt, bias=epsc[:])
            rstd = work.tile([D, 1], f32, tag="rstd", name="rstd")
            nc.vector.reciprocal(out=rstd[:], in_=sd[:])
            a_sb = persist.tile([D, 1], f32, name="a_sb")
            nc.vector.tensor_tensor(out=a_sb[:], in0=gac[:], in1=rstd[:], op=mybir.AluOpType.mult)
            c_sb = persist.tile([D, 1], f32, name="c_sb")
            t5 = work.tile([D, 1], f32, tag="t5", name="t5")
            nc.vector.tensor_tensor(out=t5[:], in0=mu[:], in1=a_sb[:], op=mybir.AluOpType.mult)
            nc.vector.tensor_tensor(out=c_sb[:], in0=bec[:], in1=t5[:], op=mybir.AluOpType.subtract)
            # hT tiles exclude the b1 bias; fold it into the BN offset:
            # relu(a*(h+b1) + c) = relu(a*h + (c + a*b1))
            t6 = work.tile([D, 1], f32, tag="t6", name="t6")
            nc.vector.tensor_tensor(out=t6[:], in0=a_sb[:], in1=b1c[:], op=mybir.AluOpType.mult)
            nc.vector.tensor_tensor(out=c_sb[:], in0=c_sb[:], in1=t6[:], op=mybir.AluOpType.add)

            # --- phase F: BN+ReLU, transpose back, dinv fold -> xs2;
            #     phase G interleaved: poolT += xs2_b^T @ R_b ---
            poolT_ps = spsum.tile([D, NUM_GRAPHS], f32, name="poolT_ps")
            xs2 = persist.tile([128, NT, D], f32, name="xs2")
            NF = NT if "noF" not in ablate else 0
            for b0 in range(0, NF, 4):
                bn = min(4, NF - b0)
                h1T = work.tile([D, 512], f32, tag="h1T", name="h1T", bufs=3)
                nc.scalar.activation(
                    h1T[:, : bn * 128],
                    hT_big[:, b0 * 128 : (b0 + bn) * 128],
                    mb.ActivationFunctionType.Relu,
                    bias=c_sb[:], scale=a_sb[:],
                )
                nm_ps = fpsum.tile([128, 4 * D], f32, tag="ps_c", name="nm_ps")
                for j in range(bn):
                    nc.tensor.transpose(
                        out=nm_ps[:, j * D : (j + 1) * D],
                        in_=h1T[:, j * 128 : (j + 1) * 128],
                        identity=ident[:D, :D],
                    )
                nc.vector.tensor_tensor(
                    out=xs2[:, b0 : b0 + bn, :], in0=nm_ps[:, : bn * D].rearrange(
                        "p (g d) -> p g d", d=D),
                    in1=dinvs[:, b0 : b0 + bn].rearrange(
                        "p (g o) -> p g o", o=1).to_broadcast([128, bn, D]),
                    op=mybir.AluOpType.mult,
                )
                for j in range(bn):
                    b = b0 + j
                    nc.tensor.matmul(
                        out=poolT_ps[:], lhsT=xs2[:, b, :], rhs=rkt_t[:, b, :],
                        start=(b == 0), stop=(b == NF - 1),
                    )

            # --- output: pool @ W2 + p1^T b2; AllGather + local reduce ---
            poolT_sb = persist.tile([D, NUM_GRAPHS], f32, name="poolT_sb")
            nc.vector.tensor_copy(out=poolT_sb[:], in_=poolT_ps[:])
            out_ps = wpsum.tile([NUM_GRAPHS, D], f32, tag="ps_a", name="out_ps")
            nc.tensor.matmul(out=out_ps[:], lhsT=poolT_sb[:], rhs=w2s[:], start=True, stop=False)
            nc.tensor.matmul(out=out_ps[:], lhsT=p1s[:], rhs=b2r[:], start=False, stop=True)
            out_sb = persist.tile([NUM_GRAPHS, D], f16, name="out_sb")
            nc.vector.tensor_copy(out=out_sb[:], in_=out_ps[:])
            nc.sync.dma_start(out=ago_in[:, :], in_=out_sb[:])
            if "nocc" not in ablate:
                cc("AllGather", mybir.AluOpType.bypass, ago_in[:, :], ago_out[:, :])
            else:
                nc.sync.dma_start(out=ago_out[0:NUM_GRAPHS, :], in_=ago_in[:, :])
            og = persist.tile([NUM_GRAPHS, NCORES, D], f16, name="og")
            nc.scalar.dma_start(
                out=og[:],
                in_=ago_out[:, :].rearrange("(j p) d -> p j d", p=NUM_GRAPHS),
            )
            for h in (4, 2, 1):
                nc.vector.tensor_tensor(
                    out=og[:, 0:h, :], in0=og[:, 0:h, :],
                    in1=og[:, h : 2 * h, :], op=mybir.AluOpType.add,
                )
            og32 = persist.tile([NUM_GRAPHS, D], f32, name="og32")
            nc.vector.tensor_copy(out=og32[:], in_=og[:, 0, :])
            nc.sync.dma_start(out=out_d[:, :], in_=og32[:])

    nc.compile()
    return nc


def _wrap16(v, n):
    """idx j at [j%16, j//16], replicated to 128 partitions (8 Q7 cores)."""
    assert v.shape[0] == n and n % 16 == 0
    t = v.astype(np.int16).reshape(n // 16, 16).T
    return np.tile(t, (8, 1))


def _bin_edges(gsrc, grow, nbins, nspill):
    """Rotation binning: occurrence o of dst row r -> bin (r + o) % nbins for
    o < nbins; higher occurrences spill into one extra bin per occurrence
    level (occurrence levels have unique rows by construction)."""
    order = np.argsort(grow, kind="stable")
    sd, ss = grow[order], gsrc[order]
    out = [(np.zeros(0, np.int64), np.zeros(0, np.int64))] * (nbins + nspill)
    if sd.shape[0] == 0:
        return out
    change = np.r_[True, sd[1:] != sd[:-1]]
    starts = np.flatnonzero(change)
    gid = np.cumsum(change) - 1
    occ = np.arange(sd.shape[0]) - starts[gid]
    assert int(occ.max()) < nbins + nspill, (int(occ.max()), nbins, nspill)
    b = np.where(occ < nbins, (sd + occ) % nbins, occ)
    return [(ss[b == i], sd[b == i]) for i in range(nbins + nspill)]


def prepare_inputs(cfg, x, edge_index, batch, W1, b1, gamma, beta, W2, b2):
    """Host-side index preprocessing + per-core input maps.  Fills cfg.seg."""
    SL, SLP, NT = cfg.SL, cfg.SLP, cfg.NT
    n = cfg.N

    x = np.ascontiguousarray(np.asarray(x, dtype=np.float32))
    src = np.asarray(edge_index[0], dtype=np.int64)
    dst = np.asarray(edge_index[1], dtype=np.int64)
    batch = np.asarray(batch, dtype=np.int64)
    W1 = np.asarray(W1, dtype=np.float32)
    b1 = np.asarray(b1, dtype=np.float32)
    gamma = np.asarray(gamma, dtype=np.float32)
    beta = np.asarray(beta, dtype=np.float32)
    W2 = np.asarray(W2, dtype=np.float32)
    b2 = np.asarray(b2, dtype=np.float32)

    deg = np.bincount(dst, minlength=n).astype(np.float32) + 1.0  # + self-loop
    dinv = (1.0 / np.sqrt(deg)).astype(np.float32)

    cnt = np.bincount(batch, minlength=NUM_GRAPHS).astype(np.float32)
    w_graph = 1.0 / np.maximum(cnt, 1.0)
    pd = w_graph[batch] * dinv          # P[batch[v], v] * dinv_v  per node

    owner = src // SL
    src_local = src - owner * SL

    # dst -> (parity, acc row): node (p, g) of core k ->
    # row k*SZJ + p*HT + g//2, column half g%2
    d_owner = dst // SL
    d_local = dst - d_owner * SL
    d_g = d_local // 128
    d_p = d_local - d_g * 128
    d_par = d_g % 2
    d_row = d_owner * cfg.SZJ + d_p * cfg.HT + d_g // 2

    per_core = [[None, None] for _ in range(NCORES)]
    for k in range(NCORES):
        sel = owner == k
        es, ed, ec = src_local[sel], d_row[sel], d_par[sel]
        for c in (0, 1):
            m = ec == c
            per_core[k][c] = (es[m], ed[m])

    # shared bin layout per parity
    seg, core_bins = [], [[] for _ in range(NCORES)]
    for c in (0, 1):
        counts = [per_core[k][c][0].shape[0] for k in range(NCORES)]
        mm = 1
        for k in range(NCORES):
            rows = per_core[k][c][1]
            if rows.shape[0]:
                mm = max(mm, int(np.bincount(rows).max()))
        nbins = max(-(-max(counts) // (cfg.CAP - 256)), 1)
        while True:
            nspill = max(mm - nbins, 0)
            allb = [
                _bin_edges(per_core[k][c][0], per_core[k][c][1], nbins, nspill)
                for k in range(NCORES)
            ]
            sizes = [
                ((max(allb[k][i][0].shape[0] for k in range(NCORES)) + 127)
                 // 128) * 128
                for i in range(nbins + nspill)
            ]
            if all(s <= cfg.CAP for s in sizes):
                break
            nbins += 1
        for i in range(nbins + nspill):
            if sizes[i] == 0:
                continue
            seg.append((c, sizes[i]))
            for k in range(NCORES):
                core_bins[k].append((c, sizes[i], allb[k][i]))

    cfg.seg = seg

    in_maps = []
    for k in range(NCORES):
        gl_parts, sc_parts = [], []
        for c, size, (es, ed) in core_bins[k]:
            m = es.shape[0]
            g = np.zeros(size, dtype=np.int64)
            s = np.full(size, cfg.SZ, dtype=np.int64)  # core 0 junk-tile row
            order = np.argsort(es, kind="stable")  # src-sorted for locality
            g[:m] = es[order]
            s[:m] = ed[order]
            gl_parts.append(_wrap16(g, size))
            sc_parts.append(_wrap16(s, size))
        gidx = np.concatenate(gl_parts, axis=1)
        sidx = np.concatenate(sc_parts, axis=1)

        lo, hi = k * SL, min((k + 1) * SL, n)
        nsl = hi - lo
        xsl = np.zeros((SLP, D), dtype=np.float32)
        xsl[:nsl] = x[lo:hi]
        xsl_pm = np.ascontiguousarray(
            xsl.reshape(NT, 128, D).transpose(1, 0, 2).reshape(128, NT * D)
        )
        dsl = np.zeros(SLP, dtype=np.float32)
        dsl[:nsl] = dinv[lo:hi]
        dinv_in = dsl.reshape(NT, 128).T.copy()

        # R_k^T [SLP, 64]: R_kT[u, g] = sum_{edges (k*SL+u) -> w} P[g,w]*dinv_w
        #                             + P[g, k*SL+u]*dinv_{k*SL+u}
        sel = owner == k
        rkt = np.zeros((SLP, NUM_GRAPHS), dtype=np.float32)
        np.add.at(rkt, (src_local[sel], batch[dst[sel]]), pd[dst[sel]])
        rkt[np.arange(nsl), batch[lo:hi]] += pd[lo:hi]
        rkt_pm = np.ascontiguousarray(
            rkt.reshape(NT, 128, NUM_GRAPHS).transpose(1, 0, 2).reshape(128, -1)
        )

        p1 = np.zeros((1, NUM_GRAPHS), dtype=np.float32)
        np.add.at(p1[0], batch[lo:hi], w_graph[batch[lo:hi]])

        in_maps.append({
            "xsl": xsl_pm,
            "dinv_in": dinv_in,
            "gidx": np.ascontiguousarray(gidx),
            "sidx": np.ascontiguousarray(sidx),
            "rkt": rkt_pm,
            "p1": p1,
            "w1": W1,
            "b1": b1.reshape(D, 1),
            "ga": gamma.reshape(D, 1),
            "be": beta.reshape(D, 1),
            "w2": W2,
            "b2": b2.reshape(1, D),
        })
    return in_maps


def kernel(x, edge_index, batch, W1, b1, gamma, beta, W2, b2):
    global LAST_EXEC_TIME_NS
    from concourse.bass_utils import run_bass_kernel_spmd

    cfg = Cfg(N_NODES, N_NODES // NCORES)
    in_maps = prepare_inputs(cfg, x, edge_index, batch, W1, b1, gamma, beta, W2, b2)

    key = (cfg.N, cfg.SL, tuple(cfg.seg))
    if key not in _NC_CACHE:
        _NC_CACHE[key] = build(cfg)
    nc = _NC_CACHE[key]
    global _LAST_IN_MAPS
    _LAST_IN_MAPS = in_maps

    trace = bool(int(os.environ.get("BASS_GNN_TRACE", "0")))
    if trace:
        try:
            res = run_bass_kernel_spmd(nc, in_maps, list(range(NCORES)), trace=True)
        except Exception:
            res = run_bass_kernel_spmd(nc, in_maps, list(range(NCORES)), trace=False)
    else:
        res = run_bass_kernel_spmd(nc, in_maps, list(range(NCORES)), trace=False)
    LAST_EXEC_TIME_NS = res.exec_time_ns
    return np.asarray(res.results[0]["out"], dtype=np.float32)


def modeled_time_ns(x=None, edge_index=None, **kw):
    """Cost-model execution time (MultiCoreSim, mocked collectives) for the
    current cached program; used when NTFF tracing is unavailable."""
    if not _NC_CACHE:
        return None
    nc = next(iter(_NC_CACHE.values()))
    ins = _LAST_IN_MAPS
    if ins is None:
        return None
    from concourse.bass_interp import MultiCoreSim

    sim = MultiCoreSim(nc, 2, debug_mock_collectives_without_correctness=True)
    for i, core in sim.cores.items():
        for name, val in ins[i].items():
            core.tensor(name)[:] = val
    sim.simulate()
    return int(sim.global_time)


# revision 63
# speedup vs baseline: 1.4629x; 1.0142x over previous
# GCN (2-layer GCNConv + BatchNorm + ReLU + global mean pool) on 8 TRN2 NeuronCores.
#
# Math (reference):
#   deg[v]  = in-degree incl. self-loop;  dinv = deg^-1/2
#   layer(x, W, b): h = D^-1/2 (A+I) D^-1/2 (x W) + b
#   h1 = relu(batchnorm(layer1));  h2 = layer2(h1);  out = segment_mean(h2, batch)
#
# Sharding (v4 — source-partitioned edges + fp16 ReduceScatter):
#   Core k owns nodes [k*SL, (k+1)*SL) and all edges whose SRC falls in that
#   range.  Layer 1:
#     * xs = dinv * x (own slice) -> local f32 gather table (DRAM); the edge
#       gather needs NO collective at all.
#     * per-edge: dma_gather xs[src] rows from the local table (f32, 256B
#       elems), convert the message tile to fp16 on DVE (hidden behind the
#       Pool-engine gather/scatter stream), then dma_scatter_add into a
#       global fp16 accumulator at the dst row.  The accumulator packs two
#       nodes per 256B row (scatter rows need 256B stride); node (p, g) of
#       core k lives at row k*SZJ + p*(NT/2) + g//2, column half g%2, so
#       scatter instructions are split by tile parity.
#     * one fp16 ReduceScatter hands each core the reduced rows of its own
#       slice (half the bytes of f32 — collective cost tracks output size).
#     * self-loops are folded in AFTER the ReduceScatter as one vector add
#       (z + xs) instead of 12.5k extra scatter slots.
#   BatchNorm stats via an accumulated A^T[A|1] matmul + algebraic reduction.
#   The tiny [64,65] stats reduction and the final [64,64] output reduction
#   use AllGather + local vector adds (cheaper than AllReduce).
#   Layer 2 + pooling collapse into dense matmuls: mean-pool P and the outer
#   D^-1/2 are linear, so out = sum_k (R_k @ xs2_k) W2 + b2 with
#   R_k[g, u] = sum_{edges u->w owned by k} P[g,w] dinv_w (+ self term),
#   built on the host from pure index data.  No second edge phase, no second
#   table, no second big collective.
#
# dma_scatter_add races (loses updates) for duplicate dst rows within one
# instruction, so edges are packed into instruction "bins" with unique dst
# rows per bin via rotation binning: occurrence o of dst row r goes to bin
# (r + o) % nbins.  The accumulator has 26112 rows, so scatter indices fit
# int16 with no bucketing.  Pad slots gather row 0 and scatter into a dead
# junk row (a reserved 64-row tile per core block).
#
# Host-side preprocessing uses only index data (edge_index, batch): degree
# computation, edge partitioning/binning, the R_k pooling matrices.  Feature
# data is never touched on the host.

import os

import numpy as np

N_NODES = 50000
N_EDGES = 800000
D = 64
NCORES = 8
NUM_GRAPHS = 64
BN_EPS = 1e-5


class Cfg:
    def __init__(self, n, sl):
        self.N = n                    # total nodes
        self.SL = sl                  # owned nodes per core
        slp = ((sl + 127) // 128) * 128
        if (slp // 128) % 2:
            slp += 128                # even tile count (node-pair packing)
        self.SLP = slp
        assert self.SL < self.SLP
        self.NT = self.SLP // 128     # 128-row node tiles per slice (even)
        self.HT = self.NT // 2
        self.SZ = 128 * self.HT       # real acc rows per core
        self.SZJ = self.SZ + 64       # + junk pair-tile
        self.CAP = 7680               # max slots per gather/scatter instruction
        # per-instruction (parity, padded slot count); filled by prepare_inputs
        self.seg = []


LAST_EXEC_TIME_NS = None
_NC_CACHE = {}
_LAST_IN_MAPS = None


def build(cfg):
    import concourse.mybir as mybir
    import concourse.tile as tile
    from concourse import bacc
    from concourse.bass import BassGpSimd
    from concourse.masks import make_identity

    f32 = mybir.dt.float32
    f16 = mybir.dt.float16
    i16 = mybir.dt.int16
    SLP, NT = cfg.SLP, cfg.NT
    NN = float(cfg.N)
    RG = [list(range(NCORES))]
    segs = cfg.seg
    tot_s = sum(c for _, c in segs)
    ACC_R = NCORES * cfg.SZJ

    nc = bacc.Bacc(
        "TRN2", target_bir_lowering=False, debug=False, num_devices=NCORES
    )

    # --- external inputs (per-core values supplied via in_maps) ---
    xsl = nc.declare_dram_parameter("xsl", [128, NT * D], f32, isOutput=False)
    dinv_in = nc.declare_dram_parameter("dinv_in", [128, NT], f32, isOutput=False)
    gidx_d = nc.declare_dram_parameter("gidx", [128, tot_s // 16], i16, isOutput=False)
    sidx_d = nc.declare_dram_parameter("sidx", [128, tot_s // 16], i16, isOutput=False)
    rkt_d = nc.declare_dram_parameter("rkt", [128, NT * D], f32, isOutput=False)
    p1_d = nc.declare_dram_parameter("p1", [1, NUM_GRAPHS], f32, isOutput=False)
    w1_d = nc.declare_dram_parameter("w1", [D, D], f32, isOutput=False)
    b1_d = nc.declare_dram_parameter("b1", [D, 1], f32, isOutput=False)
    ga_d = nc.declare_dram_parameter("ga", [D, 1], f32, isOutput=False)
    be_d = nc.declare_dram_parameter("be", [D, 1], f32, isOutput=False)
    w2_d = nc.declare_dram_parameter("w2", [D, D], f32, isOutput=False)
    b2_d = nc.declare_dram_parameter("b2", [1, D], f32, isOutput=False)
    out_d = nc.declare_dram_parameter("out", [NUM_GRAPHS, D], f32, isOutput=True)

    # --- internal DRAM ---
    table1 = nc.dram_tensor("table1", [SLP, D], f32)
    acc = nc.dram_tensor("acc", [ACC_R, 2 * D], f16)
    rs_out = nc.dram_tensor("rs_out", [cfg.SZJ, 2 * D], f16)
    # reductions as replicated-input ReduceScatters: writing this core's
    # partial into all 8 input blocks makes the RS hand every core the full
    # sum (stats), or core 0 the full sum (output — the only core read)
    ags_in = nc.dram_tensor("ags_in", [NCORES * D, D + 1], f16)
    ags_out = nc.dram_tensor("ags_out", [D, D + 1], f16)
    ago_in = nc.dram_tensor("ago_in", [NCORES * NUM_GRAPHS, D], f16)
    ago_out = nc.dram_tensor("ago_out", [NUM_GRAPHS, D], f16)

    cc_eng = os.environ.get("GNN_CC_ENG", "pool")

    def cc(kind, op, ins_ap, outs_ap):
        BassGpSimd.collective_compute(
            nc.gpsimd if cc_eng == "pool" else getattr(nc, cc_eng),
            kind, op, replica_groups=RG, ins=[ins_ap], outs=[outs_ap],
        )

    with tile.TileContext(nc) as tc:
        with (
            tc.tile_pool(name="const", bufs=1) as const,
            tc.tile_pool(name="persist", bufs=1) as persist,
            tc.tile_pool(name="work", bufs=2) as work,
            tc.tile_pool(name="msgp", bufs=3) as msgp,
            tc.tile_pool(name="msghp", bufs=2) as msghp,
            tc.tile_pool(name="spsum", bufs=1, space="PSUM") as spsum,
            tc.tile_pool(name="wpsum", bufs=2, space="PSUM") as wpsum,
            tc.tile_pool(name="fpsum", bufs=2, space="PSUM") as fpsum,
        ):
            ablate = os.environ.get("GNN_ABLATE", "")

            # --- zero tile for accumulator init (fp16) ---
            ZW = ACC_R * 2 * D // 8 // 128
            zt = persist.tile([128, ZW], f16, name="zt")
            nc.vector.memset(zt[:], 0.0)

            # --- phase A inputs: x slice halves in separate tiles so the
            #     scale/table pipeline isn't serialized by whole-tensor deps ---
            HN = NT // 2
            xs_a = persist.tile([128, HN, D], f32, name="xs_a")
            xs_b = persist.tile([128, NT - HN, D], f32, name="xs_b")
            xsl_v = xsl[:, :].rearrange("p (g d) -> p g d", d=D)
            nc.sync.dma_start(out=xs_a[:], in_=xsl_v[:, :HN, :])
            nc.sync.dma_start(out=xs_b[:], in_=xsl_v[:, HN:, :])
            gidx_t = persist.tile([128, tot_s // 16], i16, name="gidx_t")
            nc.scalar.dma_start(out=gidx_t[:], in_=gidx_d[:, :])

            # --- constants into SBUF (Pool is idle until the first gather);
            #     dinvs first: the phase-A scale waits on it ---
            dinvs = const.tile([128, NT], f32)
            nc.gpsimd.dma_start(out=dinvs[:], in_=dinv_in[:, :])
            w1s = const.tile([D, D], f32)
            nc.gpsimd.dma_start(out=w1s[:], in_=w1_d[:, :])
            w2s = const.tile([D, D], f32)
            nc.gpsimd.dma_start(out=w2s[:], in_=w2_d[:, :])
            b1c = const.tile([D, 1], f32)
            nc.gpsimd.dma_start(out=b1c[:], in_=b1_d[:, :])
            gac = const.tile([D, 1], f32)
            nc.gpsimd.dma_start(out=gac[:], in_=ga_d[:, :])
            bec = const.tile([D, 1], f32)
            nc.gpsimd.dma_start(out=bec[:], in_=be_d[:, :])
            b2r = const.tile([1, D], f32)
            nc.gpsimd.dma_start(out=b2r[:], in_=b2_d[:, :])
            p1s = const.tile([1, NUM_GRAPHS], f32)
            nc.gpsimd.dma_start(out=p1s[:], in_=p1_d[:, :])

            # --- phase A: xs = dinv * x -> local gather table (split SP/Act),
            #     interleaved with the 8 accumulator zero chunks ---
            dinv_b = dinvs[:, :].rearrange("p (g o) -> p g o", o=1).to_broadcast(
                [128, NT, D]
            )
            nc.vector.tensor_tensor(
                out=xs_a[:], in0=xs_a[:],
                in1=dinvs[:, :HN].rearrange("p (g o) -> p g o", o=1)
                .to_broadcast([128, HN, D]),
                op=mybir.AluOpType.mult,
            )
            nc.vector.tensor_tensor(
                out=xs_b[:], in0=xs_b[:],
                in1=dinvs[:, HN:].rearrange("p (g o) -> p g o", o=1)
                .to_broadcast([128, NT - HN, D]),
                op=mybir.AluOpType.mult,
            )
            # fp16 copies of xs / dinv (and identity) for the packed-DVE
            # post-RS path; built early so they hide under the edge phase
            xs16 = persist.tile([128, NT, D], f16, name="xs16")
            nc.vector.tensor_copy(out=xs16[:, :HN, :], in_=xs_a[:])
            nc.vector.tensor_copy(out=xs16[:, HN:, :], in_=xs_b[:])
            dinv16 = persist.tile([128, NT, D], f16, name="dinv16")
            nc.vector.tensor_copy(out=dinv16[:], in_=dinv_b)
            acc_flat = acc[:, :].rearrange("n d -> (n d)")

            def zchunk(eng, j):
                ap = acc_flat.rearrange("(j p x) -> j p x", j=8, p=128)[j]
                eng.dma_start(out=ap, in_=zt[:, :])

            tview = table1[:, :].rearrange("(g p) d -> p g d", p=128)
            zchunk(nc.gpsimd, 0)
            zchunk(nc.gpsimd, 1)
            zchunk(nc.gpsimd, 2)
            zchunk(nc.gpsimd, 3)
            nc.sync.dma_start(out=tview[:, :HN, :], in_=xs_a[:])
            nc.scalar.dma_start(out=tview[:, HN:, :], in_=xs_b[:])
            # sidx in halves so the Act queue can slot the table write between
            sidx_t = persist.tile([128, tot_s // 16], i16, name="sidx_t")
            SH = (tot_s // 16) // 2
            nc.scalar.dma_start(out=sidx_t[:, :SH], in_=sidx_d[:, :SH])
            nc.scalar.dma_start(out=sidx_t[:, SH:], in_=sidx_d[:, SH:])
            zchunk(nc.sync, 4)
            zchunk(nc.sync, 5)
            zchunk(nc.scalar, 6)
            zchunk(nc.scalar, 7)
            # zero blocks 1..7 of the output-reduction RS input (core 0's
            # received chunk is the only one that matters)
            nc.sync.dma_start(
                out=ago_in[NUM_GRAPHS:, :].rearrange(
                    "(j p) d -> p j d", p=NUM_GRAPHS),
                in_=zt[0:NUM_GRAPHS, : (NCORES - 1) * D].rearrange(
                    "p (j d) -> p j d", d=D),
            )

            # identity / BN constants / activation-table warmup — needed only
            # from phase D on, emitted after the edge-phase-critical work
            ident = const.tile([128, 128], f32)
            make_identity(nc, ident[:])
            ones64 = const.tile([D, 1], f32)
            nc.vector.memset(ones64[:], 1.0)
            epsc = const.tile([D, 1], f32)
            nc.vector.memset(epsc[:], BN_EPS)
            warm = const.tile([D, 1], f32)
            nc.scalar.activation(warm[:], epsc[:], mybir.ActivationFunctionType.Sqrt)
            nc.scalar.activation(warm[:], epsc[:], mybir.ActivationFunctionType.Relu)

            # --- R_k^T for layer 2 (loaded during the edge phase) ---
            rkt_t = persist.tile([128, NT, D], f32, name="rkt_t")
            nc.sync.dma_start(
                out=rkt_t[:], in_=rkt_d[:, :].rearrange("p (g d) -> p g d", d=D)
            )

            # --- edge phase: gather f32 / convert fp16 / scatter-add fp16 ---
            import concourse.mybir as mb

            if "noedge" not in ablate:
                pend = None
                off = 0
                for c, cnt in segs:
                    msg = msgp.tile([128, cfg.CAP // 128, D], f32, tag="msg",
                                    name="msg")
                    nc.gpsimd.dma_gather(
                        out_ap=msg[:, : cnt // 128, :],
                        in_ap=table1[0:SLP, :],
                        idxs_ap=gidx_t[:, off : off + cnt // 16],
                        num_idxs=cnt, num_idxs_reg=cnt, elem_size=D,
                        single_packet=False, queue_num=0,
                    )
                    msgh = msghp.tile([128, cfg.CAP // 128, D], f16, tag="msgh",
                                      name="msgh")
                    nc.vector.tensor_copy(
                        out=msgh[:, : cnt // 128, :], in_=msg[:, : cnt // 128, :]
                    )
                    if pend is not None:
                        nc.gpsimd.dma_scatter_add(*pend, elem_step=2 * D, single_packet=False, queue_num=0)
                    pend = (
                        acc[:, c * D : (c + 1) * D],
                        msgh[:, : cnt // 128, :],
                        sidx_t[:, off : off + cnt // 16],
                        cnt, cnt, D,
                    )
                    off += cnt // 16
                if pend is not None:
                    nc.gpsimd.dma_scatter_add(*pend, elem_step=2 * D, single_packet=False, queue_num=0)

            # --- ReduceScatter -> own reduced slice (fp16) ---
            if "nocc" not in ablate:
                cc("ReduceScatter", mybir.AluOpType.add, acc[:, :], rs_out[:, :])
            else:
                nc.sync.dma_start(out=rs_out[:, :], in_=acc[0 : cfg.SZJ, :])

            # warm the PE p-state during the collective (it idles otherwise,
            # and the first post-RS matmuls would run at the slow p-state)
            warm_ps = fpsum.tile([D, D], f32, tag="ps_c", name="warm_ps")
            for _ in range(24):
                nc.tensor.matmul(
                    out=warm_ps[:], lhsT=ident[:D, :D], rhs=ident[:D, :D],
                    start=True, stop=True,
                )

            # --- dense layer-1: z + self-loop, dinv scale, stats, W1 ---
            z_t = persist.tile([128, NT, D], f16, name="z_t")
            zsrc = rs_out[0 : cfg.SZ, :].rearrange("(p r) c -> p (r c)", p=128)
            zdst = z_t[:].rearrange("p g d -> p (g d)")
            HW_ = NT * D // 2
            nc.sync.dma_start(out=zdst[:, :HW_], in_=zsrc[:, :HW_])
            nc.sync.dma_start(out=zdst[:, HW_:], in_=zsrc[:, HW_:])
            # keep the PE p-state warm through the z load so the stats
            # matmuls run at full clock
            for _ in range(10):
                nc.tensor.matmul(
                    out=warm_ps[:], lhsT=z_t[:, 0, :], rhs=z_t[:, 0, :],
                    start=True, stop=True,
                )
            aggs = persist.tile([128, NT, D + 1], f16, name="aggs")
            nc.vector.memset(aggs[:, :, D : D + 1], 1.0)
            ident16 = persist.tile([128, 128], f16, name="ident16")
            nc.vector.tensor_copy(out=ident16[:], in_=ident[:])
            w1s16 = persist.tile([D, D], f16, name="w1s16")
            nc.vector.tensor_copy(out=w1s16[:], in_=w1s[:])

            stats_ps = spsum.tile([D, D + 1], f32, name="stats_ps")
            hT_big = persist.tile([D, NT * 128], f32, name="hT_big")
            ND = NT if "noD" not in ablate else 1
            # pipeline the (z+xs)*dinv prep with the stats matmuls per chunk;
            # everything fp16 so the DVE runs in its packed 2x mode
            CH = 10
            for b0 in range(0, ND, CH):
                bn = min(CH, ND - b0)
                sl = slice(b0, b0 + bn)
                nc.vector.tensor_tensor(
                    out=aggs[:, sl, :D], in0=z_t[:, sl, :], in1=xs16[:, sl, :],
                    op=mybir.AluOpType.add,
                )
                nc.vector.tensor_tensor(
                    out=aggs[:, sl, :D], in0=aggs[:, sl, :D],
                    in1=dinv16[:, sl, :], op=mybir.AluOpType.mult,
                )
                for b in range(b0, b0 + bn):
                    nc.tensor.matmul(
                        out=stats_ps[:], lhsT=aggs[:, b, :D], rhs=aggs[:, b, :],
                        start=(b == 0), stop=(b == ND - 1),
                    )
            # stats reduction launched before the transposes/W1 matmuls so the
            # collective overlaps with PE work: replicate the local stats into
            # all 8 RS input blocks -> every core's RS output = global sum
            stats_sb = persist.tile([D, D + 1], f16, name="stats_sb")
            nc.vector.tensor_copy(out=stats_sb[:], in_=stats_ps[:])
            stg8 = persist.tile([D, NCORES, D + 1], f16, name="stg8")
            nc.vector.tensor_copy(
                out=stg8[:],
                in_=stats_sb[:].rearrange("p (o c) -> p o c", o=1)
                .to_broadcast([D, NCORES, D + 1]),
            )
            nc.sync.dma_start(
                out=ags_in[:, :].rearrange("(j p) c -> p j c", p=D), in_=stg8[:]
            )
            if "nocc" not in ablate:
                cc("ReduceScatter", mybir.AluOpType.add, ags_in[:, :], ags_out[:, :])
            else:
                nc.sync.dma_start(out=ags_out[:, :], in_=ags_in[0:D, :])

            for b0 in range(0, ND, 4):
                bn = min(4, ND - b0)
                tp_ps = wpsum.tile([D, 512], f16, tag="ps_a", name="tp_ps")
                for j in range(bn):
                    nc.tensor.transpose(
                        out=tp_ps[:, j * 128 : (j + 1) * 128],
                        in_=aggs[:, b0 + j, :D], identity=ident16[:],
                    )
                aggsT = work.tile([D, 512], f16, tag="aggsT", name="aggsT", bufs=2)
                nc.vector.tensor_copy(out=aggsT[:, : bn * 128], in_=tp_ps[:, : bn * 128])
                hT_ps = wpsum.tile([D, 512], f32, tag="ps_b", name="hT_ps")
                nc.tensor.matmul(
                    out=hT_ps[:, : bn * 128], lhsT=w1s16[:], rhs=aggsT[:, : bn * 128],
                    start=True, stop=True,
                )
                nc.vector.tensor_copy(
                    out=hT_big[:, b0 * 128 : (b0 + bn) * 128],
                    in_=hT_ps[:, : bn * 128],
                )

            st16 = persist.tile([D, D + 1], f16, name="st16")
            nc.scalar.dma_start(out=st16[:], in_=ags_out[:, :])
            st32 = persist.tile([D, D + 1], f32, name="st32")
            nc.vector.tensor_copy(out=st32[:], in_=st16[:])
            st = st32[:]

            # --- BN scalar algebra ---
            q_ps = wpsum.tile([D, 1], f32, tag="ps_a", name="q_ps")
            nc.tensor.matmul(out=q_ps[:], lhsT=w1s[:], rhs=st[:, D : D + 1], start=True, stop=True)
            mu = persist.tile([D, 1], f32, name="mu")
            nc.vector.tensor_scalar(
                out=mu[:], in0=q_ps[:], scalar1=1.0 / NN, scalar2=b1c[:],
                op0=mybir.AluOpType.mult, op1=mybir.AluOpType.add,
            )
            t1_ps = wpsum.tile([D, D], f32, tag="ps_b", name="t1_ps")
            nc.tensor.matmul(out=t1_ps[:], lhsT=st[:, :D], rhs=w1s[:], start=True, stop=True)
            m_sb = work.tile([D, D], f32, tag="m_sb", name="m_sb")
            nc.vector.tensor_tensor(out=m_sb[:], in0=w1s[:], in1=t1_ps[:], op=mybir.AluOpType.mult)
            d_ps = wpsum.tile([D, 1], f32, tag="ps_b", name="d_ps")
            nc.tensor.matmul(out=d_ps[:], lhsT=m_sb[:], rhs=ones64[:], start=True, stop=True)

            var = persist.tile([D, 1], f32, name="var")
            nc.vector.tensor_scalar_mul(out=var[:], in0=d_ps[:], scalar1=1.0 / NN)
            t2 = work.tile([D, 1], f32, tag="t2", name="t2")
            nc.vector.tensor_scalar_mul(out=t2[:], in0=q_ps[:], scalar1=2.0 / NN)
            nc.vector.tensor_tensor(out=t2[:], in0=t2[:], in1=b1c[:], op=mybir.AluOpType.mult)
            nc.vector.tensor_tensor(out=var[:], in0=var[:], in1=t2[:], op=mybir.AluOpType.add)
            t3 = work.tile([D, 1], f32, tag="t3", name="t3")
            nc.vector.tensor_tensor(out=t3[:], in0=b1c[:], in1=b1c[:], op=mybir.AluOpType.mult)
            nc.vector.tensor_tensor(out=var[:], in0=var[:], in1=t3[:], op=mybir.AluOpType.add)
            t4 = work.tile([D, 1], f32, tag="t4", name="t4")
            nc.vector.tensor_tensor(out=t4[:], in0=mu[:], in1=mu[:], op=mybir.AluOpType.mult)
            nc.vector.tensor_tensor(out=var[:], in0=var[:], in1=t4[:], op=mybir.AluOpType.subtract)

            sd = work.tile([D, 1], f32, tag="sd", name="sd")
            nc.scalar.activation(sd[:], var[:], mb.ActivationFunctionType.Sqrt, bias=epsc[:])
            rstd = work.tile([D, 1], f32, tag="rstd", name="rstd")
            nc.vector.reciprocal(out=rstd[:], in_=sd[:])
            a_sb = persist.tile([D, 1], f32, name="a_sb")
            nc.vector.tensor_tensor(out=a_sb[:], in0=gac[:], in1=rstd[:], op=mybir.AluOpType.mult)
            c_sb = persist.tile([D, 1], f32, name="c_sb")
            t5 = work.tile([D, 1], f32, tag="t5", name="t5")
            nc.vector.tensor_tensor(out=t5[:], in0=mu[:], in1=a_sb[:], op=mybir.AluOpType.mult)
            nc.vector.tensor_tensor(out=c_sb[:], in0=bec[:], in1=t5[:], op=mybir.AluOpType.subtract)
            # hT tiles exclude the b1 bias; fold it into the BN offset:
            # relu(a*(h+b1) + c) = relu(a*h + (c + a*b1))
            t6 = work.tile([D, 1], f32, tag="t6", name="t6")
            nc.vector.tensor_tensor(out=t6[:], in0=a_sb[:], in1=b1c[:], op=mybir.AluOpType.mult)
            nc.vector.tensor_tensor(out=c_sb[:], in0=c_sb[:], in1=t6[:], op=mybir.AluOpType.add)

            # --- phase F: BN+ReLU, transpose back, dinv fold -> xs2;
            #     phase G interleaved: poolT += xs2_b^T @ R_b ---
            poolT_ps = spsum.tile([D, NUM_GRAPHS], f32, name="poolT_ps")
            xs2 = persist.tile([128, NT, D], f32, name="xs2")
            NF = NT if "noF" not in ablate else 0
            for b0 in range(0, NF, 4):
                bn = min(4, NF - b0)
                h1T = work.tile([D, 512], f32, tag="h1T", name="h1T", bufs=3)
                nc.scalar.activation(
                    h1T[:, : bn * 128],
                    hT_big[:, b0 * 128 : (b0 + bn) * 128],
                    mb.ActivationFunctionType.Relu,
                    bias=c_sb[:], scale=a_sb[:],
                )
                nm_ps = fpsum.tile([128, 4 * D], f32, tag="ps_c", name="nm_ps")
                for j in range(bn):
                    nc.tensor.transpose(
                        out=nm_ps[:, j * D : (j + 1) * D],
                        in_=h1T[:, j * 128 : (j + 1) * 128],
                        identity=ident[:D, :D],
                    )
                nc.vector.tensor_tensor(
                    out=xs2[:, b0 : b0 + bn, :], in0=nm_ps[:, : bn * D].rearrange(
                        "p (g d) -> p g d", d=D),
                    in1=dinvs[:, b0 : b0 + bn].rearrange(
                        "p (g o) -> p g o", o=1).to_broadcast([128, bn, D]),
                    op=mybir.AluOpType.mult,
                )
                for j in range(bn):
                    b = b0 + j
                    nc.tensor.matmul(
                        out=poolT_ps[:], lhsT=xs2[:, b, :], rhs=rkt_t[:, b, :],
                        start=(b == 0), stop=(b == NF - 1),
                    )

            # --- output: pool @ W2 + p1^T b2; AllGather + local reduce ---
            poolT_sb = persist.tile([D, NUM_GRAPHS], f32, name="poolT_sb")
            nc.vector.tensor_copy(out=poolT_sb[:], in_=poolT_ps[:])
            out_ps = wpsum.tile([NUM_GRAPHS, D], f32, tag="ps_a", name="out_ps")
            nc.tensor.matmul(out=out_ps[:], lhsT=poolT_sb[:], rhs=w2s[:], start=True, stop=False)
            nc.tensor.matmul(out=out_ps[:], lhsT=p1s[:], rhs=b2r[:], start=False, stop=True)
            out_sb = persist.tile([NUM_GRAPHS, D], f16, name="out_sb")
            nc.vector.tensor_copy(out=out_sb[:], in_=out_ps[:])
            nc.sync.dma_start(out=ago_in[0:NUM_GRAPHS, :], in_=out_sb[:])
            if "nocc" not in ablate:
                cc("ReduceScatter", mybir.AluOpType.add, ago_in[:, :], ago_out[:, :])
            else:
                nc.sync.dma_start(out=ago_out[:, :], in_=ago_in[0:NUM_GRAPHS, :])
            og = persist.tile([NUM_GRAPHS, D], f16, name="og")
            nc.scalar.dma_start(out=og[:], in_=ago_out[:, :])
            og32 = persist.tile([NUM_GRAPHS, D], f32, name="og32")
            nc.vector.tensor_copy(out=og32[:], in_=og[:])
            nc.sync.dma_start(out=out_d[:, :], in_=og32[:])

    nc.compile()
    return nc


def _wrap16(v, n):
    """idx j at [j%16, j//16], replicated to 128 partitions (8 Q7 cores)."""
    assert v.shape[0] == n and n % 16 == 0
    t = v.astype(np.int16).reshape(n // 16, 16).T
    return np.tile(t, (8, 1))


def _bin_edges(gsrc, grow, nbins, nspill):
    """Rotation binning: occurrence o of dst row r -> bin (r + o) % nbins for
    o < nbins; higher occurrences spill into one extra bin per occurrence
    level (occurrence levels have unique rows by construction)."""
    order = np.argsort(grow, kind="stable")
    sd, ss = grow[order], gsrc[order]
    out = [(np.zeros(0, np.int64), np.zeros(0, np.int64))] * (nbins + nspill)
    if sd.shape[0] == 0:
        return out
    change = np.r_[True, sd[1:] != sd[:-1]]
    starts = np.flatnonzero(change)
    gid = np.cumsum(change) - 1
    occ = np.arange(sd.shape[0]) - starts[gid]
    assert int(occ.max()) < nbins + nspill, (int(occ.max()), nbins, nspill)
    b = np.where(occ < nbins, (sd + occ) % nbins, occ)
    return [(ss[b == i], sd[b == i]) for i in range(nbins + nspill)]


def prepare_inputs(cfg, x, edge_index, batch, W1, b1, gamma, beta, W2, b2):
    """Host-side index preprocessing + per-core input maps.  Fills cfg.seg."""
    SL, SLP, NT = cfg.SL, cfg.SLP, cfg.NT
    n = cfg.N

    x = np.ascontiguousarray(np.asarray(x, dtype=np.float32))
    src = np.asarray(edge_index[0], dtype=np.int64)
    dst = np.asarray(edge_index[1], dtype=np.int64)
    batch = np.asarray(batch, dtype=np.int64)
    W1 = np.asarray(W1, dtype=np.float32)
    b1 = np.asarray(b1, dtype=np.float32)
    gamma = np.asarray(gamma, dtype=np.float32)
    beta = np.asarray(beta, dtype=np.float32)
    W2 = np.asarray(W2, dtype=np.float32)
    b2 = np.asarray(b2, dtype=np.float32)

    deg = np.bincount(dst, minlength=n).astype(np.float32) + 1.0  # + self-loop
    dinv = (1.0 / np.sqrt(deg)).astype(np.float32)

    cnt = np.bincount(batch, minlength=NUM_GRAPHS).astype(np.float32)
    w_graph = 1.0 / np.maximum(cnt, 1.0)
    pd = w_graph[batch] * dinv          # P[batch[v], v] * dinv_v  per node

    owner = src // SL
    src_local = src - owner * SL

    # dst -> (parity, acc row): node (p, g) of core k ->
    # row k*SZJ + p*HT + g//2, column half g%2
    d_owner = dst // SL
    d_local = dst - d_owner * SL
    d_g = d_local // 128
    d_p = d_local - d_g * 128
    d_par = d_g % 2
    d_row = d_owner * cfg.SZJ + d_p * cfg.HT + d_g // 2

    per_core = [[None, None] for _ in range(NCORES)]
    for k in range(NCORES):
        sel = owner == k
        es, ed, ec = src_local[sel], d_row[sel], d_par[sel]
        for c in (0, 1):
            m = ec == c
            per_core[k][c] = (es[m], ed[m])

    # shared bin layout per parity
    seg, core_bins = [], [[] for _ in range(NCORES)]
    for c in (0, 1):
        counts = [per_core[k][c][0].shape[0] for k in range(NCORES)]
        mm = 1
        for k in range(NCORES):
            rows = per_core[k][c][1]
            if rows.shape[0]:
                mm = max(mm, int(np.bincount(rows).max()))
        nbins = max(-(-max(counts) // (cfg.CAP - 256)), 1)
        while True:
            nspill = max(mm - nbins, 0)
            allb = [
                _bin_edges(per_core[k][c][0], per_core[k][c][1], nbins, nspill)
                for k in range(NCORES)
            ]
            sizes = [
                ((max(allb[k][i][0].shape[0] for k in range(NCORES)) + 127)
                 // 128) * 128
                for i in range(nbins + nspill)
            ]
            if all(s <= cfg.CAP for s in sizes):
                break
            nbins += 1
        for i in range(nbins + nspill):
            if sizes[i] == 0:
                continue
            seg.append((c, sizes[i]))
            for k in range(NCORES):
                core_bins[k].append((c, sizes[i], allb[k][i]))

    cfg.seg = seg

    in_maps = []
    for k in range(NCORES):
        gl_parts, sc_parts = [], []
        for c, size, (es, ed) in core_bins[k]:
            m = es.shape[0]
            g = np.zeros(size, dtype=np.int64)
            s = np.full(size, cfg.SZ, dtype=np.int64)  # core 0 junk-tile row
            order = np.argsort(es, kind="stable")  # src-sorted for locality
            g[:m] = es[order]
            s[:m] = ed[order]
            gl_parts.append(_wrap16(g, size))
            sc_parts.append(_wrap16(s, size))
        gidx = np.concatenate(gl_parts, axis=1)
        sidx = np.concatenate(sc_parts, axis=1)

        lo, hi = k * SL, min((k + 1) * SL, n)
        nsl = hi - lo
        xsl = np.zeros((SLP, D), dtype=np.float32)
        xsl[:nsl] = x[lo:hi]
        xsl_pm = np.ascontiguousarray(
            xsl.reshape(NT, 128, D).transpose(1, 0, 2).reshape(128, NT * D)
        )
        dsl = np.zeros(SLP, dtype=np.float32)
        dsl[:nsl] = dinv[lo:hi]
        dinv_in = dsl.reshape(NT, 128).T.copy()

        # R_k^T [SLP, 64]: R_kT[u, g] = sum_{edges (k*SL+u) -> w} P[g,w]*dinv_w
        #                             + P[g, k*SL+u]*dinv_{k*SL+u}
        sel = owner == k
        rkt = np.zeros((SLP, NUM_GRAPHS), dtype=np.float32)
        np.add.at(rkt, (src_local[sel], batch[dst[sel]]), pd[dst[sel]])
        rkt[np.arange(nsl), batch[lo:hi]] += pd[lo:hi]
        rkt_pm = np.ascontiguousarray(
            rkt.reshape(NT, 128, NUM_GRAPHS).transpose(1, 0, 2).reshape(128, -1)
        )

        p1 = np.zeros((1, NUM_GRAPHS), dtype=np.float32)
        np.add.at(p1[0], batch[lo:hi], w_graph[batch[lo:hi]])

        in_maps.append({
            "xsl": xsl_pm,
            "dinv_in": dinv_in,
            "gidx": np.ascontiguousarray(gidx),
            "sidx": np.ascontiguousarray(sidx),
            "rkt": rkt_pm,
            "p1": p1,
            "w1": W1,
            "b1": b1.reshape(D, 1),
            "ga": gamma.reshape(D, 1),
            "be": beta.reshape(D, 1),
            "w2": W2,
            "b2": b2.reshape(1, D),
        })
    return in_maps


def kernel(x, edge_index, batch, W1, b1, gamma, beta, W2, b2):
    global LAST_EXEC_TIME_NS
    from concourse.bass_utils import run_bass_kernel_spmd

    cfg = Cfg(N_NODES, N_NODES // NCORES)
    in_maps = prepare_inputs(cfg, x, edge_index, batch, W1, b1, gamma, beta, W2, b2)

    key = (cfg.N, cfg.SL, tuple(cfg.seg))
    if key not in _NC_CACHE:
        _NC_CACHE[key] = build(cfg)
    nc = _NC_CACHE[key]
    global _LAST_IN_MAPS
    _LAST_IN_MAPS = in_maps

    trace = bool(int(os.environ.get("BASS_GNN_TRACE", "0")))
    if trace:
        try:
            res = run_bass_kernel_spmd(nc, in_maps, list(range(NCORES)), trace=True)
        except Exception:
            res = run_bass_kernel_spmd(nc, in_maps, list(range(NCORES)), trace=False)
    else:
        res = run_bass_kernel_spmd(nc, in_maps, list(range(NCORES)), trace=False)
    LAST_EXEC_TIME_NS = res.exec_time_ns
    return np.asarray(res.results[0]["out"], dtype=np.float32)


def modeled_time_ns(x=None, edge_index=None, **kw):
    """Cost-model execution time (MultiCoreSim, mocked collectives) for the
    current cached program; used when NTFF tracing is unavailable."""
    if not _NC_CACHE:
        return None
    nc = next(iter(_NC_CACHE.values()))
    ins = _LAST_IN_MAPS
    if ins is None:
        return None
    from concourse.bass_interp import MultiCoreSim

    sim = MultiCoreSim(nc, 2, debug_mock_collectives_without_correctness=True)
    for i, core in sim.cores.items():
        for name, val in ins[i].items():
            core.tensor(name)[:] = val
    sim.simulate()
    return int(sim.global_time)


# revision 79
# speedup vs baseline: 1.4706x; 1.0053x over previous
# GCN (2-layer GCNConv + BatchNorm + ReLU + global mean pool) on 8 TRN2 NeuronCores.
#
# Math (reference):
#   deg[v]  = in-degree incl. self-loop;  dinv = deg^-1/2
#   layer(x, W, b): h = D^-1/2 (A+I) D^-1/2 (x W) + b
#   h1 = relu(batchnorm(layer1));  h2 = layer2(h1);  out = segment_mean(h2, batch)
#
# Sharding (v4 — source-partitioned edges + fp16 ReduceScatter):
#   Core k owns nodes [k*SL, (k+1)*SL) and all edges whose SRC falls in that
#   range.  Layer 1:
#     * xs = dinv * x (own slice) -> local f32 gather table (DRAM); the edge
#       gather needs NO collective at all.
#     * per-edge: dma_gather xs[src] rows from the local table (f32, 256B
#       elems), convert the message tile to fp16 on DVE (hidden behind the
#       Pool-engine gather/scatter stream), then dma_scatter_add into a
#       global fp16 accumulator at the dst row.  The accumulator packs two
#       nodes per 256B row (scatter rows need 256B stride); node (p, g) of
#       core k lives at row k*SZJ + p*(NT/2) + g//2, column half g%2, so
#       scatter instructions are split by tile parity.
#     * one fp16 ReduceScatter hands each core the reduced rows of its own
#       slice (half the bytes of f32 — collective cost tracks output size).
#     * self-loops are folded in AFTER the ReduceScatter as one vector add
#       (z + xs) instead of 12.5k extra scatter slots.
#   BatchNorm stats via an accumulated A^T[A|1] matmul + algebraic reduction.
#   The tiny [64,65] stats reduction and the final [64,64] output reduction
#   use AllGather + local vector adds (cheaper than AllReduce).
#   Layer 2 + pooling collapse into dense matmuls: mean-pool P and the outer
#   D^-1/2 are linear, so out = sum_k (R_k @ xs2_k) W2 + b2 with
#   R_k[g, u] = sum_{edges u->w owned by k} P[g,w] dinv_w (+ self term),
#   built on the host from pure index data.  No second edge phase, no second
#   table, no second big collective.
#
# dma_scatter_add races (loses updates) for duplicate dst rows within one
# instruction, so edges are packed into instruction "bins" with unique dst
# rows per bin via rotation binning: occurrence o of dst row r goes to bin
# (r + o) % nbins.  The accumulator has 26112 rows, so scatter indices fit
# int16 with no bucketing.  Pad slots gather row 0 and scatter into a dead
# junk row (a reserved 64-row tile per core block).
#
# Host-side preprocessing uses only index data (edge_index, batch): degree
# computation, edge partitioning/binning, the R_k pooling matrices.  Feature
# data is never touched on the host.

import os

import numpy as np

N_NODES = 50000
N_EDGES = 800000
D = 64
NCORES = 8
NUM_GRAPHS = 64
BN_EPS = 1e-5


class Cfg:
    def __init__(self, n, sl):
        self.N = n                    # total nodes
        self.SL = sl                  # owned nodes per core
        slp = ((sl + 127) // 128) * 128
        if (slp // 128) % 2:
            slp += 128                # even tile count (node-pair packing)
        self.SLP = slp
        assert self.SL < self.SLP
        self.NT = self.SLP // 128     # 128-row node tiles per slice (even)
        self.HT = self.NT // 2
        self.SZ = 128 * self.HT       # real acc rows per core
        self.SZJ = self.SZ + 64       # + junk pair-tile
        self.CAP = 7680               # max slots per gather/scatter instruction
        # per-instruction (parity, padded slot count); filled by prepare_inputs
        self.seg = []


LAST_EXEC_TIME_NS = None
_NC_CACHE = {}
_LAST_IN_MAPS = None


def build(cfg):
    import concourse.mybir as mybir
    import concourse.tile as tile
    from concourse import bacc
    from concourse.bass import BassGpSimd
    from concourse.masks import make_identity

    f32 = mybir.dt.float32
    f16 = mybir.dt.float16
    i16 = mybir.dt.int16
    SLP, NT = cfg.SLP, cfg.NT
    NN = float(cfg.N)
    RG = [list(range(NCORES))]
    segs = cfg.seg
    tot_s = sum(c for _, c in segs)
    ACC_R = NCORES * cfg.SZJ

    nc = bacc.Bacc(
        "TRN2", target_bir_lowering=False, debug=False, num_devices=NCORES
    )

    # --- external inputs (per-core values supplied via in_maps) ---
    xsl = nc.declare_dram_parameter("xsl", [128, NT * D], f32, isOutput=False)
    dinv_in = nc.declare_dram_parameter("dinv_in", [128, NT], f32, isOutput=False)
    gidx_d = nc.declare_dram_parameter("gidx", [128, tot_s // 16], i16, isOutput=False)
    sidx_d = nc.declare_dram_parameter("sidx", [128, tot_s // 16], i16, isOutput=False)
    rkt_d = nc.declare_dram_parameter("rkt", [128, NT * D], f32, isOutput=False)
    p1_d = nc.declare_dram_parameter("p1", [1, NUM_GRAPHS], f32, isOutput=False)
    w1_d = nc.declare_dram_parameter("w1", [D, D], f32, isOutput=False)
    b1_d = nc.declare_dram_parameter("b1", [D, 1], f32, isOutput=False)
    ga_d = nc.declare_dram_parameter("ga", [D, 1], f32, isOutput=False)
    be_d = nc.declare_dram_parameter("be", [D, 1], f32, isOutput=False)
    w2_d = nc.declare_dram_parameter("w2", [D, D], f32, isOutput=False)
    b2_d = nc.declare_dram_parameter("b2", [1, D], f32, isOutput=False)
    out_d = nc.declare_dram_parameter("out", [NUM_GRAPHS, D], f32, isOutput=True)

    # --- internal DRAM ---
    table1 = nc.dram_tensor("table1", [SLP, D], f32)
    acc = nc.dram_tensor("acc", [ACC_R, 2 * D], f16)
    rs_out = nc.dram_tensor("rs_out", [cfg.SZJ, 2 * D], f16)
    # reductions as replicated-input ReduceScatters: writing this core's
    # partial into all 8 input blocks makes the RS hand every core the full
    # sum (stats), or core 0 the full sum (output — the only core read)
    ags_in = nc.dram_tensor("ags_in", [NCORES * D, D + 1], f16)
    ags_out = nc.dram_tensor("ags_out", [D, D + 1], f16)
    ago_in = nc.dram_tensor("ago_in", [NCORES * NUM_GRAPHS, D], f16)
    ago_out = nc.dram_tensor("ago_out", [NUM_GRAPHS, D], f16)

    cc_eng = os.environ.get("GNN_CC_ENG", "pool")

    def cc(kind, op, ins_ap, outs_ap):
        BassGpSimd.collective_compute(
            nc.gpsimd if cc_eng == "pool" else getattr(nc, cc_eng),
            kind, op, replica_groups=RG, ins=[ins_ap], outs=[outs_ap],
        )

    with tile.TileContext(nc) as tc:
        with (
            tc.tile_pool(name="const", bufs=1) as const,
            tc.tile_pool(name="persist", bufs=1) as persist,
            tc.tile_pool(name="work", bufs=2) as work,
            tc.tile_pool(name="msgp", bufs=3) as msgp,
            tc.tile_pool(name="msghp", bufs=2) as msghp,
            tc.tile_pool(name="spsum", bufs=1, space="PSUM") as spsum,
            tc.tile_pool(name="wpsum", bufs=2, space="PSUM") as wpsum,
            tc.tile_pool(name="fpsum", bufs=2, space="PSUM") as fpsum,
        ):
            ablate = os.environ.get("GNN_ABLATE", "")

            # --- zero tile for accumulator init (fp16) ---
            ZW = ACC_R * 2 * D // 8 // 128
            zt = persist.tile([128, ZW], f16, name="zt")
            nc.vector.memset(zt[:], 0.0)

            # --- phase A inputs: x slice halves in separate tiles so the
            #     scale/table pipeline isn't serialized by whole-tensor deps ---
            HN = NT // 2
            xs_a = persist.tile([128, HN, D], f32, name="xs_a")
            xs_b = persist.tile([128, NT - HN, D], f32, name="xs_b")
            xsl_v = xsl[:, :].rearrange("p (g d) -> p g d", d=D)
            nc.sync.dma_start(out=xs_a[:], in_=xsl_v[:, :HN, :])
            nc.sync.dma_start(out=xs_b[:], in_=xsl_v[:, HN:, :])
            gidx_t = persist.tile([128, tot_s // 16], i16, name="gidx_t")
            nc.scalar.dma_start(out=gidx_t[:], in_=gidx_d[:, :])

            # --- constants into SBUF (Pool is idle until the first gather);
            #     dinvs first: the phase-A scale waits on it ---
            dinvs = const.tile([128, NT], f32)
            nc.gpsimd.dma_start(out=dinvs[:], in_=dinv_in[:, :])
            w1s = const.tile([D, D], f32)
            nc.gpsimd.dma_start(out=w1s[:], in_=w1_d[:, :])
            w2s = const.tile([D, D], f32)
            nc.gpsimd.dma_start(out=w2s[:], in_=w2_d[:, :])
            b1c = const.tile([D, 1], f32)
            nc.gpsimd.dma_start(out=b1c[:], in_=b1_d[:, :])
            gac = const.tile([D, 1], f32)
            nc.gpsimd.dma_start(out=gac[:], in_=ga_d[:, :])
            bec = const.tile([D, 1], f32)
            nc.gpsimd.dma_start(out=bec[:], in_=be_d[:, :])
            b2r = const.tile([1, D], f32)
            nc.gpsimd.dma_start(out=b2r[:], in_=b2_d[:, :])
            p1s = const.tile([1, NUM_GRAPHS], f32)
            nc.gpsimd.dma_start(out=p1s[:], in_=p1_d[:, :])

            # --- phase A: xs = dinv * x -> local gather table (split SP/Act),
            #     interleaved with the 8 accumulator zero chunks ---
            dinv_b = dinvs[:, :].rearrange("p (g o) -> p g o", o=1).to_broadcast(
                [128, NT, D]
            )
            nc.vector.tensor_tensor(
                out=xs_a[:], in0=xs_a[:],
                in1=dinvs[:, :HN].rearrange("p (g o) -> p g o", o=1)
                .to_broadcast([128, HN, D]),
                op=mybir.AluOpType.mult,
            )
            nc.vector.tensor_tensor(
                out=xs_b[:], in0=xs_b[:],
                in1=dinvs[:, HN:].rearrange("p (g o) -> p g o", o=1)
                .to_broadcast([128, NT - HN, D]),
                op=mybir.AluOpType.mult,
            )
            # fp16 copies of xs / dinv (and identity) for the packed-DVE
            # post-RS path; built early so they hide under the edge phase
            xs16 = persist.tile([128, NT, D], f16, name="xs16")
            nc.vector.tensor_copy(out=xs16[:, :HN, :], in_=xs_a[:])
            nc.vector.tensor_copy(out=xs16[:, HN:, :], in_=xs_b[:])
            dinv16 = persist.tile([128, NT, D], f16, name="dinv16")
            nc.vector.tensor_copy(out=dinv16[:], in_=dinv_b)
            acc_flat = acc[:, :].rearrange("n d -> (n d)")

            def zchunk(eng, j):
                ap = acc_flat.rearrange("(j p x) -> j p x", j=8, p=128)[j]
                eng.dma_start(out=ap, in_=zt[:, :])

            tview = table1[:, :].rearrange("(g p) d -> p g d", p=128)
            zchunk(nc.gpsimd, 0)
            zchunk(nc.gpsimd, 1)
            zchunk(nc.gpsimd, 2)
            zchunk(nc.gpsimd, 3)
            nc.sync.dma_start(out=tview[:, :HN, :], in_=xs_a[:])
            nc.scalar.dma_start(out=tview[:, HN:, :], in_=xs_b[:])
            # sidx in halves so the Act queue can slot the table write between
            sidx_t = persist.tile([128, tot_s // 16], i16, name="sidx_t")
            SH = (tot_s // 16) // 2
            nc.scalar.dma_start(out=sidx_t[:, :SH], in_=sidx_d[:, :SH])
            nc.scalar.dma_start(out=sidx_t[:, SH:], in_=sidx_d[:, SH:])
            zchunk(nc.sync, 4)
            zchunk(nc.sync, 5)
            zchunk(nc.scalar, 6)
            zchunk(nc.scalar, 7)
            # zero blocks 1..7 of the output-reduction RS input (core 0's
            # received chunk is the only one that matters)
            nc.sync.dma_start(
                out=ago_in[NUM_GRAPHS:, :].rearrange(
                    "(j p) d -> p j d", p=NUM_GRAPHS),
                in_=zt[0:NUM_GRAPHS, : (NCORES - 1) * D].rearrange(
                    "p (j d) -> p j d", d=D),
            )

            # identity / BN constants / activation-table warmup — needed only
            # from phase D on, emitted after the edge-phase-critical work
            ident = const.tile([128, 128], f32)
            make_identity(nc, ident[:])
            ones64 = const.tile([D, 1], f32)
            nc.vector.memset(ones64[:], 1.0)
            epsc = const.tile([D, 1], f32)
            nc.vector.memset(epsc[:], BN_EPS)
            warm = const.tile([D, 1], f32)
            nc.scalar.activation(warm[:], epsc[:], mybir.ActivationFunctionType.Sqrt)
            nc.scalar.activation(warm[:], epsc[:], mybir.ActivationFunctionType.Relu)

            # --- R_k^T for layer 2 (loaded during the edge phase) ---
            rkt_t = persist.tile([128, NT, D], f32, name="rkt_t")
            nc.sync.dma_start(
                out=rkt_t[:], in_=rkt_d[:, :].rearrange("p (g d) -> p g d", d=D)
            )

            # --- edge phase: gather f32 / convert fp16 / scatter-add fp16 ---
            import concourse.mybir as mb

            if "noedge" not in ablate:
                pend = None
                off = 0
                for c, cnt in segs:
                    msg = msgp.tile([128, cfg.CAP // 128, D], f32, tag="msg",
                                    name="msg")
                    nc.gpsimd.dma_gather(
                        out_ap=msg[:, : cnt // 128, :],
                        in_ap=table1[0:SLP, :],
                        idxs_ap=gidx_t[:, off : off + cnt // 16],
                        num_idxs=cnt, num_idxs_reg=cnt, elem_size=D,
                        single_packet=False, queue_num=0,
                    )
                    msgh = msghp.tile([128, cfg.CAP // 128, D], f16, tag="msgh",
                                      name="msgh")
                    nc.vector.tensor_copy(
                        out=msgh[:, : cnt // 128, :], in_=msg[:, : cnt // 128, :]
                    )
                    if pend is not None:
                        nc.gpsimd.dma_scatter_add(*pend, elem_step=2 * D, single_packet=False, queue_num=0)
                    pend = (
                        acc[:, c * D : (c + 1) * D],
                        msgh[:, : cnt // 128, :],
                        sidx_t[:, off : off + cnt // 16],
                        cnt, cnt, D,
                    )
                    off += cnt // 16
                if pend is not None:
                    nc.gpsimd.dma_scatter_add(*pend, elem_step=2 * D, single_packet=False, queue_num=0)

            # --- ReduceScatter -> own reduced slice (fp16) ---
            if "nocc" not in ablate:
                cc("ReduceScatter", mybir.AluOpType.add, acc[:, :], rs_out[:, :])
            else:
                nc.sync.dma_start(out=rs_out[:, :], in_=acc[0 : cfg.SZJ, :])

            # warm the PE p-state during the collective (it idles otherwise,
            # and the first post-RS matmuls would run at the slow p-state)
            warm_ps = fpsum.tile([D, D], f32, tag="ps_c", name="warm_ps")
            for _ in range(24):
                nc.tensor.matmul(
                    out=warm_ps[:], lhsT=ident[:D, :D], rhs=ident[:D, :D],
                    start=True, stop=True,
                )

            # --- dense layer-1: z + self-loop, dinv scale, stats, W1 ---
            # z in two tiles loaded on SP and Act in parallel; the aggs
            # chunks start as soon as the first half lands
            z_a = persist.tile([128, HN, D], f16, name="z_a")
            z_b = persist.tile([128, NT - HN, D], f16, name="z_b")
            zsrc = rs_out[0 : cfg.SZ, :].rearrange("(p r) c -> p (r c)", p=128)
            HW_ = NT * D // 2
            nc.sync.dma_start(
                out=z_a[:].rearrange("p g d -> p (g d)"), in_=zsrc[:, :HW_]
            )
            nc.scalar.dma_start(
                out=z_b[:].rearrange("p g d -> p (g d)"), in_=zsrc[:, HW_:]
            )

            def z_sl(b0, bn):
                if b0 >= HN:
                    return z_b[:, b0 - HN : b0 - HN + bn, :]
                return z_a[:, b0 : b0 + bn, :]

            # keep the PE p-state warm through the z load so the stats
            # matmuls run at full clock
            for _ in range(10):
                nc.tensor.matmul(
                    out=warm_ps[:], lhsT=z_a[:, 0, :], rhs=z_a[:, 0, :],
                    start=True, stop=True,
                )
            aggs = persist.tile([128, NT, D + 1], f16, name="aggs")
            nc.vector.memset(aggs[:, :, D : D + 1], 1.0)
            ident16 = persist.tile([128, 128], f16, name="ident16")
            nc.vector.tensor_copy(out=ident16[:], in_=ident[:])
            w1s16 = persist.tile([D, D], f16, name="w1s16")
            nc.vector.tensor_copy(out=w1s16[:], in_=w1s[:])

            stats_ps = spsum.tile([D, D + 1], f32, name="stats_ps")
            hT_big = persist.tile([D, NT * 128], f32, name="hT_big")
            ND = NT if "noD" not in ablate else 1
            # pipeline the (z+xs)*dinv prep with the stats matmuls per chunk;
            # everything fp16 so the DVE runs in its packed 2x mode; chunks
            # never straddle the z_a/z_b boundary
            chunks = [(0, 10), (10, 20), (20, HN), (HN, 35), (35, 45), (45, ND)]
            chunks = [(a, min(b, ND)) for a, b in chunks if a < ND]
            for b0, b1 in chunks:
                bn = b1 - b0
                sl = slice(b0, b0 + bn)
                nc.vector.tensor_tensor(
                    out=aggs[:, sl, :D], in0=z_sl(b0, bn), in1=xs16[:, sl, :],
                    op=mybir.AluOpType.add,
                )
                nc.vector.tensor_tensor(
                    out=aggs[:, sl, :D], in0=aggs[:, sl, :D],
                    in1=dinv16[:, sl, :], op=mybir.AluOpType.mult,
                )
                for b in range(b0, b0 + bn):
                    nc.tensor.matmul(
                        out=stats_ps[:], lhsT=aggs[:, b, :D], rhs=aggs[:, b, :],
                        start=(b == 0), stop=(b == ND - 1),
                    )
            # stats reduction launched before the transposes/W1 matmuls so the
            # collective overlaps with PE work: replicate the local stats into
            # all 8 RS input blocks -> every core's RS output = global sum
            stats_sb = persist.tile([D, D + 1], f16, name="stats_sb")
            nc.vector.tensor_copy(out=stats_sb[:], in_=stats_ps[:])
            stg8 = persist.tile([D, NCORES, D + 1], f16, name="stg8")
            nc.vector.tensor_copy(
                out=stg8[:],
                in_=stats_sb[:].rearrange("p (o c) -> p o c", o=1)
                .to_broadcast([D, NCORES, D + 1]),
            )
            nc.sync.dma_start(
                out=ags_in[:, :].rearrange("(j p) c -> p j c", p=D), in_=stg8[:]
            )
            if "nocc" not in ablate:
                cc("ReduceScatter", mybir.AluOpType.add, ags_in[:, :], ags_out[:, :])
            else:
                nc.sync.dma_start(out=ags_out[:, :], in_=ags_in[0:D, :])

            for b0 in range(0, ND, 4):
                bn = min(4, ND - b0)
                tp_ps = wpsum.tile([D, 512], f16, tag="ps_a", name="tp_ps")
                for j in range(bn):
                    nc.tensor.transpose(
                        out=tp_ps[:, j * 128 : (j + 1) * 128],
                        in_=aggs[:, b0 + j, :D], identity=ident16[:],
                    )
                aggsT = work.tile([D, 512], f16, tag="aggsT", name="aggsT", bufs=2)
                nc.vector.tensor_copy(out=aggsT[:, : bn * 128], in_=tp_ps[:, : bn * 128])
                hT_ps = wpsum.tile([D, 512], f32, tag="ps_b", name="hT_ps")
                nc.tensor.matmul(
                    out=hT_ps[:, : bn * 128], lhsT=w1s16[:], rhs=aggsT[:, : bn * 128],
                    start=True, stop=True,
                )
                nc.vector.tensor_copy(
                    out=hT_big[:, b0 * 128 : (b0 + bn) * 128],
                    in_=hT_ps[:, : bn * 128],
                )

            st16 = persist.tile([D, D + 1], f16, name="st16")
            nc.scalar.dma_start(out=st16[:], in_=ags_out[:, :])
            st32 = persist.tile([D, D + 1], f32, name="st32")
            nc.vector.tensor_copy(out=st32[:], in_=st16[:])
            st = st32[:]

            # --- BN scalar algebra ---
            q_ps = wpsum.tile([D, 1], f32, tag="ps_a", name="q_ps")
            nc.tensor.matmul(out=q_ps[:], lhsT=w1s[:], rhs=st[:, D : D + 1], start=True, stop=True)
            mu = persist.tile([D, 1], f32, name="mu")
            nc.vector.tensor_scalar(
                out=mu[:], in0=q_ps[:], scalar1=1.0 / NN, scalar2=b1c[:],
                op0=mybir.AluOpType.mult, op1=mybir.AluOpType.add,
            )
            t1_ps = wpsum.tile([D, D], f32, tag="ps_b", name="t1_ps")
            nc.tensor.matmul(out=t1_ps[:], lhsT=st[:, :D], rhs=w1s[:], start=True, stop=True)
            m_sb = work.tile([D, D], f32, tag="m_sb", name="m_sb")
            nc.vector.tensor_tensor(out=m_sb[:], in0=w1s[:], in1=t1_ps[:], op=mybir.AluOpType.mult)
            d_ps = wpsum.tile([D, 1], f32, tag="ps_b", name="d_ps")
            nc.tensor.matmul(out=d_ps[:], lhsT=m_sb[:], rhs=ones64[:], start=True, stop=True)

            var = persist.tile([D, 1], f32, name="var")
            nc.vector.tensor_scalar_mul(out=var[:], in0=d_ps[:], scalar1=1.0 / NN)
            t2 = work.tile([D, 1], f32, tag="t2", name="t2")
            nc.vector.tensor_scalar_mul(out=t2[:], in0=q_ps[:], scalar1=2.0 / NN)
            nc.vector.tensor_tensor(out=t2[:], in0=t2[:], in1=b1c[:], op=mybir.AluOpType.mult)
            nc.vector.tensor_tensor(out=var[:], in0=var[:], in1=t2[:], op=mybir.AluOpType.add)
            t3 = work.tile([D, 1], f32, tag="t3", name="t3")
            nc.vector.tensor_tensor(out=t3[:], in0=b1c[:], in1=b1c[:], op=mybir.AluOpType.mult)
            nc.vector.tensor_tensor(out=var[:], in0=var[:], in1=t3[:], op=mybir.AluOpType.add)
            t4 = work.tile([D, 1], f32, tag="t4", name="t4")
            nc.vector.tensor_tensor(out=t4[:], in0=mu[:], in1=mu[:], op=mybir.AluOpType.mult)
            nc.vector.tensor_tensor(out=var[:], in0=var[:], in1=t4[:], op=mybir.AluOpType.subtract)

            sd = work.tile([D, 1], f32, tag="sd", name="sd")
            nc.scalar.activation(sd[:], var[:], mb.ActivationFunctionType.Sqrt, bias=epsc[:])
            rstd = work.tile([D, 1], f32, tag="rstd", name="rstd")
            nc.vector.reciprocal(out=rstd[:], in_=sd[:])
            a_sb = persist.tile([D, 1], f32, name="a_sb")
            nc.vector.tensor_tensor(out=a_sb[:], in0=gac[:], in1=rstd[:], op=mybir.AluOpType.mult)
            c_sb = persist.tile([D, 1], f32, name="c_sb")
            t5 = work.tile([D, 1], f32, tag="t5", name="t5")
            nc.vector.tensor_tensor(out=t5[:], in0=mu[:], in1=a_sb[:], op=mybir.AluOpType.mult)
            nc.vector.tensor_tensor(out=c_sb[:], in0=bec[:], in1=t5[:], op=mybir.AluOpType.subtract)
            # hT tiles exclude the b1 bias; fold it into the BN offset:
            # relu(a*(h+b1) + c) = relu(a*h + (c + a*b1))
            t6 = work.tile([D, 1], f32, tag="t6", name="t6")
            nc.vector.tensor_tensor(out=t6[:], in0=a_sb[:], in1=b1c[:], op=mybir.AluOpType.mult)
            nc.vector.tensor_tensor(out=c_sb[:], in0=c_sb[:], in1=t6[:], op=mybir.AluOpType.add)

            # --- phase F: BN+ReLU, transpose back, dinv fold -> xs2;
            #     phase G interleaved: poolT += xs2_b^T @ R_b ---
            poolT_ps = spsum.tile([D, NUM_GRAPHS], f32, name="poolT_ps")
            xs2 = persist.tile([128, NT, D], f32, name="xs2")
            NF = NT if "noF" not in ablate else 0
            for b0 in range(0, NF, 8):
                bn = min(8, NF - b0)
                h1T = work.tile([D, 1024], f32, tag="h1T", name="h1T", bufs=3)
                nc.scalar.activation(
                    h1T[:, : bn * 128],
                    hT_big[:, b0 * 128 : (b0 + bn) * 128],
                    mb.ActivationFunctionType.Relu,
                    bias=c_sb[:], scale=a_sb[:],
                )
                nm_ps = fpsum.tile([128, 8 * D], f32, tag="ps_c", name="nm_ps")
                for j in range(bn):
                    nc.tensor.transpose(
                        out=nm_ps[:, j * D : (j + 1) * D],
                        in_=h1T[:, j * 128 : (j + 1) * 128],
                        identity=ident[:D, :D],
                    )
                nc.vector.tensor_tensor(
                    out=xs2[:, b0 : b0 + bn, :], in0=nm_ps[:, : bn * D].rearrange(
                        "p (g d) -> p g d", d=D),
                    in1=dinvs[:, b0 : b0 + bn].rearrange(
                        "p (g o) -> p g o", o=1).to_broadcast([128, bn, D]),
                    op=mybir.AluOpType.mult,
                )
                for j in range(bn):
                    b = b0 + j
                    nc.tensor.matmul(
                        out=poolT_ps[:], lhsT=xs2[:, b, :], rhs=rkt_t[:, b, :],
                        start=(b == 0), stop=(b == NF - 1),
                    )

            # --- output: pool @ W2 + p1^T b2; AllGather + local reduce ---
            poolT_sb = persist.tile([D, NUM_GRAPHS], f32, name="poolT_sb")
            nc.vector.tensor_copy(out=poolT_sb[:], in_=poolT_ps[:])
            out_ps = wpsum.tile([NUM_GRAPHS, D], f32, tag="ps_a", name="out_ps")
            nc.tensor.matmul(out=out_ps[:], lhsT=poolT_sb[:], rhs=w2s[:], start=True, stop=False)
            nc.tensor.matmul(out=out_ps[:], lhsT=p1s[:], rhs=b2r[:], start=False, stop=True)
            out_sb = persist.tile([NUM_GRAPHS, D], f16, name="out_sb")
            nc.vector.tensor_copy(out=out_sb[:], in_=out_ps[:])
            nc.sync.dma_start(out=ago_in[0:NUM_GRAPHS, :], in_=out_sb[:])
            if "nocc" not in ablate:
                cc("ReduceScatter", mybir.AluOpType.add, ago_in[:, :], ago_out[:, :])
            else:
                nc.sync.dma_start(out=ago_out[:, :], in_=ago_in[0:NUM_GRAPHS, :])
            og = persist.tile([NUM_GRAPHS, D], f16, name="og")
            nc.scalar.dma_start(out=og[:], in_=ago_out[:, :])
            og32 = persist.tile([NUM_GRAPHS, D], f32, name="og32")
            nc.vector.tensor_copy(out=og32[:], in_=og[:])
            nc.sync.dma_start(out=out_d[:, :], in_=og32[:])

    nc.compile()
    return nc


def _wrap16(v, n):
    """idx j at [j%16, j//16], replicated to 128 partitions (8 Q7 cores)."""
    assert v.shape[0] == n and n % 16 == 0
    t = v.astype(np.int16).reshape(n // 16, 16).T
    return np.tile(t, (8, 1))


def _bin_edges(gsrc, grow, nbins, nspill):
    """Rotation binning: occurrence o of dst row r -> bin (r + o) % nbins for
    o < nbins; higher occurrences spill into one extra bin per occurrence
    level (occurrence levels have unique rows by construction)."""
    order = np.argsort(grow, kind="stable")
    sd, ss = grow[order], gsrc[order]
    out = [(np.zeros(0, np.int64), np.zeros(0, np.int64))] * (nbins + nspill)
    if sd.shape[0] == 0:
        return out
    change = np.r_[True, sd[1:] != sd[:-1]]
    starts = np.flatnonzero(change)
    gid = np.cumsum(change) - 1
    occ = np.arange(sd.shape[0]) - starts[gid]
    assert int(occ.max()) < nbins + nspill, (int(occ.max()), nbins, nspill)
    b = np.where(occ < nbins, (sd + occ) % nbins, occ)
    return [(ss[b == i], sd[b == i]) for i in range(nbins + nspill)]


def prepare_inputs(cfg, x, edge_index, batch, W1, b1, gamma, beta, W2, b2):
    """Host-side index preprocessing + per-core input maps.  Fills cfg.seg."""
    SL, SLP, NT = cfg.SL, cfg.SLP, cfg.NT
    n = cfg.N

    x = np.ascontiguousarray(np.asarray(x, dtype=np.float32))
    src = np.asarray(edge_index[0], dtype=np.int64)
    dst = np.asarray(edge_index[1], dtype=np.int64)
    batch = np.asarray(batch, dtype=np.int64)
    W1 = np.asarray(W1, dtype=np.float32)
    b1 = np.asarray(b1, dtype=np.float32)
    gamma = np.asarray(gamma, dtype=np.float32)
    beta = np.asarray(beta, dtype=np.float32)
    W2 = np.asarray(W2, dtype=np.float32)
    b2 = np.asarray(b2, dtype=np.float32)

    deg = np.bincount(dst, minlength=n).astype(np.float32) + 1.0  # + self-loop
    dinv = (1.0 / np.sqrt(deg)).astype(np.float32)

    cnt = np.bincount(batch, minlength=NUM_GRAPHS).astype(np.float32)
    w_graph = 1.0 / np.maximum(cnt, 1.0)
    pd = w_graph[batch] * dinv          # P[batch[v], v] * dinv_v  per node

    owner = src // SL
    src_local = src - owner * SL

    # dst -> (parity, acc row): node (p, g) of core k ->
    # row k*SZJ + p*HT + g//2, column half g%2
    d_owner = dst // SL
    d_local = dst - d_owner * SL
    d_g = d_local // 128
    d_p = d_local - d_g * 128
    d_par = d_g % 2
    d_row = d_owner * cfg.SZJ + d_p * cfg.HT + d_g // 2

    per_core = [[None, None] for _ in range(NCORES)]
    for k in range(NCORES):
        sel = owner == k
        es, ed, ec = src_local[sel], d_row[sel], d_par[sel]
        for c in (0, 1):
            m = ec == c
            per_core[k][c] = (es[m], ed[m])

    # shared bin layout per parity
    seg, core_bins = [], [[] for _ in range(NCORES)]
    for c in (0, 1):
        counts = [per_core[k][c][0].shape[0] for k in range(NCORES)]
        mm = 1
        for k in range(NCORES):
            rows = per_core[k][c][1]
            if rows.shape[0]:
                mm = max(mm, int(np.bincount(rows).max()))
        nbins = max(-(-max(counts) // (cfg.CAP - 256)), 1)
        while True:
            nspill = max(mm - nbins, 0)
            allb = [
                _bin_edges(per_core[k][c][0], per_core[k][c][1], nbins, nspill)
                for k in range(NCORES)
            ]
            sizes = [
                ((max(allb[k][i][0].shape[0] for k in range(NCORES)) + 127)
                 // 128) * 128
                for i in range(nbins + nspill)
            ]
            if all(s <= cfg.CAP for s in sizes):
                break
            nbins += 1
        for i in range(nbins + nspill):
            if sizes[i] == 0:
                continue
            seg.append((c, sizes[i]))
            for k in range(NCORES):
                core_bins[k].append((c, sizes[i], allb[k][i]))

    cfg.seg = seg

    in_maps = []
    for k in range(NCORES):
        gl_parts, sc_parts = [], []
        for c, size, (es, ed) in core_bins[k]:
            m = es.shape[0]
            g = np.zeros(size, dtype=np.int64)
            s = np.full(size, cfg.SZ, dtype=np.int64)  # core 0 junk-tile row
            order = np.argsort(es, kind="stable")  # src-sorted for locality
            g[:m] = es[order]
            s[:m] = ed[order]
            gl_parts.append(_wrap16(g, size))
            sc_parts.append(_wrap16(s, size))
        gidx = np.concatenate(gl_parts, axis=1)
        sidx = np.concatenate(sc_parts, axis=1)

        lo, hi = k * SL, min((k + 1) * SL, n)
        nsl = hi - lo
        xsl = np.zeros((SLP, D), dtype=np.float32)
        xsl[:nsl] = x[lo:hi]
        xsl_pm = np.ascontiguousarray(
            xsl.reshape(NT, 128, D).transpose(1, 0, 2).reshape(128, NT * D)
        )
        dsl = np.zeros(SLP, dtype=np.float32)
        dsl[:nsl] = dinv[lo:hi]
        dinv_in = dsl.reshape(NT, 128).T.copy()

        # R_k^T [SLP, 64]: R_kT[u, g] = sum_{edges (k*SL+u) -> w} P[g,w]*dinv_w
        #                             + P[g, k*SL+u]*dinv_{k*SL+u}
        sel = owner == k
        rkt = np.zeros((SLP, NUM_GRAPHS), dtype=np.float32)
        np.add.at(rkt, (src_local[sel], batch[dst[sel]]), pd[dst[sel]])
        rkt[np.arange(nsl), batch[lo:hi]] += pd[lo:hi]
        rkt_pm = np.ascontiguousarray(
            rkt.reshape(NT, 128, NUM_GRAPHS).transpose(1, 0, 2).reshape(128, -1)
        )

        p1 = np.zeros((1, NUM_GRAPHS), dtype=np.float32)
        np.add.at(p1[0], batch[lo:hi], w_graph[batch[lo:hi]])

        in_maps.append({
            "xsl": xsl_pm,
            "dinv_in": dinv_in,
            "gidx": np.ascontiguousarray(gidx),
            "sidx": np.ascontiguousarray(sidx),
            "rkt": rkt_pm,
            "p1": p1,
            "w1": W1,
            "b1": b1.reshape(D, 1),
            "ga": gamma.reshape(D, 1),
            "be": beta.reshape(D, 1),
            "w2": W2,
            "b2": b2.reshape(1, D),
        })
    return in_maps


def kernel(x, edge_index, batch, W1, b1, gamma, beta, W2, b2):
    global LAST_EXEC_TIME_NS
    from concourse.bass_utils import run_bass_kernel_spmd

    cfg = Cfg(N_NODES, N_NODES // NCORES)
    in_maps = prepare_inputs(cfg, x, edge_index, batch, W1, b1, gamma, beta, W2, b2)

    key = (cfg.N, cfg.SL, tuple(cfg.seg))
    if key not in _NC_CACHE:
        _NC_CACHE[key] = build(cfg)
    nc = _NC_CACHE[key]
    global _LAST_IN_MAPS
    _LAST_IN_MAPS = in_maps

    trace = bool(int(os.environ.get("BASS_GNN_TRACE", "0")))
    if trace:
        try:
            res = run_bass_kernel_spmd(nc, in_maps, list(range(NCORES)), trace=True)
        except Exception:
            res = run_bass_kernel_spmd(nc, in_maps, list(range(NCORES)), trace=False)
    else:
        res = run_bass_kernel_spmd(nc, in_maps, list(range(NCORES)), trace=False)
    LAST_EXEC_TIME_NS = res.exec_time_ns
    return np.asarray(res.results[0]["out"], dtype=np.float32)


def modeled_time_ns(x=None, edge_index=None, **kw):
    """Cost-model execution time (MultiCoreSim, mocked collectives) for the
    current cached program; used when NTFF tracing is unavailable."""
    if not _NC_CACHE:
        return None
    nc = next(iter(_NC_CACHE.values()))
    ins = _LAST_IN_MAPS
    if ins is None:
        return None
    from concourse.bass_interp import MultiCoreSim

    sim = MultiCoreSim(nc, 2, debug_mock_collectives_without_correctness=True)
    for i, core in sim.cores.items():
        for name, val in ins[i].items():
            core.tensor(name)[:] = val
    sim.simulate()
    return int(sim.global_time)


# revision 84
# speedup vs baseline: 1.4727x; 1.0014x over previous
# GCN (2-layer GCNConv + BatchNorm + ReLU + global mean pool) on 8 TRN2 NeuronCores.
#
# Math (reference):
#   deg[v]  = in-degree incl. self-loop;  dinv = deg^-1/2
#   layer(x, W, b): h = D^-1/2 (A+I) D^-1/2 (x W) + b
#   h1 = relu(batchnorm(layer1));  h2 = layer2(h1);  out = segment_mean(h2, batch)
#
# Sharding (v4 — source-partitioned edges + fp16 ReduceScatter):
#   Core k owns nodes [k*SL, (k+1)*SL) and all edges whose SRC falls in that
#   range.  Layer 1:
#     * xs = dinv * x (own slice) -> local f32 gather table (DRAM); the edge
#       gather needs NO collective at all.
#     * per-edge: dma_gather xs[src] rows from the local table (f32, 256B
#       elems), convert the message tile to fp16 on DVE (hidden behind the
#       Pool-engine gather/scatter stream), then dma_scatter_add into a
#       global fp16 accumulator at the dst row.  The accumulator packs two
#       nodes per 256B row (scatter rows need 256B stride); node (p, g) of
#       core k lives at row k*SZJ + p*(NT/2) + g//2, column half g%2, so
#       scatter instructions are split by tile parity.
#     * one fp16 ReduceScatter hands each core the reduced rows of its own
#       slice (half the bytes of f32 — collective cost tracks output size).
#     * self-loops are folded in AFTER the ReduceScatter as one vector add
#       (z + xs) instead of 12.5k extra scatter slots.
#   BatchNorm stats via an accumulated A^T[A|1] matmul + algebraic reduction.
#   The tiny [64,65] stats reduction and the final [64,64] output reduction
#   use AllGather + local vector adds (cheaper than AllReduce).
#   Layer 2 + pooling collapse into dense matmuls: mean-pool P and the outer
#   D^-1/2 are linear, so out = sum_k (R_k @ xs2_k) W2 + b2 with
#   R_k[g, u] = sum_{edges u->w owned by k} P[g,w] dinv_w (+ self term),
#   built on the host from pure index data.  No second edge phase, no second
#   table, no second big collective.
#
# dma_scatter_add races (loses updates) for duplicate dst rows within one
# instruction, so edges are packed into instruction "bins" with unique dst
# rows per bin via rotation binning: occurrence o of dst row r goes to bin
# (r + o) % nbins.  The accumulator has 25600 rows, so scatter indices fit
# int16 with no bucketing.  Pad slots gather row 0 and scatter into a dead
# (dinv=0, zero-padded) node row, one per tile parity.
#
# Host-side preprocessing uses only index data (edge_index, batch): degree
# computation, edge partitioning/binning, the R_k pooling matrices.  Feature
# data is never touched on the host.

import os

import numpy as np

N_NODES = 50000
N_EDGES = 800000
D = 64
NCORES = 8
NUM_GRAPHS = 64
BN_EPS = 1e-5


class Cfg:
    def __init__(self, n, sl):
        self.N = n                    # total nodes
        self.SL = sl                  # owned nodes per core
        slp = ((sl + 127) // 128) * 128
        if (slp // 128) % 2:
            slp += 128                # even tile count (node-pair packing)
        self.SLP = slp
        assert self.SL < self.SLP
        self.NT = self.SLP // 128     # 128-row node tiles per slice (even)
        self.HT = self.NT // 2
        self.SZ = 128 * self.HT       # acc rows per core
        self.SZJ = self.SZ            # pads reuse dead node rows (no junk tile)
        # dead (dinv=0) rows in core 0's block, one per tile parity, used as
        # scatter-pad targets; races there only lose junk
        assert self.SL <= (self.NT - 1) * 128
        p0 = self.SL - (self.NT - 2) * 128          # first dead p in tile NT-2
        assert 0 <= p0 < 128
        self.TRASH = [p0 * self.HT + (self.NT - 2) // 2,
                      0 * self.HT + (self.NT - 1) // 2]
        self.CAP = 7680               # max slots per gather/scatter instruction
        # per-instruction (parity, padded slot count); filled by prepare_inputs
        self.seg = []


LAST_EXEC_TIME_NS = None
_NC_CACHE = {}
_LAST_IN_MAPS = None


def build(cfg):
    import concourse.mybir as mybir
    import concourse.tile as tile
    from concourse import bacc
    from concourse.bass import BassGpSimd
    from concourse.masks import make_identity

    f32 = mybir.dt.float32
    f16 = mybir.dt.float16
    i16 = mybir.dt.int16
    SLP, NT = cfg.SLP, cfg.NT
    NN = float(cfg.N)
    RG = [list(range(NCORES))]
    segs = cfg.seg
    tot_s = sum(c for _, c in segs)
    ACC_R = NCORES * cfg.SZJ

    nc = bacc.Bacc(
        "TRN2", target_bir_lowering=False, debug=False, num_devices=NCORES
    )

    # --- external inputs (per-core values supplied via in_maps) ---
    xsl = nc.declare_dram_parameter("xsl", [128, NT * D], f32, isOutput=False)
    dinv_in = nc.declare_dram_parameter("dinv_in", [128, NT], f32, isOutput=False)
    gidx_d = nc.declare_dram_parameter("gidx", [128, tot_s // 16], i16, isOutput=False)
    sidx_d = nc.declare_dram_parameter("sidx", [128, tot_s // 16], i16, isOutput=False)
    rkt_d = nc.declare_dram_parameter("rkt", [128, NT * D], f32, isOutput=False)
    p1_d = nc.declare_dram_parameter("p1", [1, NUM_GRAPHS], f32, isOutput=False)
    w1_d = nc.declare_dram_parameter("w1", [D, D], f32, isOutput=False)
    b1_d = nc.declare_dram_parameter("b1", [D, 1], f32, isOutput=False)
    ga_d = nc.declare_dram_parameter("ga", [D, 1], f32, isOutput=False)
    be_d = nc.declare_dram_parameter("be", [D, 1], f32, isOutput=False)
    w2_d = nc.declare_dram_parameter("w2", [D, D], f32, isOutput=False)
    b2_d = nc.declare_dram_parameter("b2", [1, D], f32, isOutput=False)
    out_d = nc.declare_dram_parameter("out", [NUM_GRAPHS, D], f32, isOutput=True)

    # --- internal DRAM ---
    table1 = nc.dram_tensor("table1", [SLP, D], f32)
    acc = nc.dram_tensor("acc", [ACC_R, 2 * D], f16)
    rs_out = nc.dram_tensor("rs_out", [cfg.SZJ, 2 * D], f16)
    # reductions as replicated-input ReduceScatters: writing this core's
    # partial into all 8 input blocks makes the RS hand every core the full
    # sum (stats), or core 0 the full sum (output — the only core read)
    ags_in = nc.dram_tensor("ags_in", [NCORES * D, D + 1], f16)
    ags_out = nc.dram_tensor("ags_out", [D, D + 1], f16)
    ago_in = nc.dram_tensor("ago_in", [NCORES * NUM_GRAPHS, D], f16)
    ago_out = nc.dram_tensor("ago_out", [NUM_GRAPHS, D], f16)

    cc_eng = os.environ.get("GNN_CC_ENG", "pool")

    def cc(kind, op, ins_ap, outs_ap):
        BassGpSimd.collective_compute(
            nc.gpsimd if cc_eng == "pool" else getattr(nc, cc_eng),
            kind, op, replica_groups=RG, ins=[ins_ap], outs=[outs_ap],
        )

    with tile.TileContext(nc) as tc:
        with (
            tc.tile_pool(name="const", bufs=1) as const,
            tc.tile_pool(name="persist", bufs=1) as persist,
            tc.tile_pool(name="work", bufs=2) as work,
            tc.tile_pool(name="msgp", bufs=3) as msgp,
            tc.tile_pool(name="msghp", bufs=2) as msghp,
            tc.tile_pool(name="spsum", bufs=1, space="PSUM") as spsum,
            tc.tile_pool(name="wpsum", bufs=2, space="PSUM") as wpsum,
            tc.tile_pool(name="fpsum", bufs=2, space="PSUM") as fpsum,
        ):
            ablate = os.environ.get("GNN_ABLATE", "")

            # --- zero tile for accumulator init (fp16) ---
            ZW = ACC_R * 2 * D // 8 // 128
            zt = persist.tile([128, ZW], f16, name="zt")
            nc.vector.memset(zt[:], 0.0)

            # --- phase A inputs: x slice halves in separate tiles so the
            #     scale/table pipeline isn't serialized by whole-tensor deps ---
            HN = NT // 2
            xs_a = persist.tile([128, HN, D], f32, name="xs_a")
            xs_b = persist.tile([128, NT - HN, D], f32, name="xs_b")
            xsl_v = xsl[:, :].rearrange("p (g d) -> p g d", d=D)
            nc.sync.dma_start(out=xs_a[:], in_=xsl_v[:, :HN, :])
            nc.sync.dma_start(out=xs_b[:], in_=xsl_v[:, HN:, :])
            gidx_t = persist.tile([128, tot_s // 16], i16, name="gidx_t")
            nc.scalar.dma_start(out=gidx_t[:], in_=gidx_d[:, :])

            # --- constants into SBUF (Pool is idle until the first gather);
            #     dinvs first: the phase-A scale waits on it ---
            dinvs = const.tile([128, NT], f32)
            nc.gpsimd.dma_start(out=dinvs[:], in_=dinv_in[:, :])
            w1s = const.tile([D, D], f32)
            nc.gpsimd.dma_start(out=w1s[:], in_=w1_d[:, :])
            w2s = const.tile([D, D], f32)
            nc.gpsimd.dma_start(out=w2s[:], in_=w2_d[:, :])
            b1c = const.tile([D, 1], f32)
            nc.gpsimd.dma_start(out=b1c[:], in_=b1_d[:, :])
            gac = const.tile([D, 1], f32)
            nc.gpsimd.dma_start(out=gac[:], in_=ga_d[:, :])
            bec = const.tile([D, 1], f32)
            nc.gpsimd.dma_start(out=bec[:], in_=be_d[:, :])
            b2r = const.tile([1, D], f32)
            nc.gpsimd.dma_start(out=b2r[:], in_=b2_d[:, :])
            p1s = const.tile([1, NUM_GRAPHS], f32)
            nc.gpsimd.dma_start(out=p1s[:], in_=p1_d[:, :])

            # --- phase A: xs = dinv * x -> local gather table (split SP/Act),
            #     interleaved with the 8 accumulator zero chunks ---
            dinv_b = dinvs[:, :].rearrange("p (g o) -> p g o", o=1).to_broadcast(
                [128, NT, D]
            )
            nc.vector.tensor_tensor(
                out=xs_a[:], in0=xs_a[:],
                in1=dinvs[:, :HN].rearrange("p (g o) -> p g o", o=1)
                .to_broadcast([128, HN, D]),
                op=mybir.AluOpType.mult,
            )
            nc.vector.tensor_tensor(
                out=xs_b[:], in0=xs_b[:],
                in1=dinvs[:, HN:].rearrange("p (g o) -> p g o", o=1)
                .to_broadcast([128, NT - HN, D]),
                op=mybir.AluOpType.mult,
            )
            # fp16 copies of xs / dinv (and identity) for the packed-DVE
            # post-RS path; built early so they hide under the edge phase
            xs16 = persist.tile([128, NT, D], f16, name="xs16")
            nc.vector.tensor_copy(out=xs16[:, :HN, :], in_=xs_a[:])
            nc.vector.tensor_copy(out=xs16[:, HN:, :], in_=xs_b[:])
            dinv16 = persist.tile([128, NT, D], f16, name="dinv16")
            nc.vector.tensor_copy(out=dinv16[:], in_=dinv_b)
            acc_flat = acc[:, :].rearrange("n d -> (n d)")

            def zchunk(eng, j):
                ap = acc_flat.rearrange("(j p x) -> j p x", j=8, p=128)[j]
                eng.dma_start(out=ap, in_=zt[:, :])

            tview = table1[:, :].rearrange("(g p) d -> p g d", p=128)
            zchunk(nc.gpsimd, 0)
            zchunk(nc.gpsimd, 1)
            zchunk(nc.gpsimd, 2)
            zchunk(nc.gpsimd, 3)
            nc.sync.dma_start(out=tview[:, :HN, :], in_=xs_a[:])
            nc.scalar.dma_start(out=tview[:, HN:, :], in_=xs_b[:])
            # sidx in halves so the Act queue can slot the table write between
            sidx_t = persist.tile([128, tot_s // 16], i16, name="sidx_t")
            SH = (tot_s // 16) // 2
            nc.scalar.dma_start(out=sidx_t[:, :SH], in_=sidx_d[:, :SH])
            nc.scalar.dma_start(out=sidx_t[:, SH:], in_=sidx_d[:, SH:])
            zchunk(nc.sync, 4)
            zchunk(nc.sync, 5)
            zchunk(nc.scalar, 6)
            zchunk(nc.scalar, 7)
            # zero blocks 1..7 of the output-reduction RS input (core 0's
            # received chunk is the only one that matters)
            nc.sync.dma_start(
                out=ago_in[NUM_GRAPHS:, :].rearrange(
                    "(j p) d -> p j d", p=NUM_GRAPHS),
                in_=zt[0:NUM_GRAPHS, : (NCORES - 1) * D].rearrange(
                    "p (j d) -> p j d", d=D),
            )

            # identity / BN constants / activation-table warmup — needed only
            # from phase D on, emitted after the edge-phase-critical work
            ident = const.tile([128, 128], f32)
            make_identity(nc, ident[:])
            ones64 = const.tile([D, 1], f32)
            nc.vector.memset(ones64[:], 1.0)
            epsc = const.tile([D, 1], f32)
            nc.vector.memset(epsc[:], BN_EPS)
            warm = const.tile([D, 1], f32)
            nc.scalar.activation(warm[:], epsc[:], mybir.ActivationFunctionType.Sqrt)
            nc.scalar.activation(warm[:], epsc[:], mybir.ActivationFunctionType.Relu)

            # --- R_k^T for layer 2 (loaded during the edge phase) ---
            rkt_t = persist.tile([128, NT, D], f32, name="rkt_t")
            nc.sync.dma_start(
                out=rkt_t[:], in_=rkt_d[:, :].rearrange("p (g d) -> p g d", d=D)
            )

            # --- edge phase: gather f32 / convert fp16 / scatter-add fp16 ---
            import concourse.mybir as mb

            if "noedge" not in ablate:
                pend = None
                off = 0
                for c, cnt in segs:
                    msg = msgp.tile([128, cfg.CAP // 128, D], f32, tag="msg",
                                    name="msg")
                    nc.gpsimd.dma_gather(
                        out_ap=msg[:, : cnt // 128, :],
                        in_ap=table1[0:SLP, :],
                        idxs_ap=gidx_t[:, off : off + cnt // 16],
                        num_idxs=cnt, num_idxs_reg=cnt, elem_size=D,
                        single_packet=False, queue_num=0,
                    )
                    msgh = msghp.tile([128, cfg.CAP // 128, D], f16, tag="msgh",
                                      name="msgh")
                    nc.vector.tensor_copy(
                        out=msgh[:, : cnt // 128, :], in_=msg[:, : cnt // 128, :]
                    )
                    if pend is not None:
                        nc.gpsimd.dma_scatter_add(*pend, elem_step=2 * D, single_packet=False, queue_num=0)
                    pend = (
                        acc[:, c * D : (c + 1) * D],
                        msgh[:, : cnt // 128, :],
                        sidx_t[:, off : off + cnt // 16],
                        cnt, cnt, D,
                    )
                    off += cnt // 16
                if pend is not None:
                    nc.gpsimd.dma_scatter_add(*pend, elem_step=2 * D, single_packet=False, queue_num=0)

            # --- ReduceScatter -> own reduced slice (fp16) ---
            if "nocc" not in ablate:
                cc("ReduceScatter", mybir.AluOpType.add, acc[:, :], rs_out[:, :])
            else:
                nc.sync.dma_start(out=rs_out[:, :], in_=acc[0 : cfg.SZJ, :])

            # warm the PE p-state during the collective (it idles otherwise,
            # and the first post-RS matmuls would run at the slow p-state)
            warm_ps = fpsum.tile([D, D], f32, tag="ps_c", name="warm_ps")
            for _ in range(24):
                nc.tensor.matmul(
                    out=warm_ps[:], lhsT=ident[:D, :D], rhs=ident[:D, :D],
                    start=True, stop=True,
                )

            # --- dense layer-1: z + self-loop, dinv scale, stats, W1 ---
            # z in two tiles loaded on SP and Act in parallel; the aggs
            # chunks start as soon as the first half lands
            z_a = persist.tile([128, HN, D], f16, name="z_a")
            z_b = persist.tile([128, NT - HN, D], f16, name="z_b")
            zsrc = rs_out[0 : cfg.SZ, :].rearrange("(p r) c -> p (r c)", p=128)
            HW_ = NT * D // 2
            nc.sync.dma_start(
                out=z_a[:].rearrange("p g d -> p (g d)"), in_=zsrc[:, :HW_]
            )
            nc.scalar.dma_start(
                out=z_b[:].rearrange("p g d -> p (g d)"), in_=zsrc[:, HW_:]
            )

            def z_sl(b0, bn):
                if b0 >= HN:
                    return z_b[:, b0 - HN : b0 - HN + bn, :]
                return z_a[:, b0 : b0 + bn, :]

            # keep the PE p-state warm through the z load so the stats
            # matmuls run at full clock
            for _ in range(10):
                nc.tensor.matmul(
                    out=warm_ps[:], lhsT=z_a[:, 0, :], rhs=z_a[:, 0, :],
                    start=True, stop=True,
                )
            aggs = persist.tile([128, NT, D + 1], f16, name="aggs")
            nc.vector.memset(aggs[:, :, D : D + 1], 1.0)
            ident16 = persist.tile([128, 128], f16, name="ident16")
            nc.vector.tensor_copy(out=ident16[:], in_=ident[:])
            w1s16 = persist.tile([D, D], f16, name="w1s16")
            nc.vector.tensor_copy(out=w1s16[:], in_=w1s[:])

            stats_ps = spsum.tile([D, D + 1], f32, name="stats_ps")
            hT_big = persist.tile([D, NT * 128], f32, name="hT_big")
            ND = NT if "noD" not in ablate else 1
            # pipeline the (z+xs)*dinv prep with the stats matmuls per chunk;
            # everything fp16 so the DVE runs in its packed 2x mode; chunks
            # never straddle the z_a/z_b boundary
            chunks = [(0, 10), (10, 20), (20, HN), (HN, 35), (35, 45), (45, ND)]
            chunks = [(a, min(b, ND)) for a, b in chunks if a < ND]
            for b0, b1 in chunks:
                bn = b1 - b0
                sl = slice(b0, b0 + bn)
                nc.vector.tensor_tensor(
                    out=aggs[:, sl, :D], in0=z_sl(b0, bn), in1=xs16[:, sl, :],
                    op=mybir.AluOpType.add,
                )
                nc.vector.tensor_tensor(
                    out=aggs[:, sl, :D], in0=aggs[:, sl, :D],
                    in1=dinv16[:, sl, :], op=mybir.AluOpType.mult,
                )
                for b in range(b0, b0 + bn):
                    nc.tensor.matmul(
                        out=stats_ps[:], lhsT=aggs[:, b, :D], rhs=aggs[:, b, :],
                        start=(b == 0), stop=(b == ND - 1),
                    )
            # stats reduction launched before the transposes/W1 matmuls so the
            # collective overlaps with PE work: replicate the local stats into
            # all 8 RS input blocks -> every core's RS output = global sum
            stats_sb = persist.tile([D, D + 1], f16, name="stats_sb")
            nc.vector.tensor_copy(out=stats_sb[:], in_=stats_ps[:])
            stg8 = persist.tile([D, NCORES, D + 1], f16, name="stg8")
            nc.vector.tensor_copy(
                out=stg8[:],
                in_=stats_sb[:].rearrange("p (o c) -> p o c", o=1)
                .to_broadcast([D, NCORES, D + 1]),
            )
            nc.sync.dma_start(
                out=ags_in[:, :].rearrange("(j p) c -> p j c", p=D), in_=stg8[:]
            )
            if "nocc" not in ablate:
                cc("ReduceScatter", mybir.AluOpType.add, ags_in[:, :], ags_out[:, :])
            else:
                nc.sync.dma_start(out=ags_out[:, :], in_=ags_in[0:D, :])

            for b0 in range(0, ND, 4):
                bn = min(4, ND - b0)
                tp_ps = wpsum.tile([D, 512], f16, tag="ps_a", name="tp_ps")
                for j in range(bn):
                    nc.tensor.transpose(
                        out=tp_ps[:, j * 128 : (j + 1) * 128],
                        in_=aggs[:, b0 + j, :D], identity=ident16[:],
                    )
                aggsT = work.tile([D, 512], f16, tag="aggsT", name="aggsT", bufs=2)
                nc.vector.tensor_copy(out=aggsT[:, : bn * 128], in_=tp_ps[:, : bn * 128])
                hT_ps = wpsum.tile([D, 512], f32, tag="ps_b", name="hT_ps")
                nc.tensor.matmul(
                    out=hT_ps[:, : bn * 128], lhsT=w1s16[:], rhs=aggsT[:, : bn * 128],
                    start=True, stop=True,
                )
                nc.vector.tensor_copy(
                    out=hT_big[:, b0 * 128 : (b0 + bn) * 128],
                    in_=hT_ps[:, : bn * 128],
                )

            st16 = persist.tile([D, D + 1], f16, name="st16")
            nc.scalar.dma_start(out=st16[:], in_=ags_out[:, :])
            st32 = persist.tile([D, D + 1], f32, name="st32")
            nc.vector.tensor_copy(out=st32[:], in_=st16[:])
            st = st32[:]

            # --- BN scalar algebra ---
            q_ps = wpsum.tile([D, 1], f32, tag="ps_a", name="q_ps")
            nc.tensor.matmul(out=q_ps[:], lhsT=w1s[:], rhs=st[:, D : D + 1], start=True, stop=True)
            mu = persist.tile([D, 1], f32, name="mu")
            nc.vector.tensor_scalar(
                out=mu[:], in0=q_ps[:], scalar1=1.0 / NN, scalar2=b1c[:],
                op0=mybir.AluOpType.mult, op1=mybir.AluOpType.add,
            )
            t1_ps = wpsum.tile([D, D], f32, tag="ps_b", name="t1_ps")
            nc.tensor.matmul(out=t1_ps[:], lhsT=st[:, :D], rhs=w1s[:], start=True, stop=True)
            m_sb = work.tile([D, D], f32, tag="m_sb", name="m_sb")
            nc.vector.tensor_tensor(out=m_sb[:], in0=w1s[:], in1=t1_ps[:], op=mybir.AluOpType.mult)
            d_ps = wpsum.tile([D, 1], f32, tag="ps_b", name="d_ps")
            nc.tensor.matmul(out=d_ps[:], lhsT=m_sb[:], rhs=ones64[:], start=True, stop=True)

            var = persist.tile([D, 1], f32, name="var")
            nc.vector.tensor_scalar_mul(out=var[:], in0=d_ps[:], scalar1=1.0 / NN)
            t2 = work.tile([D, 1], f32, tag="t2", name="t2")
            nc.vector.tensor_scalar_mul(out=t2[:], in0=q_ps[:], scalar1=2.0 / NN)
            nc.vector.tensor_tensor(out=t2[:], in0=t2[:], in1=b1c[:], op=mybir.AluOpType.mult)
            nc.vector.tensor_tensor(out=var[:], in0=var[:], in1=t2[:], op=mybir.AluOpType.add)
            t3 = work.tile([D, 1], f32, tag="t3", name="t3")
            nc.vector.tensor_tensor(out=t3[:], in0=b1c[:], in1=b1c[:], op=mybir.AluOpType.mult)
            nc.vector.tensor_tensor(out=var[:], in0=var[:], in1=t3[:], op=mybir.AluOpType.add)
            t4 = work.tile([D, 1], f32, tag="t4", name="t4")
            nc.vector.tensor_tensor(out=t4[:], in0=mu[:], in1=mu[:], op=mybir.AluOpType.mult)
            nc.vector.tensor_tensor(out=var[:], in0=var[:], in1=t4[:], op=mybir.AluOpType.subtract)

            sd = work.tile([D, 1], f32, tag="sd", name="sd")
            nc.scalar.activation(sd[:], var[:], mb.ActivationFunctionType.Sqrt, bias=epsc[:])
            rstd = work.tile([D, 1], f32, tag="rstd", name="rstd")
            nc.vector.reciprocal(out=rstd[:], in_=sd[:])
            a_sb = persist.tile([D, 1], f32, name="a_sb")
            nc.vector.tensor_tensor(out=a_sb[:], in0=gac[:], in1=rstd[:], op=mybir.AluOpType.mult)
            c_sb = persist.tile([D, 1], f32, name="c_sb")
            t5 = work.tile([D, 1], f32, tag="t5", name="t5")
            nc.vector.tensor_tensor(out=t5[:], in0=mu[:], in1=a_sb[:], op=mybir.AluOpType.mult)
            nc.vector.tensor_tensor(out=c_sb[:], in0=bec[:], in1=t5[:], op=mybir.AluOpType.subtract)
            # hT tiles exclude the b1 bias; fold it into the BN offset:
            # relu(a*(h+b1) + c) = relu(a*h + (c + a*b1))
            t6 = work.tile([D, 1], f32, tag="t6", name="t6")
            nc.vector.tensor_tensor(out=t6[:], in0=a_sb[:], in1=b1c[:], op=mybir.AluOpType.mult)
            nc.vector.tensor_tensor(out=c_sb[:], in0=c_sb[:], in1=t6[:], op=mybir.AluOpType.add)

            # --- phase F: BN+ReLU, transpose back, dinv fold -> xs2;
            #     phase G interleaved: poolT += xs2_b^T @ R_b ---
            poolT_ps = spsum.tile([D, NUM_GRAPHS], f32, name="poolT_ps")
            xs2 = persist.tile([128, NT, D], f32, name="xs2")
            NF = NT if "noF" not in ablate else 0
            for b0 in range(0, NF, 8):
                bn = min(8, NF - b0)
                h1T = work.tile([D, 1024], f32, tag="h1T", name="h1T", bufs=3)
                nc.scalar.activation(
                    h1T[:, : bn * 128],
                    hT_big[:, b0 * 128 : (b0 + bn) * 128],
                    mb.ActivationFunctionType.Relu,
                    bias=c_sb[:], scale=a_sb[:],
                )
                nm_ps = fpsum.tile([128, 8 * D], f32, tag="ps_c", name="nm_ps")
                for j in range(bn):
                    nc.tensor.transpose(
                        out=nm_ps[:, j * D : (j + 1) * D],
                        in_=h1T[:, j * 128 : (j + 1) * 128],
                        identity=ident[:D, :D],
                    )
                nc.vector.tensor_tensor(
                    out=xs2[:, b0 : b0 + bn, :], in0=nm_ps[:, : bn * D].rearrange(
                        "p (g d) -> p g d", d=D),
                    in1=dinvs[:, b0 : b0 + bn].rearrange(
                        "p (g o) -> p g o", o=1).to_broadcast([128, bn, D]),
                    op=mybir.AluOpType.mult,
                )
                for j in range(bn):
                    b = b0 + j
                    nc.tensor.matmul(
                        out=poolT_ps[:], lhsT=xs2[:, b, :], rhs=rkt_t[:, b, :],
                        start=(b == 0), stop=(b == NF - 1),
                    )

            # --- output: pool @ W2 + p1^T b2; AllGather + local reduce ---
            poolT_sb = persist.tile([D, NUM_GRAPHS], f32, name="poolT_sb")
            nc.vector.tensor_copy(out=poolT_sb[:], in_=poolT_ps[:])
            out_ps = wpsum.tile([NUM_GRAPHS, D], f32, tag="ps_a", name="out_ps")
            nc.tensor.matmul(out=out_ps[:], lhsT=poolT_sb[:], rhs=w2s[:], start=True, stop=False)
            nc.tensor.matmul(out=out_ps[:], lhsT=p1s[:], rhs=b2r[:], start=False, stop=True)
            out_sb = persist.tile([NUM_GRAPHS, D], f16, name="out_sb")
            nc.vector.tensor_copy(out=out_sb[:], in_=out_ps[:])
            nc.sync.dma_start(out=ago_in[0:NUM_GRAPHS, :], in_=out_sb[:])
            if "nocc" not in ablate:
                cc("ReduceScatter", mybir.AluOpType.add, ago_in[:, :], ago_out[:, :])
            else:
                nc.sync.dma_start(out=ago_out[:, :], in_=ago_in[0:NUM_GRAPHS, :])
            og = persist.tile([NUM_GRAPHS, D], f16, name="og")
            nc.scalar.dma_start(out=og[:], in_=ago_out[:, :])
            og32 = persist.tile([NUM_GRAPHS, D], f32, name="og32")
            nc.vector.tensor_copy(out=og32[:], in_=og[:])
            nc.sync.dma_start(out=out_d[:, :], in_=og32[:])

    nc.compile()
    return nc


def _wrap16(v, n):
    """idx j at [j%16, j//16], replicated to 128 partitions (8 Q7 cores)."""
    assert v.shape[0] == n and n % 16 == 0
    t = v.astype(np.int16).reshape(n // 16, 16).T
    return np.tile(t, (8, 1))


def _bin_edges(gsrc, grow, nbins, nspill):
    """Rotation binning: occurrence o of dst row r -> bin (r + o) % nbins for
    o < nbins; higher occurrences spill into one extra bin per occurrence
    level (occurrence levels have unique rows by construction)."""
    order = np.argsort(grow, kind="stable")
    sd, ss = grow[order], gsrc[order]
    out = [(np.zeros(0, np.int64), np.zeros(0, np.int64))] * (nbins + nspill)
    if sd.shape[0] == 0:
        return out
    change = np.r_[True, sd[1:] != sd[:-1]]
    starts = np.flatnonzero(change)
    gid = np.cumsum(change) - 1
    occ = np.arange(sd.shape[0]) - starts[gid]
    assert int(occ.max()) < nbins + nspill, (int(occ.max()), nbins, nspill)
    b = np.where(occ < nbins, (sd + occ) % nbins, occ)
    return [(ss[b == i], sd[b == i]) for i in range(nbins + nspill)]


def prepare_inputs(cfg, x, edge_index, batch, W1, b1, gamma, beta, W2, b2):
    """Host-side index preprocessing + per-core input maps.  Fills cfg.seg."""
    SL, SLP, NT = cfg.SL, cfg.SLP, cfg.NT
    n = cfg.N

    x = np.ascontiguousarray(np.asarray(x, dtype=np.float32))
    src = np.asarray(edge_index[0], dtype=np.int64)
    dst = np.asarray(edge_index[1], dtype=np.int64)
    batch = np.asarray(batch, dtype=np.int64)
    W1 = np.asarray(W1, dtype=np.float32)
    b1 = np.asarray(b1, dtype=np.float32)
    gamma = np.asarray(gamma, dtype=np.float32)
    beta = np.asarray(beta, dtype=np.float32)
    W2 = np.asarray(W2, dtype=np.float32)
    b2 = np.asarray(b2, dtype=np.float32)

    deg = np.bincount(dst, minlength=n).astype(np.float32) + 1.0  # + self-loop
    dinv = (1.0 / np.sqrt(deg)).astype(np.float32)

    cnt = np.bincount(batch, minlength=NUM_GRAPHS).astype(np.float32)
    w_graph = 1.0 / np.maximum(cnt, 1.0)
    pd = w_graph[batch] * dinv          # P[batch[v], v] * dinv_v  per node

    owner = src // SL
    src_local = src - owner * SL

    # dst -> (parity, acc row): node (p, g) of core k ->
    # row k*SZJ + p*HT + g//2, column half g%2
    d_owner = dst // SL
    d_local = dst - d_owner * SL
    d_g = d_local // 128
    d_p = d_local - d_g * 128
    d_par = d_g % 2
    d_row = d_owner * cfg.SZJ + d_p * cfg.HT + d_g // 2

    per_core = [[None, None] for _ in range(NCORES)]
    for k in range(NCORES):
        sel = owner == k
        es, ed, ec = src_local[sel], d_row[sel], d_par[sel]
        for c in (0, 1):
            m = ec == c
            per_core[k][c] = (es[m], ed[m])

    # shared bin layout per parity
    seg, core_bins = [], [[] for _ in range(NCORES)]
    for c in (0, 1):
        counts = [per_core[k][c][0].shape[0] for k in range(NCORES)]
        mm = 1
        for k in range(NCORES):
            rows = per_core[k][c][1]
            if rows.shape[0]:
                mm = max(mm, int(np.bincount(rows).max()))
        nbins = max(-(-max(counts) // (cfg.CAP - 256)), 1)
        while True:
            nspill = max(mm - nbins, 0)
            allb = [
                _bin_edges(per_core[k][c][0], per_core[k][c][1], nbins, nspill)
                for k in range(NCORES)
            ]
            sizes = [
                ((max(allb[k][i][0].shape[0] for k in range(NCORES)) + 127)
                 // 128) * 128
                for i in range(nbins + nspill)
            ]
            if all(s <= cfg.CAP for s in sizes):
                break
            nbins += 1
        for i in range(nbins + nspill):
            if sizes[i] == 0:
                continue
            seg.append((c, sizes[i]))
            for k in range(NCORES):
                core_bins[k].append((c, sizes[i], allb[k][i]))

    cfg.seg = seg

    in_maps = []
    for k in range(NCORES):
        gl_parts, sc_parts = [], []
        for c, size, (es, ed) in core_bins[k]:
            m = es.shape[0]
            g = np.zeros(size, dtype=np.int64)
            s = np.full(size, cfg.TRASH[c], dtype=np.int64)  # dead row (dinv=0)
            order = np.argsort(es, kind="stable")  # src-sorted for locality
            g[:m] = es[order]
            s[:m] = ed[order]
            gl_parts.append(_wrap16(g, size))
            sc_parts.append(_wrap16(s, size))
        gidx = np.concatenate(gl_parts, axis=1)
        sidx = np.concatenate(sc_parts, axis=1)

        lo, hi = k * SL, min((k + 1) * SL, n)
        nsl = hi - lo
        xsl = np.zeros((SLP, D), dtype=np.float32)
        xsl[:nsl] = x[lo:hi]
        xsl_pm = np.ascontiguousarray(
            xsl.reshape(NT, 128, D).transpose(1, 0, 2).reshape(128, NT * D)
        )
        dsl = np.zeros(SLP, dtype=np.float32)
        dsl[:nsl] = dinv[lo:hi]
        dinv_in = dsl.reshape(NT, 128).T.copy()

        # R_k^T [SLP, 64]: R_kT[u, g] = sum_{edges (k*SL+u) -> w} P[g,w]*dinv_w
        #                             + P[g, k*SL+u]*dinv_{k*SL+u}
        sel = owner == k
        rkt = np.zeros((SLP, NUM_GRAPHS), dtype=np.float32)
        np.add.at(rkt, (src_local[sel], batch[dst[sel]]), pd[dst[sel]])
        rkt[np.arange(nsl), batch[lo:hi]] += pd[lo:hi]
        rkt_pm = np.ascontiguousarray(
            rkt.reshape(NT, 128, NUM_GRAPHS).transpose(1, 0, 2).reshape(128, -1)
        )

        p1 = np.zeros((1, NUM_GRAPHS), dtype=np.float32)
        np.add.at(p1[0], batch[lo:hi], w_graph[batch[lo:hi]])

        in_maps.append({
            "xsl": xsl_pm,
            "dinv_in": dinv_in,
            "gidx": np.ascontiguousarray(gidx),
            "sidx": np.ascontiguousarray(sidx),
            "rkt": rkt_pm,
            "p1": p1,
            "w1": W1,
            "b1": b1.reshape(D, 1),
            "ga": gamma.reshape(D, 1),
            "be": beta.reshape(D, 1),
            "w2": W2,
            "b2": b2.reshape(1, D),
        })
    return in_maps


def kernel(x, edge_index, batch, W1, b1, gamma, beta, W2, b2):
    global LAST_EXEC_TIME_NS
    from concourse.bass_utils import run_bass_kernel_spmd

    cfg = Cfg(N_NODES, N_NODES // NCORES)
    in_maps = prepare_inputs(cfg, x, edge_index, batch, W1, b1, gamma, beta, W2, b2)

    key = (cfg.N, cfg.SL, tuple(cfg.seg))
    if key not in _NC_CACHE:
        _NC_CACHE[key] = build(cfg)
    nc = _NC_CACHE[key]
    global _LAST_IN_MAPS
    _LAST_IN_MAPS = in_maps

    trace = bool(int(os.environ.get("BASS_GNN_TRACE", "0")))
    if trace:
        try:
            res = run_bass_kernel_spmd(nc, in_maps, list(range(NCORES)), trace=True)
        except Exception:
            res = run_bass_kernel_spmd(nc, in_maps, list(range(NCORES)), trace=False)
    else:
        res = run_bass_kernel_spmd(nc, in_maps, list(range(NCORES)), trace=False)
    LAST_EXEC_TIME_NS = res.exec_time_ns
    return np.asarray(res.results[0]["out"], dtype=np.float32)


def modeled_time_ns(x=None, edge_index=None, **kw):
    """Cost-model execution time (MultiCoreSim, mocked collectives) for the
    current cached program; used when NTFF tracing is unavailable."""
    if not _NC_CACHE:
        return None
    nc = next(iter(_NC_CACHE.values()))
    ins = _LAST_IN_MAPS
    if ins is None:
        return None
    from concourse.bass_interp import MultiCoreSim

    sim = MultiCoreSim(nc, 2, debug_mock_collectives_without_correctness=True)
    for i, core in sim.cores.items():
        for name, val in ins[i].items():
            core.tensor(name)[:] = val
    sim.simulate()
    return int(sim.global_time)


# revision 90
# speedup vs baseline: 1.4735x; 1.0005x over previous
# GCN (2-layer GCNConv + BatchNorm + ReLU + global mean pool) on 8 TRN2 NeuronCores.
#
# Math (reference):
#   deg[v]  = in-degree incl. self-loop;  dinv = deg^-1/2
#   layer(x, W, b): h = D^-1/2 (A+I) D^-1/2 (x W) + b
#   h1 = relu(batchnorm(layer1));  h2 = layer2(h1);  out = segment_mean(h2, batch)
#
# Sharding (v4 — source-partitioned edges + fp16 ReduceScatter):
#   Core k owns nodes [k*SL, (k+1)*SL) and all edges whose SRC falls in that
#   range.  Layer 1:
#     * xs = dinv * x (own slice) -> local f32 gather table (DRAM); the edge
#       gather needs NO collective at all.
#     * per-edge: dma_gather xs[src] rows from the local table (f32, 256B
#       elems), convert the message tile to fp16 on DVE (hidden behind the
#       Pool-engine gather/scatter stream), then dma_scatter_add into a
#       global fp16 accumulator at the dst row.  The accumulator packs two
#       nodes per 256B row (scatter rows need 256B stride); node (p, g) of
#       core k lives at row k*SZJ + p*(NT/2) + g//2, column half g%2, so
#       scatter instructions are split by tile parity.
#     * one fp16 ReduceScatter hands each core the reduced rows of its own
#       slice (half the bytes of f32 — collective cost tracks output size).
#     * self-loops are folded in AFTER the ReduceScatter as one vector add
#       (z + xs) instead of 12.5k extra scatter slots.
#   BatchNorm stats via an accumulated A^T[A|1] matmul + algebraic reduction.
#   The tiny [64,65] stats reduction and the final [64,64] output reduction
#   use AllGather + local vector adds (cheaper than AllReduce).
#   Layer 2 + pooling collapse into dense matmuls: mean-pool P and the outer
#   D^-1/2 are linear, so out = sum_k (R_k @ xs2_k) W2 + b2 with
#   R_k[g, u] = sum_{edges u->w owned by k} P[g,w] dinv_w (+ self term),
#   built on the host from pure index data.  No second edge phase, no second
#   table, no second big collective.
#
# dma_scatter_add races (loses updates) for duplicate dst rows within one
# instruction, so edges are packed into instruction "bins" with unique dst
# rows per bin via rotation binning: occurrence o of dst row r goes to bin
# (r + o) % nbins.  The accumulator has 25600 rows, so scatter indices fit
# int16 with no bucketing.  Pad slots gather row 0 and scatter into a dead
# (dinv=0, zero-padded) node row, one per tile parity.
#
# Host-side preprocessing uses only index data (edge_index, batch): degree
# computation, edge partitioning/binning, the R_k pooling matrices.  Feature
# data is never touched on the host.

import os

import numpy as np

N_NODES = 50000
N_EDGES = 800000
D = 64
NCORES = 8
NUM_GRAPHS = 64
BN_EPS = 1e-5


class Cfg:
    def __init__(self, n, sl):
        self.N = n                    # total nodes
        self.SL = sl                  # owned nodes per core
        slp = ((sl + 127) // 128) * 128
        if (slp // 128) % 2:
            slp += 128                # even tile count (node-pair packing)
        self.SLP = slp
        assert self.SL < self.SLP
        self.NT = self.SLP // 128     # 128-row node tiles per slice (even)
        self.HT = self.NT // 2
        self.SZ = 128 * self.HT       # acc rows per core
        self.SZJ = self.SZ            # pads reuse dead node rows (no junk tile)
        # dead (dinv=0) rows in core 0's block, one per tile parity, used as
        # scatter-pad targets; races there only lose junk
        assert self.SL <= (self.NT - 1) * 128
        p0 = self.SL - (self.NT - 2) * 128          # first dead p in tile NT-2
        assert 0 <= p0 < 128
        self.TRASH = [p0 * self.HT + (self.NT - 2) // 2,
                      0 * self.HT + (self.NT - 1) // 2]
        self.CAP = 7680               # max slots per gather/scatter instruction
        # per-instruction (parity, padded slot count); filled by prepare_inputs
        self.seg = []


LAST_EXEC_TIME_NS = None
_NC_CACHE = {}
_LAST_IN_MAPS = None


def build(cfg):
    import concourse.mybir as mybir
    import concourse.tile as tile
    from concourse import bacc
    from concourse.bass import BassGpSimd
    from concourse.masks import make_identity

    f32 = mybir.dt.float32
    f16 = mybir.dt.float16
    i16 = mybir.dt.int16
    SLP, NT = cfg.SLP, cfg.NT
    NN = float(cfg.N)
    RG = [list(range(NCORES))]
    segs = cfg.seg
    tot_s = sum(c for _, c in segs)
    ACC_R = NCORES * cfg.SZJ

    nc = bacc.Bacc(
        "TRN2", target_bir_lowering=False, debug=False, num_devices=NCORES
    )

    # --- external inputs (per-core values supplied via in_maps) ---
    xsl = nc.declare_dram_parameter("xsl", [128, NT * D], f32, isOutput=False)
    dinv_in = nc.declare_dram_parameter("dinv_in", [128, NT], f32, isOutput=False)
    gidx_d = nc.declare_dram_parameter("gidx", [128, tot_s // 16], i16, isOutput=False)
    sidx_d = nc.declare_dram_parameter("sidx", [128, tot_s // 16], i16, isOutput=False)
    rkt_d = nc.declare_dram_parameter("rkt", [128, NT * D], f32, isOutput=False)
    p1_d = nc.declare_dram_parameter("p1", [1, NUM_GRAPHS], f32, isOutput=False)
    w1_d = nc.declare_dram_parameter("w1", [D, D], f32, isOutput=False)
    b1_d = nc.declare_dram_parameter("b1", [D, 1], f32, isOutput=False)
    ga_d = nc.declare_dram_parameter("ga", [D, 1], f32, isOutput=False)
    be_d = nc.declare_dram_parameter("be", [D, 1], f32, isOutput=False)
    w2_d = nc.declare_dram_parameter("w2", [D, D], f32, isOutput=False)
    b2_d = nc.declare_dram_parameter("b2", [1, D], f32, isOutput=False)
    out_d = nc.declare_dram_parameter("out", [NUM_GRAPHS, D], f32, isOutput=True)

    # --- internal DRAM ---
    table1 = nc.dram_tensor("table1", [SLP, D], f32)
    acc = nc.dram_tensor("acc", [ACC_R, 2 * D], f16)
    rs_out = nc.dram_tensor("rs_out", [cfg.SZJ, 2 * D], f16)
    # reductions as replicated-input ReduceScatters: writing this core's
    # partial into all 8 input blocks makes the RS hand every core the full
    # sum (stats), or core 0 the full sum (output — the only core read)
    ags_in = nc.dram_tensor("ags_in", [NCORES * D, D + 1], f16)
    ags_out = nc.dram_tensor("ags_out", [D, D + 1], f16)
    ago_in = nc.dram_tensor("ago_in", [NCORES * NUM_GRAPHS, D], f16)
    ago_out = nc.dram_tensor("ago_out", [NUM_GRAPHS, D], f16)

    cc_eng = os.environ.get("GNN_CC_ENG", "pool")

    def cc(kind, op, ins_ap, outs_ap):
        BassGpSimd.collective_compute(
            nc.gpsimd if cc_eng == "pool" else getattr(nc, cc_eng),
            kind, op, replica_groups=RG, ins=[ins_ap], outs=[outs_ap],
        )

    with tile.TileContext(nc) as tc:
        with (
            tc.tile_pool(name="const", bufs=1) as const,
            tc.tile_pool(name="persist", bufs=1) as persist,
            tc.tile_pool(name="work", bufs=2) as work,
            tc.tile_pool(name="msgp", bufs=3) as msgp,
            tc.tile_pool(name="msghp", bufs=2) as msghp,
            tc.tile_pool(name="spsum", bufs=1, space="PSUM") as spsum,
            tc.tile_pool(name="wpsum", bufs=2, space="PSUM") as wpsum,
            tc.tile_pool(name="fpsum", bufs=2, space="PSUM") as fpsum,
        ):
            ablate = os.environ.get("GNN_ABLATE", "")

            # --- zero tile for accumulator init (fp16) ---
            ZW = ACC_R * 2 * D // 8 // 128
            zt = persist.tile([128, ZW], f16, name="zt")
            nc.vector.memset(zt[:], 0.0)

            # --- phase A inputs: x slice halves in separate tiles so the
            #     scale/table pipeline isn't serialized by whole-tensor deps ---
            HN = NT // 2
            xs_a = persist.tile([128, HN, D], f32, name="xs_a")
            xs_b = persist.tile([128, NT - HN, D], f32, name="xs_b")
            xsl_v = xsl[:, :].rearrange("p (g d) -> p g d", d=D)
            nc.sync.dma_start(out=xs_a[:], in_=xsl_v[:, :HN, :])
            nc.sync.dma_start(out=xs_b[:], in_=xsl_v[:, HN:, :])
            gidx_t = persist.tile([128, tot_s // 16], i16, name="gidx_t")
            nc.scalar.dma_start(out=gidx_t[:], in_=gidx_d[:, :])

            # --- constants into SBUF (Pool is idle until the first gather);
            #     dinvs first: the phase-A scale waits on it ---
            dinvs = const.tile([128, NT], f32)
            nc.gpsimd.dma_start(out=dinvs[:], in_=dinv_in[:, :])
            w1s = const.tile([D, D], f32)
            nc.gpsimd.dma_start(out=w1s[:], in_=w1_d[:, :])
            w2s = const.tile([D, D], f32)
            nc.gpsimd.dma_start(out=w2s[:], in_=w2_d[:, :])
            b1c = const.tile([D, 1], f32)
            nc.gpsimd.dma_start(out=b1c[:], in_=b1_d[:, :])
            gac = const.tile([D, 1], f32)
            nc.gpsimd.dma_start(out=gac[:], in_=ga_d[:, :])
            bec = const.tile([D, 1], f32)
            nc.gpsimd.dma_start(out=bec[:], in_=be_d[:, :])
            b2r = const.tile([1, D], f32)
            nc.gpsimd.dma_start(out=b2r[:], in_=b2_d[:, :])
            p1s = const.tile([1, NUM_GRAPHS], f32)
            nc.gpsimd.dma_start(out=p1s[:], in_=p1_d[:, :])

            # --- phase A: xs = dinv * x -> local gather table (split SP/Act),
            #     interleaved with the 8 accumulator zero chunks ---
            dinv_b = dinvs[:, :].rearrange("p (g o) -> p g o", o=1).to_broadcast(
                [128, NT, D]
            )
            nc.vector.tensor_tensor(
                out=xs_a[:], in0=xs_a[:],
                in1=dinvs[:, :HN].rearrange("p (g o) -> p g o", o=1)
                .to_broadcast([128, HN, D]),
                op=mybir.AluOpType.mult,
            )
            nc.vector.tensor_tensor(
                out=xs_b[:], in0=xs_b[:],
                in1=dinvs[:, HN:].rearrange("p (g o) -> p g o", o=1)
                .to_broadcast([128, NT - HN, D]),
                op=mybir.AluOpType.mult,
            )
            # fp16 copies of xs / dinv (and identity) for the packed-DVE
            # post-RS path; built early so they hide under the edge phase
            xs16 = persist.tile([128, NT, D], f16, name="xs16")
            nc.vector.tensor_copy(out=xs16[:, :HN, :], in_=xs_a[:])
            nc.vector.tensor_copy(out=xs16[:, HN:, :], in_=xs_b[:])
            dinv16 = persist.tile([128, NT, D], f16, name="dinv16")
            nc.vector.tensor_copy(out=dinv16[:], in_=dinv_b)
            acc_flat = acc[:, :].rearrange("n d -> (n d)")

            def zchunk(eng, j):
                ap = acc_flat.rearrange("(j p x) -> j p x", j=8, p=128)[j]
                eng.dma_start(out=ap, in_=zt[:, :])

            tview = table1[:, :].rearrange("(g p) d -> p g d", p=128)
            zchunk(nc.gpsimd, 0)
            zchunk(nc.gpsimd, 1)
            zchunk(nc.gpsimd, 2)
            zchunk(nc.gpsimd, 3)
            nc.sync.dma_start(out=tview[:, :HN, :], in_=xs_a[:])
            nc.scalar.dma_start(out=tview[:, HN:, :], in_=xs_b[:])
            # sidx in halves so the Act queue can slot the table write between
            sidx_t = persist.tile([128, tot_s // 16], i16, name="sidx_t")
            SH = (tot_s // 16) // 2
            nc.scalar.dma_start(out=sidx_t[:, :SH], in_=sidx_d[:, :SH])
            nc.scalar.dma_start(out=sidx_t[:, SH:], in_=sidx_d[:, SH:])
            zchunk(nc.sync, 4)
            zchunk(nc.sync, 5)
            zchunk(nc.scalar, 6)
            zchunk(nc.scalar, 7)
            # zero blocks 1..7 of the output-reduction RS input (core 0's
            # received chunk is the only one that matters)
            nc.sync.dma_start(
                out=ago_in[NUM_GRAPHS:, :].rearrange(
                    "(j p) d -> p j d", p=NUM_GRAPHS),
                in_=zt[0:NUM_GRAPHS, : (NCORES - 1) * D].rearrange(
                    "p (j d) -> p j d", d=D),
            )

            # identity / BN constants / activation-table warmup — needed only
            # from phase D on, emitted after the edge-phase-critical work
            ident = const.tile([128, 128], f32)
            make_identity(nc, ident[:])
            ones64 = const.tile([D, 1], f32)
            nc.vector.memset(ones64[:], 1.0)
            epsc = const.tile([D, 1], f32)
            nc.vector.memset(epsc[:], BN_EPS)
            warm = const.tile([D, 1], f32)
            nc.scalar.activation(warm[:], epsc[:], mybir.ActivationFunctionType.Sqrt)
            nc.scalar.activation(warm[:], epsc[:], mybir.ActivationFunctionType.Relu)

            # --- R_k^T for layer 2 (loaded during the edge phase) ---
            rkt_t = persist.tile([128, NT, D], f32, name="rkt_t")
            nc.sync.dma_start(
                out=rkt_t[:], in_=rkt_d[:, :].rearrange("p (g d) -> p g d", d=D)
            )

            # --- edge phase: gather f32 / convert fp16 / scatter-add fp16 ---
            import concourse.mybir as mb

            if "noedge" not in ablate:
                pend = None
                off = 0
                for c, cnt in segs:
                    msg = msgp.tile([128, cfg.CAP // 128, D], f32, tag="msg",
                                    name="msg")
                    nc.gpsimd.dma_gather(
                        out_ap=msg[:, : cnt // 128, :],
                        in_ap=table1[0:SLP, :],
                        idxs_ap=gidx_t[:, off : off + cnt // 16],
                        num_idxs=cnt, num_idxs_reg=cnt, elem_size=D,
                        single_packet=False, queue_num=0,
                    )
                    msgh = msghp.tile([128, cfg.CAP // 128, D], f16, tag="msgh",
                                      name="msgh")
                    nc.vector.tensor_copy(
                        out=msgh[:, : cnt // 128, :], in_=msg[:, : cnt // 128, :]
                    )
                    if pend is not None:
                        nc.gpsimd.dma_scatter_add(*pend, elem_step=2 * D, single_packet=False, queue_num=0)
                    pend = (
                        acc[:, c * D : (c + 1) * D],
                        msgh[:, : cnt // 128, :],
                        sidx_t[:, off : off + cnt // 16],
                        cnt, cnt, D,
                    )
                    off += cnt // 16
                if pend is not None:
                    nc.gpsimd.dma_scatter_add(*pend, elem_step=2 * D, single_packet=False, queue_num=0)

            # --- ReduceScatter -> own reduced slice (fp16) ---
            if "nocc" not in ablate:
                cc("ReduceScatter", mybir.AluOpType.add, acc[:, :], rs_out[:, :])
            else:
                nc.sync.dma_start(out=rs_out[:, :], in_=acc[0 : cfg.SZJ, :])

            # warm the PE p-state during the collective (it idles otherwise,
            # and the first post-RS matmuls would run at the slow p-state)
            warm_ps = fpsum.tile([D, D], f32, tag="ps_c", name="warm_ps")
            for _ in range(24):
                nc.tensor.matmul(
                    out=warm_ps[:], lhsT=ident[:D, :D], rhs=ident[:D, :D],
                    start=True, stop=True,
                )

            # --- dense layer-1: z + self-loop, dinv scale, stats, W1 ---
            # z in two tiles loaded on SP and Act in parallel; the aggs
            # chunks start as soon as the first half lands
            z_a = persist.tile([128, HN, D], f16, name="z_a")
            z_b = persist.tile([128, NT - HN, D], f16, name="z_b")
            zsrc = rs_out[0 : cfg.SZ, :].rearrange("(p r) c -> p (r c)", p=128)
            HW_ = NT * D // 2
            nc.sync.dma_start(
                out=z_a[:].rearrange("p g d -> p (g d)"), in_=zsrc[:, :HW_]
            )
            nc.scalar.dma_start(
                out=z_b[:].rearrange("p g d -> p (g d)"), in_=zsrc[:, HW_:]
            )

            def z_sl(b0, bn):
                if b0 >= HN:
                    return z_b[:, b0 - HN : b0 - HN + bn, :]
                return z_a[:, b0 : b0 + bn, :]

            # keep the PE p-state warm through the z load so the stats
            # matmuls run at full clock
            for _ in range(10):
                nc.tensor.matmul(
                    out=warm_ps[:], lhsT=z_a[:, 0, :], rhs=z_a[:, 0, :],
                    start=True, stop=True,
                )
            aggs = persist.tile([128, NT, D + 1], f16, name="aggs")
            nc.vector.memset(aggs[:, :, D : D + 1], 1.0)
            ident16 = persist.tile([128, 128], f16, name="ident16")
            nc.vector.tensor_copy(out=ident16[:], in_=ident[:])
            w1s16 = persist.tile([D, D], f16, name="w1s16")
            nc.vector.tensor_copy(out=w1s16[:], in_=w1s[:])

            stats_ps = spsum.tile([D, D + 1], f32, name="stats_ps")
            hT_big = persist.tile([D, NT * 128], f32, name="hT_big")
            ND = NT if "noD" not in ablate else 1
            # pipeline the (z+xs)*dinv prep with the stats matmuls per chunk;
            # everything fp16 so the DVE runs in its packed 2x mode; chunks
            # never straddle the z_a/z_b boundary
            chunks = [(0, 10), (10, 20), (20, HN), (HN, 35), (35, 45), (45, ND)]
            chunks = [(a, min(b, ND)) for a, b in chunks if a < ND]
            for b0, b1 in chunks:
                bn = b1 - b0
                sl = slice(b0, b0 + bn)
                nc.vector.tensor_tensor(
                    out=aggs[:, sl, :D], in0=z_sl(b0, bn), in1=xs16[:, sl, :],
                    op=mybir.AluOpType.add,
                )
                nc.vector.tensor_tensor(
                    out=aggs[:, sl, :D], in0=aggs[:, sl, :D],
                    in1=dinv16[:, sl, :], op=mybir.AluOpType.mult,
                )
                for b in range(b0, b0 + bn):
                    nc.tensor.matmul(
                        out=stats_ps[:], lhsT=aggs[:, b, :D], rhs=aggs[:, b, :],
                        start=(b == 0), stop=(b == ND - 1),
                    )
            # stats reduction launched before the transposes/W1 matmuls so the
            # collective overlaps with PE work: replicate the local stats into
            # all 8 RS input blocks -> every core's RS output = global sum
            stg8 = persist.tile([D, NCORES, D + 1], f16, name="stg8")
            nc.vector.tensor_copy(
                out=stg8[:],
                in_=stats_ps[:].rearrange("p (o c) -> p o c", o=1)
                .to_broadcast([D, NCORES, D + 1]),
            )
            nc.sync.dma_start(
                out=ags_in[:, :].rearrange("(j p) c -> p j c", p=D), in_=stg8[:]
            )
            if "nocc" not in ablate:
                cc("ReduceScatter", mybir.AluOpType.add, ags_in[:, :], ags_out[:, :])
            else:
                nc.sync.dma_start(out=ags_out[:, :], in_=ags_in[0:D, :])

            for b0 in range(0, ND, 4):
                bn = min(4, ND - b0)
                tp_ps = wpsum.tile([D, 512], f16, tag="ps_a", name="tp_ps")
                for j in range(bn):
                    nc.tensor.transpose(
                        out=tp_ps[:, j * 128 : (j + 1) * 128],
                        in_=aggs[:, b0 + j, :D], identity=ident16[:],
                    )
                aggsT = work.tile([D, 512], f16, tag="aggsT", name="aggsT", bufs=2)
                nc.vector.tensor_copy(out=aggsT[:, : bn * 128], in_=tp_ps[:, : bn * 128])
                hT_ps = wpsum.tile([D, 512], f32, tag="ps_b", name="hT_ps")
                nc.tensor.matmul(
                    out=hT_ps[:, : bn * 128], lhsT=w1s16[:], rhs=aggsT[:, : bn * 128],
                    start=True, stop=True,
                )
                nc.vector.tensor_copy(
                    out=hT_big[:, b0 * 128 : (b0 + bn) * 128],
                    in_=hT_ps[:, : bn * 128],
                )

            st16 = persist.tile([D, D + 1], f16, name="st16")
            nc.scalar.dma_start(out=st16[:], in_=ags_out[:, :])
            st32 = persist.tile([D, D + 1], f32, name="st32")
            nc.vector.tensor_copy(out=st32[:], in_=st16[:])
            st = st32[:]

            # --- BN scalar algebra ---
            q_ps = wpsum.tile([D, 1], f32, tag="ps_a", name="q_ps")
            nc.tensor.matmul(out=q_ps[:], lhsT=w1s[:], rhs=st[:, D : D + 1], start=True, stop=True)
            mu = persist.tile([D, 1], f32, name="mu")
            nc.vector.tensor_scalar(
                out=mu[:], in0=q_ps[:], scalar1=1.0 / NN, scalar2=b1c[:],
                op0=mybir.AluOpType.mult, op1=mybir.AluOpType.add,
            )
            t1_ps = wpsum.tile([D, D], f32, tag="ps_b", name="t1_ps")
            nc.tensor.matmul(out=t1_ps[:], lhsT=st[:, :D], rhs=w1s[:], start=True, stop=True)
            m_sb = work.tile([D, D], f32, tag="m_sb", name="m_sb")
            nc.vector.tensor_tensor(out=m_sb[:], in0=w1s[:], in1=t1_ps[:], op=mybir.AluOpType.mult)
            d_ps = wpsum.tile([D, 1], f32, tag="ps_b", name="d_ps")
            nc.tensor.matmul(out=d_ps[:], lhsT=m_sb[:], rhs=ones64[:], start=True, stop=True)

            var = persist.tile([D, 1], f32, name="var")
            nc.vector.tensor_scalar_mul(out=var[:], in0=d_ps[:], scalar1=1.0 / NN)
            t2 = work.tile([D, 1], f32, tag="t2", name="t2")
            nc.vector.tensor_scalar_mul(out=t2[:], in0=q_ps[:], scalar1=2.0 / NN)
            nc.vector.tensor_tensor(out=t2[:], in0=t2[:], in1=b1c[:], op=mybir.AluOpType.mult)
            nc.vector.tensor_tensor(out=var[:], in0=var[:], in1=t2[:], op=mybir.AluOpType.add)
            t3 = work.tile([D, 1], f32, tag="t3", name="t3")
            nc.vector.tensor_tensor(out=t3[:], in0=b1c[:], in1=b1c[:], op=mybir.AluOpType.mult)
            nc.vector.tensor_tensor(out=var[:], in0=var[:], in1=t3[:], op=mybir.AluOpType.add)
            t4 = work.tile([D, 1], f32, tag="t4", name="t4")
            nc.vector.tensor_tensor(out=t4[:], in0=mu[:], in1=mu[:], op=mybir.AluOpType.mult)
            nc.vector.tensor_tensor(out=var[:], in0=var[:], in1=t4[:], op=mybir.AluOpType.subtract)

            sd = work.tile([D, 1], f32, tag="sd", name="sd")
            nc.scalar.activation(sd[:], var[:], mb.ActivationFunctionType.Sqrt, bias=epsc[:])
            rstd = work.tile([D, 1], f32, tag="rstd", name="rstd")
            nc.vector.reciprocal(out=rstd[:], in_=sd[:])
            a_sb = persist.tile([D, 1], f32, name="a_sb")
            nc.vector.tensor_tensor(out=a_sb[:], in0=gac[:], in1=rstd[:], op=mybir.AluOpType.mult)
            c_sb = persist.tile([D, 1], f32, name="c_sb")
            t5 = work.tile([D, 1], f32, tag="t5", name="t5")
            nc.vector.tensor_tensor(out=t5[:], in0=mu[:], in1=a_sb[:], op=mybir.AluOpType.mult)
            nc.vector.tensor_tensor(out=c_sb[:], in0=bec[:], in1=t5[:], op=mybir.AluOpType.subtract)
            # hT tiles exclude the b1 bias; fold it into the BN offset:
            # relu(a*(h+b1) + c) = relu(a*h + (c + a*b1))
            t6 = work.tile([D, 1], f32, tag="t6", name="t6")
            nc.vector.tensor_tensor(out=t6[:], in0=a_sb[:], in1=b1c[:], op=mybir.AluOpType.mult)
            nc.vector.tensor_tensor(out=c_sb[:], in0=c_sb[:], in1=t6[:], op=mybir.AluOpType.add)

            # --- phase F: BN+ReLU, transpose back, dinv fold -> xs2;
            #     phase G interleaved: poolT += xs2_b^T @ R_b ---
            poolT_ps = spsum.tile([D, NUM_GRAPHS], f32, name="poolT_ps")
            xs2 = persist.tile([128, NT, D], f32, name="xs2")
            NF = NT if "noF" not in ablate else 0
            for b0 in range(0, NF, 8):
                bn = min(8, NF - b0)
                h1T = work.tile([D, 1024], f32, tag="h1T", name="h1T", bufs=3)
                nc.scalar.activation(
                    h1T[:, : bn * 128],
                    hT_big[:, b0 * 128 : (b0 + bn) * 128],
                    mb.ActivationFunctionType.Relu,
                    bias=c_sb[:], scale=a_sb[:],
                )
                nm_ps = fpsum.tile([128, 8 * D], f32, tag="ps_c", name="nm_ps")
                for j in range(bn):
                    nc.tensor.transpose(
                        out=nm_ps[:, j * D : (j + 1) * D],
                        in_=h1T[:, j * 128 : (j + 1) * 128],
                        identity=ident[:D, :D],
                    )
                nc.vector.tensor_tensor(
                    out=xs2[:, b0 : b0 + bn, :], in0=nm_ps[:, : bn * D].rearrange(
                        "p (g d) -> p g d", d=D),
                    in1=dinvs[:, b0 : b0 + bn].rearrange(
                        "p (g o) -> p g o", o=1).to_broadcast([128, bn, D]),
                    op=mybir.AluOpType.mult,
                )
                for j in range(bn):
                    b = b0 + j
                    nc.tensor.matmul(
                        out=poolT_ps[:], lhsT=xs2[:, b, :], rhs=rkt_t[:, b, :],
                        start=(b == 0), stop=(b == NF - 1),
                    )

            # --- output: pool @ W2 + p1^T b2; AllGather + local reduce ---
            poolT_sb = persist.tile([D, NUM_GRAPHS], f32, name="poolT_sb")
            nc.vector.tensor_copy(out=poolT_sb[:], in_=poolT_ps[:])
            out_ps = wpsum.tile([NUM_GRAPHS, D], f32, tag="ps_a", name="out_ps")
            nc.tensor.matmul(out=out_ps[:], lhsT=poolT_sb[:], rhs=w2s[:], start=True, stop=False)
            nc.tensor.matmul(out=out_ps[:], lhsT=p1s[:], rhs=b2r[:], start=False, stop=True)
            out_sb = persist.tile([NUM_GRAPHS, D], f16, name="out_sb")
            nc.vector.tensor_copy(out=out_sb[:], in_=out_ps[:])
            nc.sync.dma_start(out=ago_in[0:NUM_GRAPHS, :], in_=out_sb[:])
            if "nocc" not in ablate:
                cc("ReduceScatter", mybir.AluOpType.add, ago_in[:, :], ago_out[:, :])
            else:
                nc.sync.dma_start(out=ago_out[:, :], in_=ago_in[0:NUM_GRAPHS, :])
            og = persist.tile([NUM_GRAPHS, D], f16, name="og")
            nc.scalar.dma_start(out=og[:], in_=ago_out[:, :])
            og32 = persist.tile([NUM_GRAPHS, D], f32, name="og32")
            nc.vector.tensor_copy(out=og32[:], in_=og[:])
            nc.sync.dma_start(out=out_d[:, :], in_=og32[:])

    nc.compile()
    return nc


def _wrap16(v, n):
    """idx j at [j%16, j//16], replicated to 128 partitions (8 Q7 cores)."""
    assert v.shape[0] == n and n % 16 == 0
    t = v.astype(np.int16).reshape(n // 16, 16).T
    return np.tile(t, (8, 1))


def _bin_edges(gsrc, grow, nbins, nspill):
    """Rotation binning: occurrence o of dst row r -> bin (r + o) % nbins for
    o < nbins; higher occurrences spill into one extra bin per occurrence
    level (occurrence levels have unique rows by construction)."""
    order = np.argsort(grow, kind="stable")
    sd, ss = grow[order], gsrc[order]
    out = [(np.zeros(0, np.int64), np.zeros(0, np.int64))] * (nbins + nspill)
    if sd.shape[0] == 0:
        return out
    change = np.r_[True, sd[1:] != sd[:-1]]
    starts = np.flatnonzero(change)
    gid = np.cumsum(change) - 1
    occ = np.arange(sd.shape[0]) - starts[gid]
    assert int(occ.max()) < nbins + nspill, (int(occ.max()), nbins, nspill)
    b = np.where(occ < nbins, (sd + occ) % nbins, occ)
    return [(ss[b == i], sd[b == i]) for i in range(nbins + nspill)]


def prepare_inputs(cfg, x, edge_index, batch, W1, b1, gamma, beta, W2, b2):
    """Host-side index preprocessing + per-core input maps.  Fills cfg.seg."""
    SL, SLP, NT = cfg.SL, cfg.SLP, cfg.NT
    n = cfg.N

    x = np.ascontiguousarray(np.asarray(x, dtype=np.float32))
    src = np.asarray(edge_index[0], dtype=np.int64)
    dst = np.asarray(edge_index[1], dtype=np.int64)
    batch = np.asarray(batch, dtype=np.int64)
    W1 = np.asarray(W1, dtype=np.float32)
    b1 = np.asarray(b1, dtype=np.float32)
    gamma = np.asarray(gamma, dtype=np.float32)
    beta = np.asarray(beta, dtype=np.float32)
    W2 = np.asarray(W2, dtype=np.float32)
    b2 = np.asarray(b2, dtype=np.float32)

    deg = np.bincount(dst, minlength=n).astype(np.float32) + 1.0  # + self-loop
    dinv = (1.0 / np.sqrt(deg)).astype(np.float32)

    cnt = np.bincount(batch, minlength=NUM_GRAPHS).astype(np.float32)
    w_graph = 1.0 / np.maximum(cnt, 1.0)
    pd = w_graph[batch] * dinv          # P[batch[v], v] * dinv_v  per node

    owner = src // SL
    src_local = src - owner * SL

    # dst -> (parity, acc row): node (p, g) of core k ->
    # row k*SZJ + p*HT + g//2, column half g%2
    d_owner = dst // SL
    d_local = dst - d_owner * SL
    d_g = d_local // 128
    d_p = d_local - d_g * 128
    d_par = d_g % 2
    d_row = d_owner * cfg.SZJ + d_p * cfg.HT + d_g // 2

    per_core = [[None, None] for _ in range(NCORES)]
    for k in range(NCORES):
        sel = owner == k
        es, ed, ec = src_local[sel], d_row[sel], d_par[sel]
        for c in (0, 1):
            m = ec == c
            per_core[k][c] = (es[m], ed[m])

    # shared bin layout per parity
    seg, core_bins = [], [[] for _ in range(NCORES)]
    for c in (0, 1):
        counts = [per_core[k][c][0].shape[0] for k in range(NCORES)]
        mm = 1
        for k in range(NCORES):
            rows = per_core[k][c][1]
            if rows.shape[0]:
                mm = max(mm, int(np.bincount(rows).max()))
        nbins = max(-(-max(counts) // (cfg.CAP - 256)), 1)
        while True:
            nspill = max(mm - nbins, 0)
            allb = [
                _bin_edges(per_core[k][c][0], per_core[k][c][1], nbins, nspill)
                for k in range(NCORES)
            ]
            sizes = [
                ((max(allb[k][i][0].shape[0] for k in range(NCORES)) + 127)
                 // 128) * 128
                for i in range(nbins + nspill)
            ]
            if all(s <= cfg.CAP for s in sizes):
                break
            nbins += 1
        for i in range(nbins + nspill):
            if sizes[i] == 0:
                continue
            seg.append((c, sizes[i]))
            for k in range(NCORES):
                core_bins[k].append((c, sizes[i], allb[k][i]))

    cfg.seg = seg

    in_maps = []
    for k in range(NCORES):
        gl_parts, sc_parts = [], []
        for c, size, (es, ed) in core_bins[k]:
            m = es.shape[0]
            g = np.zeros(size, dtype=np.int64)
            s = np.full(size, cfg.TRASH[c], dtype=np.int64)  # dead row (dinv=0)
            order = np.argsort(es, kind="stable")  # src-sorted for locality
            g[:m] = es[order]
            s[:m] = ed[order]
            gl_parts.append(_wrap16(g, size))
            sc_parts.append(_wrap16(s, size))
        gidx = np.concatenate(gl_parts, axis=1)
        sidx = np.concatenate(sc_parts, axis=1)

        lo, hi = k * SL, min((k + 1) * SL, n)
        nsl = hi - lo
        xsl = np.zeros((SLP, D), dtype=np.float32)
        xsl[:nsl] = x[lo:hi]
        xsl_pm = np.ascontiguousarray(
            xsl.reshape(NT, 128, D).transpose(1, 0, 2).reshape(128, NT * D)
        )
        dsl = np.zeros(SLP, dtype=np.float32)
        dsl[:nsl] = dinv[lo:hi]
        dinv_in = dsl.reshape(NT, 128).T.copy()

        # R_k^T [SLP, 64]: R_kT[u, g] = sum_{edges (k*SL+u) -> w} P[g,w]*dinv_w
        #                             + P[g, k*SL+u]*dinv_{k*SL+u}
        sel = owner == k
        rkt = np.zeros((SLP, NUM_GRAPHS), dtype=np.float32)
        np.add.at(rkt, (src_local[sel], batch[dst[sel]]), pd[dst[sel]])
        rkt[np.arange(nsl), batch[lo:hi]] += pd[lo:hi]
        rkt_pm = np.ascontiguousarray(
            rkt.reshape(NT, 128, NUM_GRAPHS).transpose(1, 0, 2).reshape(128, -1)
        )

        p1 = np.zeros((1, NUM_GRAPHS), dtype=np.float32)
        np.add.at(p1[0], batch[lo:hi], w_graph[batch[lo:hi]])

        in_maps.append({
            "xsl": xsl_pm,
            "dinv_in": dinv_in,
            "gidx": np.ascontiguousarray(gidx),
            "sidx": np.ascontiguousarray(sidx),
            "rkt": rkt_pm,
            "p1": p1,
            "w1": W1,
            "b1": b1.reshape(D, 1),
            "ga": gamma.reshape(D, 1),
            "be": beta.reshape(D, 1),
            "w2": W2,
            "b2": b2.reshape(1, D),
        })
    return in_maps


def kernel(x, edge_index, batch, W1, b1, gamma, beta, W2, b2):
    global LAST_EXEC_TIME_NS
    from concourse.bass_utils import run_bass_kernel_spmd

    cfg = Cfg(N_NODES, N_NODES // NCORES)
    in_maps = prepare_inputs(cfg, x, edge_index, batch, W1, b1, gamma, beta, W2, b2)

    key = (cfg.N, cfg.SL, tuple(cfg.seg))
    if key not in _NC_CACHE:
        _NC_CACHE[key] = build(cfg)
    nc = _NC_CACHE[key]
    global _LAST_IN_MAPS
    _LAST_IN_MAPS = in_maps

    trace = bool(int(os.environ.get("BASS_GNN_TRACE", "0")))
    if trace:
        try:
            res = run_bass_kernel_spmd(nc, in_maps, list(range(NCORES)), trace=True)
        except Exception:
            res = run_bass_kernel_spmd(nc, in_maps, list(range(NCORES)), trace=False)
    else:
        res = run_bass_kernel_spmd(nc, in_maps, list(range(NCORES)), trace=False)
    LAST_EXEC_TIME_NS = res.exec_time_ns
    return np.asarray(res.results[0]["out"], dtype=np.float32)


def modeled_time_ns(x=None, edge_index=None, **kw):
    """Cost-model execution time (MultiCoreSim, mocked collectives) for the
    current cached program; used when NTFF tracing is unavailable."""
    if not _NC_CACHE:
        return None
    nc = next(iter(_NC_CACHE.values()))
    ins = _LAST_IN_MAPS
    if ins is None:
        return None
    from concourse.bass_interp import MultiCoreSim

    sim = MultiCoreSim(nc, 2, debug_mock_collectives_without_correctness=True)
    for i, core in sim.cores.items():
        for name, val in ins[i].items():
            core.tensor(name)[:] = val
    sim.simulate()
    return int(sim.global_time)


# revision 93
# speedup vs baseline: 1.4799x; 1.0044x over previous
# GCN (2-layer GCNConv + BatchNorm + ReLU + global mean pool) on 8 TRN2 NeuronCores.
#
# Math (reference):
#   deg[v]  = in-degree incl. self-loop;  dinv = deg^-1/2
#   layer(x, W, b): h = D^-1/2 (A+I) D^-1/2 (x W) + b
#   h1 = relu(batchnorm(layer1));  h2 = layer2(h1);  out = segment_mean(h2, batch)
#
# Sharding (v4 — source-partitioned edges + fp16 ReduceScatter):
#   Core k owns nodes [k*SL, (k+1)*SL) and all edges whose SRC falls in that
#   range.  Layer 1:
#     * xs = dinv * x (own slice) -> local f32 gather table (DRAM); the edge
#       gather needs NO collective at all.
#     * per-edge: dma_gather xs[src] rows from the local table (f32, 256B
#       elems), convert the message tile to fp16 on DVE (hidden behind the
#       Pool-engine gather/scatter stream), then dma_scatter_add into a
#       global fp16 accumulator at the dst row.  The accumulator packs two
#       nodes per 256B row (scatter rows need 256B stride); node (p, g) of
#       core k lives at row k*SZJ + p*(NT/2) + g//2, column half g%2, so
#       scatter instructions are split by tile parity.
#     * one fp16 ReduceScatter hands each core the reduced rows of its own
#       slice (half the bytes of f32 — collective cost tracks output size).
#     * self-loops are folded in AFTER the ReduceScatter as one vector add
#       (z + xs) instead of 12.5k extra scatter slots.
#   BatchNorm stats via an accumulated A^T[A|1] matmul + algebraic reduction.
#   The tiny [64,65] stats reduction and the final [64,64] output reduction
#   use AllGather + local vector adds (cheaper than AllReduce).
#   Layer 2 + pooling collapse into dense matmuls: mean-pool P and the outer
#   D^-1/2 are linear, so out = sum_k (R_k @ xs2_k) W2 + b2 with
#   R_k[g, u] = sum_{edges u->w owned by k} P[g,w] dinv_w (+ self term),
#   built on the host from pure index data.  No second edge phase, no second
#   table, no second big collective.
#
# dma_scatter_add races (loses updates) for duplicate dst rows within one
# instruction, so edges are packed into instruction "bins" with unique dst
# rows per bin via rotation binning: occurrence o of dst row r goes to bin
# (r + o) % nbins.  The accumulator has 25600 rows, so scatter indices fit
# int16 with no bucketing.  Pad slots gather row 0 and scatter into a dead
# (dinv=0, zero-padded) node row, one per tile parity.
#
# Host-side preprocessing uses only index data (edge_index, batch): degree
# computation, edge partitioning/binning, the R_k pooling matrices.  Feature
# data is never touched on the host.

import os

import numpy as np

N_NODES = 50000
N_EDGES = 800000
D = 64
NCORES = 8
NUM_GRAPHS = 64
BN_EPS = 1e-5


class Cfg:
    def __init__(self, n, sl):
        self.N = n                    # total nodes
        self.SL = sl                  # owned nodes per core
        slp = ((sl + 127) // 128) * 128
        if (slp // 128) % 2:
            slp += 128                # even tile count (node-pair packing)
        self.SLP = slp
        assert self.SL < self.SLP
        self.NT = self.SLP // 128     # 128-row node tiles per slice (even)
        self.HT = self.NT // 2
        self.SZ = 128 * self.HT       # acc rows per core
        self.SZJ = self.SZ            # pads reuse dead node rows (no junk tile)
        # dead (dinv=0) rows in core 0's block, one per tile parity, used as
        # scatter-pad targets; races there only lose junk
        assert self.SL <= (self.NT - 1) * 128
        p0 = self.SL - (self.NT - 2) * 128          # first dead p in tile NT-2
        assert 0 <= p0 < 128
        self.TRASH = [p0 * self.HT + (self.NT - 2) // 2,
                      0 * self.HT + (self.NT - 1) // 2]
        self.CAP = 7680               # max slots per gather/scatter instruction
        # per-instruction (parity, padded slot count); filled by prepare_inputs
        self.seg = []


LAST_EXEC_TIME_NS = None
_NC_CACHE = {}
_LAST_IN_MAPS = None


def build(cfg):
    import concourse.mybir as mybir
    import concourse.tile as tile
    from concourse import bacc
    from concourse.bass import BassGpSimd
    from concourse.masks import make_identity

    f32 = mybir.dt.float32
    f16 = mybir.dt.float16
    i16 = mybir.dt.int16
    SLP, NT = cfg.SLP, cfg.NT
    NN = float(cfg.N)
    RG = [list(range(NCORES))]
    segs = cfg.seg
    tot_s = sum(c for _, c in segs)
    ACC_R = NCORES * cfg.SZJ

    nc = bacc.Bacc(
        "TRN2", target_bir_lowering=False, debug=False, num_devices=NCORES
    )

    # --- external inputs (per-core values supplied via in_maps) ---
    xsl = nc.declare_dram_parameter("xsl", [128, NT * D], f32, isOutput=False)
    dinv_in = nc.declare_dram_parameter("dinv_in", [128, NT], f32, isOutput=False)
    gidx_d = nc.declare_dram_parameter("gidx", [128, tot_s // 16], i16, isOutput=False)
    sidx_d = nc.declare_dram_parameter("sidx", [128, tot_s // 16], i16, isOutput=False)
    rkt_d = nc.declare_dram_parameter("rkt", [128, NT * D], f32, isOutput=False)
    p1_d = nc.declare_dram_parameter("p1", [1, NUM_GRAPHS], f32, isOutput=False)
    w1_d = nc.declare_dram_parameter("w1", [D, D], f32, isOutput=False)
    b1_d = nc.declare_dram_parameter("b1", [D, 1], f32, isOutput=False)
    ga_d = nc.declare_dram_parameter("ga", [D, 1], f32, isOutput=False)
    be_d = nc.declare_dram_parameter("be", [D, 1], f32, isOutput=False)
    w2_d = nc.declare_dram_parameter("w2", [D, D], f32, isOutput=False)
    b2_d = nc.declare_dram_parameter("b2", [1, D], f32, isOutput=False)
    out_d = nc.declare_dram_parameter("out", [NUM_GRAPHS, D], f32, isOutput=True)

    # --- internal DRAM ---
    table1 = nc.dram_tensor("table1", [SLP, D], f32)
    acc = nc.dram_tensor("acc", [ACC_R, 2 * D], f16)
    rs_out = nc.dram_tensor("rs_out", [cfg.SZJ, 2 * D], f16)
    # reductions as replicated-input ReduceScatters: writing this core's
    # partial into all 8 input blocks makes the RS hand every core the full
    # sum (stats), or core 0 the full sum (output — the only core read)
    ags_in = nc.dram_tensor("ags_in", [NCORES * D, D + 1], f16)
    ags_out = nc.dram_tensor("ags_out", [D, D + 1], f16)
    ago_in = nc.dram_tensor("ago_in", [NCORES * NUM_GRAPHS, D], f16)
    ago_out = nc.dram_tensor("ago_out", [NUM_GRAPHS, D], f16)

    cc_eng = os.environ.get("GNN_CC_ENG", "pool")

    def cc(kind, op, ins_ap, outs_ap):
        BassGpSimd.collective_compute(
            nc.gpsimd if cc_eng == "pool" else getattr(nc, cc_eng),
            kind, op, replica_groups=RG, ins=[ins_ap], outs=[outs_ap],
        )

    with tile.TileContext(nc) as tc:
        with (
            tc.tile_pool(name="const", bufs=1) as const,
            tc.tile_pool(name="persist", bufs=1) as persist,
            tc.tile_pool(name="work", bufs=2) as work,
            tc.tile_pool(name="msgp", bufs=3) as msgp,
            tc.tile_pool(name="msghp", bufs=2) as msghp,
            tc.tile_pool(name="spsum", bufs=1, space="PSUM") as spsum,
            tc.tile_pool(name="wpsum", bufs=2, space="PSUM") as wpsum,
            tc.tile_pool(name="fpsum", bufs=2, space="PSUM") as fpsum,
        ):
            ablate = os.environ.get("GNN_ABLATE", "")

            # --- zero tile for accumulator init (fp16) ---
            ZW = ACC_R * 2 * D // 8 // 128
            zt = persist.tile([128, ZW], f16, name="zt")
            nc.vector.memset(zt[:], 0.0)

            # --- phase A inputs: x slice halves in separate tiles so the
            #     scale/table pipeline isn't serialized by whole-tensor deps ---
            HN = NT // 2
            xs_a = persist.tile([128, HN, D], f32, name="xs_a")
            xs_b = persist.tile([128, NT - HN, D], f32, name="xs_b")
            xsl_v = xsl[:, :].rearrange("p (g d) -> p g d", d=D)
            nc.sync.dma_start(out=xs_a[:], in_=xsl_v[:, :HN, :])
            nc.sync.dma_start(out=xs_b[:], in_=xsl_v[:, HN:, :])
            gidx_t = persist.tile([128, tot_s // 16], i16, name="gidx_t")
            nc.scalar.dma_start(out=gidx_t[:], in_=gidx_d[:, :])

            # --- constants into SBUF (Pool is idle until the first gather);
            #     dinvs first: the phase-A scale waits on it ---
            dinvs = const.tile([128, NT], f32)
            nc.gpsimd.dma_start(out=dinvs[:], in_=dinv_in[:, :])
            w1s = const.tile([D, D], f32)
            nc.gpsimd.dma_start(out=w1s[:], in_=w1_d[:, :])
            w2s = const.tile([D, D], f32)
            nc.gpsimd.dma_start(out=w2s[:], in_=w2_d[:, :])
            b1c = const.tile([D, 1], f32)
            nc.gpsimd.dma_start(out=b1c[:], in_=b1_d[:, :])
            gac = const.tile([D, 1], f32)
            nc.gpsimd.dma_start(out=gac[:], in_=ga_d[:, :])
            bec = const.tile([D, 1], f32)
            nc.gpsimd.dma_start(out=bec[:], in_=be_d[:, :])
            b2r = const.tile([1, D], f32)
            nc.gpsimd.dma_start(out=b2r[:], in_=b2_d[:, :])
            p1s = const.tile([1, NUM_GRAPHS], f32)
            nc.gpsimd.dma_start(out=p1s[:], in_=p1_d[:, :])

            # --- phase A: xs = dinv * x -> local gather table (split SP/Act),
            #     interleaved with the 8 accumulator zero chunks ---
            dinv_b = dinvs[:, :].rearrange("p (g o) -> p g o", o=1).to_broadcast(
                [128, NT, D]
            )
            nc.vector.tensor_tensor(
                out=xs_a[:], in0=xs_a[:],
                in1=dinvs[:, :HN].rearrange("p (g o) -> p g o", o=1)
                .to_broadcast([128, HN, D]),
                op=mybir.AluOpType.mult,
            )
            nc.vector.tensor_tensor(
                out=xs_b[:], in0=xs_b[:],
                in1=dinvs[:, HN:].rearrange("p (g o) -> p g o", o=1)
                .to_broadcast([128, NT - HN, D]),
                op=mybir.AluOpType.mult,
            )
            # fp16 copies of xs / dinv (and identity) for the packed-DVE
            # post-RS path; built early so they hide under the edge phase
            xs16 = persist.tile([128, NT, D], f16, name="xs16")
            nc.vector.tensor_copy(out=xs16[:, :HN, :], in_=xs_a[:])
            nc.vector.tensor_copy(out=xs16[:, HN:, :], in_=xs_b[:])
            dinv16 = persist.tile([128, NT, D], f16, name="dinv16")
            nc.vector.tensor_copy(out=dinv16[:], in_=dinv_b)
            acc_flat = acc[:, :].rearrange("n d -> (n d)")

            def zchunk(eng, j):
                ap = acc_flat.rearrange("(j p x) -> j p x", j=8, p=128)[j]
                eng.dma_start(out=ap, in_=zt[:, :])

            tview = table1[:, :].rearrange("(g p) d -> p g d", p=128)
            zchunk(nc.gpsimd, 0)
            zchunk(nc.gpsimd, 1)
            zchunk(nc.gpsimd, 2)
            zchunk(nc.gpsimd, 3)
            nc.sync.dma_start(out=tview[:, :HN, :], in_=xs_a[:])
            nc.scalar.dma_start(out=tview[:, HN:, :], in_=xs_b[:])
            # sidx in halves so the Act queue can slot the table write between
            sidx_t = persist.tile([128, tot_s // 16], i16, name="sidx_t")
            SH = (tot_s // 16) // 2
            nc.scalar.dma_start(out=sidx_t[:, :SH], in_=sidx_d[:, :SH])
            nc.scalar.dma_start(out=sidx_t[:, SH:], in_=sidx_d[:, SH:])
            zchunk(nc.sync, 4)
            zchunk(nc.sync, 5)
            zchunk(nc.scalar, 6)
            zchunk(nc.scalar, 7)
            # zero blocks 1..7 of the output-reduction RS input (core 0's
            # received chunk is the only one that matters)
            nc.sync.dma_start(
                out=ago_in[NUM_GRAPHS:, :].rearrange(
                    "(j p) d -> p j d", p=NUM_GRAPHS),
                in_=zt[0:NUM_GRAPHS, : (NCORES - 1) * D].rearrange(
                    "p (j d) -> p j d", d=D),
            )

            # identity / BN constants / activation-table warmup — needed only
            # from phase D on, emitted after the edge-phase-critical work
            ident = const.tile([128, 128], f32)
            make_identity(nc, ident[:])
            ones64 = const.tile([D, 1], f32)
            nc.vector.memset(ones64[:], 1.0)
            epsc = const.tile([D, 1], f32)
            nc.vector.memset(epsc[:], BN_EPS)
            warm = const.tile([D, 1], f32)
            nc.scalar.activation(warm[:], epsc[:], mybir.ActivationFunctionType.Sqrt)
            nc.scalar.activation(warm[:], epsc[:], mybir.ActivationFunctionType.Relu)

            # --- R_k^T for layer 2 (loaded during the edge phase) ---
            rkt_t = persist.tile([128, NT, D], f32, name="rkt_t")
            nc.sync.dma_start(
                out=rkt_t[:], in_=rkt_d[:, :].rearrange("p (g d) -> p g d", d=D)
            )

            # --- edge phase: gather f32 / convert fp16 / scatter-add fp16 ---
            import concourse.mybir as mb

            if "noedge" not in ablate:
                pend = None
                off = 0
                for c, cnt in segs:
                    msg = msgp.tile([128, cfg.CAP // 128, D], f32, tag="msg",
                                    name="msg")
                    nc.gpsimd.dma_gather(
                        out_ap=msg[:, : cnt // 128, :],
                        in_ap=table1[0:SLP, :],
                        idxs_ap=gidx_t[:, off : off + cnt // 16],
                        num_idxs=cnt, num_idxs_reg=cnt, elem_size=D,
                        single_packet=False, queue_num=0,
                    )
                    msgh = msghp.tile([128, cfg.CAP // 128, D], f16, tag="msgh",
                                      name="msgh")
                    nc.vector.tensor_copy(
                        out=msgh[:, : cnt // 128, :], in_=msg[:, : cnt // 128, :]
                    )
                    if pend is not None:
                        nc.gpsimd.dma_scatter_add(*pend, elem_step=2 * D, single_packet=False, queue_num=0)
                    pend = (
                        acc[:, c * D : (c + 1) * D],
                        msgh[:, : cnt // 128, :],
                        sidx_t[:, off : off + cnt // 16],
                        cnt, cnt, D,
                    )
                    off += cnt // 16
                if pend is not None:
                    nc.gpsimd.dma_scatter_add(*pend, elem_step=2 * D, single_packet=False, queue_num=0)

            # --- ReduceScatter -> own reduced slice (fp16) ---
            if "nocc" not in ablate:
                cc("ReduceScatter", mybir.AluOpType.add, acc[:, :], rs_out[:, :])
            else:
                nc.sync.dma_start(out=rs_out[:, :], in_=acc[0 : cfg.SZJ, :])

            # warm the PE p-state during the collective (it idles otherwise,
            # and the first post-RS matmuls would run at the slow p-state)
            warm_ps = fpsum.tile([D, D], f32, tag="ps_c", name="warm_ps")
            for _ in range(24):
                nc.tensor.matmul(
                    out=warm_ps[:], lhsT=ident[:D, :D], rhs=ident[:D, :D],
                    start=True, stop=True,
                )

            # --- dense layer-1: z + self-loop, dinv scale, stats, W1 ---
            # z in two tiles loaded on SP and Act in parallel; the aggs
            # chunks start as soon as the first half lands
            z_a = persist.tile([128, HN, D], f16, name="z_a")
            z_b = persist.tile([128, NT - HN, D], f16, name="z_b")
            zsrc = rs_out[0 : cfg.SZ, :].rearrange("(p r) c -> p (r c)", p=128)
            HW_ = NT * D // 2
            nc.sync.dma_start(
                out=z_a[:].rearrange("p g d -> p (g d)"), in_=zsrc[:, :HW_]
            )
            nc.scalar.dma_start(
                out=z_b[:].rearrange("p g d -> p (g d)"), in_=zsrc[:, HW_:]
            )

            def z_sl(b0, bn):
                if b0 >= HN:
                    return z_b[:, b0 - HN : b0 - HN + bn, :]
                return z_a[:, b0 : b0 + bn, :]

            # keep the PE p-state warm through the z load so the stats
            # matmuls run at full clock
            for _ in range(10):
                nc.tensor.matmul(
                    out=warm_ps[:], lhsT=z_a[:, 0, :], rhs=z_a[:, 0, :],
                    start=True, stop=True,
                )
            aggs = persist.tile([128, NT, D + 1], f16, name="aggs")
            nc.vector.memset(aggs[:, :, D : D + 1], 1.0)
            ident16 = persist.tile([128, 128], f16, name="ident16")
            nc.vector.tensor_copy(out=ident16[:], in_=ident[:])
            w1s16 = persist.tile([D, D], f16, name="w1s16")
            nc.vector.tensor_copy(out=w1s16[:], in_=w1s[:])

            stats_ps = spsum.tile([D, D + 1], f32, name="stats_ps")
            hT_big = persist.tile([D, NT * 128], f32, name="hT_big")
            ND = NT if "noD" not in ablate else 1
            # pipeline the (z+xs)*dinv prep with the stats matmuls per chunk;
            # everything fp16 so the DVE runs in its packed 2x mode; chunks
            # never straddle the z_a/z_b boundary
            chunks = [(0, 10), (10, 20), (20, HN), (HN, 35), (35, 45), (45, ND)]
            chunks = [(a, min(b, ND)) for a, b in chunks if a < ND]
            for b0, b1 in chunks:
                bn = b1 - b0
                sl = slice(b0, b0 + bn)
                nc.vector.tensor_tensor(
                    out=aggs[:, sl, :D], in0=z_sl(b0, bn), in1=xs16[:, sl, :],
                    op=mybir.AluOpType.add,
                )
                nc.vector.tensor_tensor(
                    out=aggs[:, sl, :D], in0=aggs[:, sl, :D],
                    in1=dinv16[:, sl, :], op=mybir.AluOpType.mult,
                )
                for b in range(b0, b0 + bn):
                    nc.tensor.matmul(
                        out=stats_ps[:], lhsT=aggs[:, b, :D], rhs=aggs[:, b, :],
                        start=(b == 0), stop=(b == ND - 1),
                    )
            # stats reduction launched before the transposes/W1 matmuls so the
            # collective overlaps with PE work: replicate the local stats into
            # all 8 RS input blocks -> every core's RS output = global sum
            stg8 = persist.tile([D, NCORES, D + 1], f16, name="stg8")
            nc.vector.tensor_copy(
                out=stg8[:],
                in_=stats_ps[:].rearrange("p (o c) -> p o c", o=1)
                .to_broadcast([D, NCORES, D + 1]),
            )
            nc.sync.dma_start(
                out=ags_in[:, :].rearrange("(j p) c -> p j c", p=D), in_=stg8[:]
            )
            if "nocc" not in ablate:
                cc("ReduceScatter", mybir.AluOpType.add, ags_in[:, :], ags_out[:, :])
            else:
                nc.sync.dma_start(out=ags_out[:, :], in_=ags_in[0:D, :])

            for b0 in range(0, ND, 4):
                bn = min(4, ND - b0)
                tp_ps = wpsum.tile([D, 512], f16, tag="ps_a", name="tp_ps")
                for j in range(bn):
                    nc.tensor.transpose(
                        out=tp_ps[:, j * 128 : (j + 1) * 128],
                        in_=aggs[:, b0 + j, :D], identity=ident16[:],
                    )
                aggsT = work.tile([D, 512], f16, tag="aggsT", name="aggsT", bufs=2)
                nc.vector.tensor_copy(out=aggsT[:, : bn * 128], in_=tp_ps[:, : bn * 128])
                hT_ps = wpsum.tile([D, 512], f32, tag="ps_b", name="hT_ps")
                nc.tensor.matmul(
                    out=hT_ps[:, : bn * 128], lhsT=w1s16[:], rhs=aggsT[:, : bn * 128],
                    start=True, stop=True,
                )
                nc.vector.tensor_copy(
                    out=hT_big[:, b0 * 128 : (b0 + bn) * 128],
                    in_=hT_ps[:, : bn * 128],
                )

            st16 = persist.tile([D, D + 1], f16, name="st16")
            nc.scalar.dma_start(out=st16[:], in_=ags_out[:, :])
            st = st16[:]

            # --- BN scalar algebra ---
            q_ps = wpsum.tile([D, 1], f32, tag="ps_a", name="q_ps")
            nc.tensor.matmul(out=q_ps[:], lhsT=w1s16[:], rhs=st[:, D : D + 1], start=True, stop=True)
            mu = persist.tile([D, 1], f32, name="mu")
            nc.vector.tensor_scalar(
                out=mu[:], in0=q_ps[:], scalar1=1.0 / NN, scalar2=b1c[:],
                op0=mybir.AluOpType.mult, op1=mybir.AluOpType.add,
            )
            t1_ps = wpsum.tile([D, D], f32, tag="ps_b", name="t1_ps")
            nc.tensor.matmul(out=t1_ps[:], lhsT=st[:, :D], rhs=w1s16[:], start=True, stop=True)
            m_sb = work.tile([D, D], f32, tag="m_sb", name="m_sb")
            nc.vector.tensor_tensor(out=m_sb[:], in0=w1s[:], in1=t1_ps[:], op=mybir.AluOpType.mult)
            d_ps = wpsum.tile([D, 1], f32, tag="ps_b", name="d_ps")
            nc.tensor.matmul(out=d_ps[:], lhsT=m_sb[:], rhs=ones64[:], start=True, stop=True)

            var = persist.tile([D, 1], f32, name="var")
            nc.vector.tensor_scalar_mul(out=var[:], in0=d_ps[:], scalar1=1.0 / NN)
            t2 = work.tile([D, 1], f32, tag="t2", name="t2")
            nc.vector.tensor_scalar_mul(out=t2[:], in0=q_ps[:], scalar1=2.0 / NN)
            nc.vector.tensor_tensor(out=t2[:], in0=t2[:], in1=b1c[:], op=mybir.AluOpType.mult)
            nc.vector.tensor_tensor(out=var[:], in0=var[:], in1=t2[:], op=mybir.AluOpType.add)
            t3 = work.tile([D, 1], f32, tag="t3", name="t3")
            nc.vector.tensor_tensor(out=t3[:], in0=b1c[:], in1=b1c[:], op=mybir.AluOpType.mult)
            nc.vector.tensor_tensor(out=var[:], in0=var[:], in1=t3[:], op=mybir.AluOpType.add)
            t4 = work.tile([D, 1], f32, tag="t4", name="t4")
            nc.vector.tensor_tensor(out=t4[:], in0=mu[:], in1=mu[:], op=mybir.AluOpType.mult)
            nc.vector.tensor_tensor(out=var[:], in0=var[:], in1=t4[:], op=mybir.AluOpType.subtract)

            sd = work.tile([D, 1], f32, tag="sd", name="sd")
            nc.scalar.activation(sd[:], var[:], mb.ActivationFunctionType.Sqrt, bias=epsc[:])
            rstd = work.tile([D, 1], f32, tag="rstd", name="rstd")
            nc.vector.reciprocal(out=rstd[:], in_=sd[:])
            a_sb = persist.tile([D, 1], f32, name="a_sb")
            nc.vector.tensor_tensor(out=a_sb[:], in0=gac[:], in1=rstd[:], op=mybir.AluOpType.mult)
            c_sb = persist.tile([D, 1], f32, name="c_sb")
            t5 = work.tile([D, 1], f32, tag="t5", name="t5")
            nc.vector.tensor_tensor(out=t5[:], in0=mu[:], in1=a_sb[:], op=mybir.AluOpType.mult)
            nc.vector.tensor_tensor(out=c_sb[:], in0=bec[:], in1=t5[:], op=mybir.AluOpType.subtract)
            # hT tiles exclude the b1 bias; fold it into the BN offset:
            # relu(a*(h+b1) + c) = relu(a*h + (c + a*b1))
            t6 = work.tile([D, 1], f32, tag="t6", name="t6")
            nc.vector.tensor_tensor(out=t6[:], in0=a_sb[:], in1=b1c[:], op=mybir.AluOpType.mult)
            nc.vector.tensor_tensor(out=c_sb[:], in0=c_sb[:], in1=t6[:], op=mybir.AluOpType.add)

            # --- phase F: BN+ReLU, transpose back, dinv fold -> xs2;
            #     phase G interleaved: poolT += xs2_b^T @ R_b ---
            poolT_ps = spsum.tile([D, NUM_GRAPHS], f32, name="poolT_ps")
            xs2 = persist.tile([128, NT, D], f32, name="xs2")
            NF = NT if "noF" not in ablate else 0
            for b0 in range(0, NF, 8):
                bn = min(8, NF - b0)
                h1T = work.tile([D, 1024], f32, tag="h1T", name="h1T", bufs=3)
                nc.scalar.activation(
                    h1T[:, : bn * 128],
                    hT_big[:, b0 * 128 : (b0 + bn) * 128],
                    mb.ActivationFunctionType.Relu,
                    bias=c_sb[:], scale=a_sb[:],
                )
                nm_ps = fpsum.tile([128, 8 * D], f32, tag="ps_c", name="nm_ps")
                for j in range(bn):
                    nc.tensor.transpose(
                        out=nm_ps[:, j * D : (j + 1) * D],
                        in_=h1T[:, j * 128 : (j + 1) * 128],
                        identity=ident[:D, :D],
                    )
                nc.vector.tensor_tensor(
                    out=xs2[:, b0 : b0 + bn, :], in0=nm_ps[:, : bn * D].rearrange(
                        "p (g d) -> p g d", d=D),
                    in1=dinvs[:, b0 : b0 + bn].rearrange(
                        "p (g o) -> p g o", o=1).to_broadcast([128, bn, D]),
                    op=mybir.AluOpType.mult,
                )
                for j in range(bn):
                    b = b0 + j
                    nc.tensor.matmul(
                        out=poolT_ps[:], lhsT=xs2[:, b, :], rhs=rkt_t[:, b, :],
                        start=(b == 0), stop=(b == NF - 1),
                    )

            # --- output: pool @ W2 + p1^T b2; AllGather + local reduce ---
            poolT_sb = persist.tile([D, NUM_GRAPHS], f32, name="poolT_sb")
            nc.vector.tensor_copy(out=poolT_sb[:], in_=poolT_ps[:])
            out_ps = wpsum.tile([NUM_GRAPHS, D], f32, tag="ps_a", name="out_ps")
            nc.tensor.matmul(out=out_ps[:], lhsT=poolT_sb[:], rhs=w2s[:], start=True, stop=False)
            nc.tensor.matmul(out=out_ps[:], lhsT=p1s[:], rhs=b2r[:], start=False, stop=True)
            out_sb = persist.tile([NUM_GRAPHS, D], f16, name="out_sb")
            nc.vector.tensor_copy(out=out_sb[:], in_=out_ps[:])
            nc.sync.dma_start(out=ago_in[0:NUM_GRAPHS, :], in_=out_sb[:])
            if "nocc" not in ablate:
                cc("ReduceScatter", mybir.AluOpType.add, ago_in[:, :], ago_out[:, :])
            else:
                nc.sync.dma_start(out=ago_out[:, :], in_=ago_in[0:NUM_GRAPHS, :])
            og = persist.tile([NUM_GRAPHS, D], f16, name="og")
            nc.scalar.dma_start(out=og[:], in_=ago_out[:, :])
            og32 = persist.tile([NUM_GRAPHS, D], f32, name="og32")
            nc.vector.tensor_copy(out=og32[:], in_=og[:])
            nc.sync.dma_start(out=out_d[:, :], in_=og32[:])

    nc.compile()
    return nc


def _wrap16(v, n):
    """idx j at [j%16, j//16], replicated to 128 partitions (8 Q7 cores)."""
    assert v.shape[0] == n and n % 16 == 0
    t = v.astype(np.int16).reshape(n // 16, 16).T
    return np.tile(t, (8, 1))


def _bin_edges(gsrc, grow, nbins, nspill):
    """Rotation binning: occurrence o of dst row r -> bin (r + o) % nbins for
    o < nbins; higher occurrences spill into one extra bin per occurrence
    level (occurrence levels have unique rows by construction)."""
    order = np.argsort(grow, kind="stable")
    sd, ss = grow[order], gsrc[order]
    out = [(np.zeros(0, np.int64), np.zeros(0, np.int64))] * (nbins + nspill)
    if sd.shape[0] == 0:
        return out
    change = np.r_[True, sd[1:] != sd[:-1]]
    starts = np.flatnonzero(change)
    gid = np.cumsum(change) - 1
    occ = np.arange(sd.shape[0]) - starts[gid]
    assert int(occ.max()) < nbins + nspill, (int(occ.max()), nbins, nspill)
    b = np.where(occ < nbins, (sd + occ) % nbins, occ)
    return [(ss[b == i], sd[b == i]) for i in range(nbins + nspill)]


def prepare_inputs(cfg, x, edge_index, batch, W1, b1, gamma, beta, W2, b2):
    """Host-side index preprocessing + per-core input maps.  Fills cfg.seg."""
    SL, SLP, NT = cfg.SL, cfg.SLP, cfg.NT
    n = cfg.N

    x = np.ascontiguousarray(np.asarray(x, dtype=np.float32))
    src = np.asarray(edge_index[0], dtype=np.int64)
    dst = np.asarray(edge_index[1], dtype=np.int64)
    batch = np.asarray(batch, dtype=np.int64)
    W1 = np.asarray(W1, dtype=np.float32)
    b1 = np.asarray(b1, dtype=np.float32)
    gamma = np.asarray(gamma, dtype=np.float32)
    beta = np.asarray(beta, dtype=np.float32)
    W2 = np.asarray(W2, dtype=np.float32)
    b2 = np.asarray(b2, dtype=np.float32)

    deg = np.bincount(dst, minlength=n).astype(np.float32) + 1.0  # + self-loop
    dinv = (1.0 / np.sqrt(deg)).astype(np.float32)

    cnt = np.bincount(batch, minlength=NUM_GRAPHS).astype(np.float32)
    w_graph = 1.0 / np.maximum(cnt, 1.0)
    pd = w_graph[batch] * dinv          # P[batch[v], v] * dinv_v  per node

    owner = src // SL
    src_local = src - owner * SL

    # dst -> (parity, acc row): node (p, g) of core k ->
    # row k*SZJ + p*HT + g//2, column half g%2
    d_owner = dst // SL
    d_local = dst - d_owner * SL
    d_g = d_local // 128
    d_p = d_local - d_g * 128
    d_par = d_g % 2
    d_row = d_owner * cfg.SZJ + d_p * cfg.HT + d_g // 2

    per_core = [[None, None] for _ in range(NCORES)]
    for k in range(NCORES):
        sel = owner == k
        es, ed, ec = src_local[sel], d_row[sel], d_par[sel]
        for c in (0, 1):
            m = ec == c
            per_core[k][c] = (es[m], ed[m])

    # shared bin layout per parity
    seg, core_bins = [], [[] for _ in range(NCORES)]
    for c in (0, 1):
        counts = [per_core[k][c][0].shape[0] for k in range(NCORES)]
        mm = 1
        for k in range(NCORES):
            rows = per_core[k][c][1]
            if rows.shape[0]:
                mm = max(mm, int(np.bincount(rows).max()))
        nbins = max(-(-max(counts) // (cfg.CAP - 256)), 1)
        while True:
            nspill = max(mm - nbins, 0)
            allb = [
                _bin_edges(per_core[k][c][0], per_core[k][c][1], nbins, nspill)
                for k in range(NCORES)
            ]
            sizes = [
                ((max(allb[k][i][0].shape[0] for k in range(NCORES)) + 127)
                 // 128) * 128
                for i in range(nbins + nspill)
            ]
            if all(s <= cfg.CAP for s in sizes):
                break
            nbins += 1
        for i in range(nbins + nspill):
            if sizes[i] == 0:
                continue
            seg.append((c, sizes[i]))
            for k in range(NCORES):
                core_bins[k].append((c, sizes[i], allb[k][i]))

    cfg.seg = seg

    in_maps = []
    for k in range(NCORES):
        gl_parts, sc_parts = [], []
        for c, size, (es, ed) in core_bins[k]:
            m = es.shape[0]
            g = np.zeros(size, dtype=np.int64)
            s = np.full(size, cfg.TRASH[c], dtype=np.int64)  # dead row (dinv=0)
            order = np.argsort(es, kind="stable")  # src-sorted for locality
            g[:m] = es[order]
            s[:m] = ed[order]
            gl_parts.append(_wrap16(g, size))
            sc_parts.append(_wrap16(s, size))
        gidx = np.concatenate(gl_parts, axis=1)
        sidx = np.concatenate(sc_parts, axis=1)

        lo, hi = k * SL, min((k + 1) * SL, n)
        nsl = hi - lo
        xsl = np.zeros((SLP, D), dtype=np.float32)
        xsl[:nsl] = x[lo:hi]
        xsl_pm = np.ascontiguousarray(
            xsl.reshape(NT, 128, D).transpose(1, 0, 2).reshape(128, NT * D)
        )
        dsl = np.zeros(SLP, dtype=np.float32)
        dsl[:nsl] = dinv[lo:hi]
        dinv_in = dsl.reshape(NT, 128).T.copy()

        # R_k^T [SLP, 64]: R_kT[u, g] = sum_{edges (k*SL+u) -> w} P[g,w]*dinv_w
        #                             + P[g, k*SL+u]*dinv_{k*SL+u}
        sel = owner == k
        rkt = np.zeros((SLP, NUM_GRAPHS), dtype=np.float32)
        np.add.at(rkt, (src_local[sel], batch[dst[sel]]), pd[dst[sel]])
        rkt[np.arange(nsl), batch[lo:hi]] += pd[lo:hi]
        rkt_pm = np.ascontiguousarray(
            rkt.reshape(NT, 128, NUM_GRAPHS).transpose(1, 0, 2).reshape(128, -1)
        )

        p1 = np.zeros((1, NUM_GRAPHS), dtype=np.float32)
        np.add.at(p1[0], batch[lo:hi], w_graph[batch[lo:hi]])

        in_maps.append({
            "xsl": xsl_pm,
            "dinv_in": dinv_in,
            "gidx": np.ascontiguousarray(gidx),
            "sidx": np.ascontiguousarray(sidx),
            "rkt": rkt_pm,
            "p1": p1,
            "w1": W1,
            "b1": b1.reshape(D, 1),
            "ga": gamma.reshape(D, 1),
            "be": beta.reshape(D, 1),
            "w2": W2,
            "b2": b2.reshape(1, D),
        })
    return in_maps


def kernel(x, edge_index, batch, W1, b1, gamma, beta, W2, b2):
    global LAST_EXEC_TIME_NS
    from concourse.bass_utils import run_bass_kernel_spmd

    cfg = Cfg(N_NODES, N_NODES // NCORES)
    in_maps = prepare_inputs(cfg, x, edge_index, batch, W1, b1, gamma, beta, W2, b2)

    key = (cfg.N, cfg.SL, tuple(cfg.seg))
    if key not in _NC_CACHE:
        _NC_CACHE[key] = build(cfg)
    nc = _NC_CACHE[key]
    global _LAST_IN_MAPS
    _LAST_IN_MAPS = in_maps

    trace = bool(int(os.environ.get("BASS_GNN_TRACE", "0")))
    if trace:
        try:
            res = run_bass_kernel_spmd(nc, in_maps, list(range(NCORES)), trace=True)
        except Exception:
            res = run_bass_kernel_spmd(nc, in_maps, list(range(NCORES)), trace=False)
    else:
        res = run_bass_kernel_spmd(nc, in_maps, list(range(NCORES)), trace=False)
    LAST_EXEC_TIME_NS = res.exec_time_ns
    return np.asarray(res.results[0]["out"], dtype=np.float32)


def modeled_time_ns(x=None, edge_index=None, **kw):
    """Cost-model execution time (MultiCoreSim, mocked collectives) for the
    current cached program; used when NTFF tracing is unavailable."""
    if not _NC_CACHE:
        return None
    nc = next(iter(_NC_CACHE.values()))
    ins = _LAST_IN_MAPS
    if ins is None:
        return None
    from concourse.bass_interp import MultiCoreSim

    sim = MultiCoreSim(nc, 2, debug_mock_collectives_without_correctness=True)
    for i, core in sim.cores.items():
        for name, val in ins[i].items():
            core.tensor(name)[:] = val
    sim.simulate()
    return int(sim.global_time)
